# revision 1
# baseline (speedup 1.0000x reference)
"""Trainium2 Bass kernel for DevConv-style GNN message passing.

Reference computation:
    rel_t = (x[row] - x[col]) @ W_theta.T          # [E, 128]
    aggr  = segment_max(rel_t, row, N)             # [N, 128], empty -> 0
    out   = aggr @ W_phi.T                         # [N, 128]

Key reformulation: with y = x @ W_theta.T, within a segment (fixed row d)
    max_e (y[d] - y[col_e]) = y[d] - min_e y[col_e]     (per channel)
so the per-edge matmul disappears and only ONE gather per edge (y[col]) is
needed, followed by a segmented min.

The gather is bound by SWDGE descriptor generation on the Pool engine
(~one descriptor per gathered row), so runtime ~ padded-slot count. Two
host-side layout optimizations minimize it:
  * balanced chunk coloring: dma_gather indices are int16, so the y table
    is split into 4 chunks of 25088 rows. A greedy quota-balanced coloring
    assigns each node to a chunk so every destination's edges spread
    ~deg/4 per chunk (vs Binomial(deg,1/4) for a range split).
  * count-vector tiling: destinations are grouped into 128-row tiles (and
    striped across the 8 cores) sorted by their per-chunk count vectors,
    so the per-tile-per-chunk max count (the padded rect width) is tight.
Together padding inflation drops ~1.95x -> ~1.37x.

Per core:
  Phase A: y = x @ W_theta.T for ALL nodes (bf16) into the chunked HBM
           table (+1 sentinel row of +3e38 per chunk); x arrives already
           permuted into table order so stores are contiguous.
  Phase B: per 128-dest tile, all chunks' dma_gathers land in ONE SBUF
           rect [128 x sum_k B[t,k] slots] (pad slots point at the chunk
           sentinel); a single strided DVE tensor_reduce(min) folds the
           rect -> m[d] = min_e y[col_e]. (One 1-input reduce per tile
           instead of a 2-input fold tree keeps the DVE out of 2-port
           perf mode, which would lock GpSimd out of the shared SBUF port
           and stall SWDGE descriptor generation.)
  Phase C: aggr = y_own - m  (y_own computed on-chip from x_own),
           out_tile = aggr @ W_phi.T via PE transpose + matmul.
Host un-permutes the concatenated core outputs and zeroes empty nodes.
"""
import sys
import os

sys.path.insert(0, "/opt/trn_rl_repo")

from contextlib import ExitStack
from dataclasses import dataclass

import numpy as np
import ml_dtypes

import concourse.bass as bass
import concourse.tile as tile
from concourse import bacc, mybir
from concourse.masks import make_identity

import time

import jax
from jax.sharding import Mesh, PartitionSpec
from jax.experimental.shard_map import shard_map

from concourse.bass2jax import (
    _bass_exec_p, install_neuronx_cc_hook, partition_id_tensor)


class BassRunner:
    """Keeps a jitted PJRT executable for a Bass program so it can be run
    repeatedly on device-resident inputs (for wall-clock timing)."""

    def __init__(self, nc, n_cores: int):
        install_neuronx_cc_hook()
        self.nc = nc
        self.n_cores = n_cores
        partition_name = nc.partition_id_tensor.name if nc.partition_id_tensor else None
        in_names, out_names, out_avals = [], [], []
        for alloc in nc.m.functions[0].allocations:
            if not isinstance(alloc, mybir.MemoryLocationSet):
                continue
            name = alloc.memorylocations[0].name
            if alloc.kind == "ExternalInput":
                if name != partition_name:
                    in_names.append(name)
            elif alloc.kind == "ExternalOutput":
                out_names.append(name)
                out_avals.append(jax.core.ShapedArray(
                    tuple(alloc.tensor_shape), mybir.dt.np(alloc.dtype)))
        self.in_names, self.out_names, self.out_avals = in_names, out_names, out_avals
        self.n_params = len(in_names)
        all_in_names = list(in_names) + list(out_names)
        if partition_name is not None:
            all_in_names.append(partition_name)

        def _body(*args):
            operands = list(args)
            if partition_name is not None:
                operands.append(partition_id_tensor())
            outs = _bass_exec_p.bind(
                *operands,
                out_avals=tuple(out_avals),
                in_names=tuple(all_in_names),
                out_names=tuple(out_names),
                lowering_input_output_aliases=(),
                sim_require_finite=True,
                sim_require_nnan=True,
                nc=nc,
            )
            return tuple(outs)

        devices = jax.devices()[:n_cores]
        self.mesh = Mesh(np.asarray(devices), ("core",))
        n_outs = len(out_names)
        in_specs = (PartitionSpec("core"),) * (self.n_params + n_outs)
        out_specs = (PartitionSpec("core"),) * n_outs
        self.fn = jax.jit(
            shard_map(_body, mesh=self.mesh, in_specs=in_specs,
                      out_specs=out_specs, check_rep=False),
            keep_unused=True,
        )
        self._dev_args = None

    def prepare(self, in_maps):
        assert len(in_maps) == self.n_cores
        concat_in = [
            np.concatenate([np.asarray(in_maps[c][n]) for c in range(self.n_cores)],
                           axis=0)
            for n in self.in_names
        ]
        concat_zeros = [
            np.zeros((self.n_cores * a.shape[0], *a.shape[1:]), a.dtype)
            for a in self.out_avals
        ]
        sharding = jax.sharding.NamedSharding(self.mesh, PartitionSpec("core"))
        self._dev_args = [jax.device_put(v, sharding) for v in concat_in + concat_zeros]
        return self

    def run(self):
        outs = self.fn(*self._dev_args)
        jax.block_until_ready(outs)
        return outs

    def results(self, outs):
        return [
            {n: np.asarray(outs[i]).reshape(self.n_cores, *self.out_avals[i].shape)[c]
             for i, n in enumerate(self.out_names)}
            for c in range(self.n_cores)
        ]

    def time_ns(self, iters=5, warmup=2):
        for _ in range(warmup):
            self.run()
        ts = []
        for _ in range(iters):
            t0 = time.perf_counter()
            self.run()
            ts.append((time.perf_counter() - t0) * 1e9)
        return min(ts)


CH = 128
F32 = mybir.dt.float32
BF16 = mybir.dt.bfloat16
I16 = mybir.dt.int16
SENT_VAL = 3.0e38
IDX_PER_INST = 1024          # dma_gather crashes >= 2048 idx/instruction
BLK_PER_INST = IDX_PER_INST // 128


@dataclass(frozen=True)
class Cfg:
    N: int = 100_000
    E: int = 3_200_000
    n_cores: int = 8
    chunk_real: int = 25_088     # table rows per chunk (512-aligned, < int16 max)
    idx_per_inst: int = 1024
    balanced: bool = True        # balanced chunk coloring + count-vector tiling

    @property
    def n_chunks(self):
        return (self.N + self.chunk_real - 1) // self.chunk_real

    @property
    def chunk_stride(self):
        return self.chunk_real + 1

    @property
    def npc(self):
        assert self.N % self.n_cores == 0
        return self.N // self.n_cores

    @property
    def tiles(self):
        return (self.npc + 127) // 128


def _wrap16(seg: np.ndarray) -> np.ndarray:
    """Per-instruction idx wrap: flat [n] -> [128, n//16]; idx i at
    (partition i%16, col i//16), replicated across the 8 gpsimd groups."""
    n = seg.shape[-1]
    w = seg.reshape(*seg.shape[:-1], n // 16, 16)
    w = np.swapaxes(w, -1, -2)                       # [..., 16, n//16]
    return np.tile(w, (1,) * (seg.ndim - 1) + (8, 1))


def _color_chunks(row, col, deg, N, NK, cap):
    """Greedy quota-balanced assignment of cols to NK chunks; returns
    (chunk_of, cnt_dk) where cnt_dk[d,k] = per-dest per-chunk edge count."""
    o = np.argsort(col, kind="stable")
    dest_s = row[o]
    starts = np.searchsorted(col[o], np.arange(N + 1))
    quota = -(-deg // NK)
    cnt_dk = np.zeros((N, NK), np.int32)
    chunk_of = np.full(N, -1, np.int64)
    chunk_sz = np.zeros(NK, np.int64)
    col_order = np.argsort(-(starts[1:] - starts[:-1]), kind="stable")
    for _ in range(2):
        for c in col_order:
            ds = dest_s[starts[c]: starts[c + 1]]
            kprev = chunk_of[c]
            if kprev >= 0:
                if len(ds):
                    cnt_dk[ds, kprev] -= 1
                chunk_sz[kprev] -= 1
            if len(ds) == 0:
                k = int(np.argmin(chunk_sz))
            else:
                cc = cnt_dk[ds]
                over = np.maximum(0, cc + 1 - quota[ds][:, None])
                sc = (over * 1000.0 + cc).sum(axis=0).astype(np.float64)
                sc += chunk_sz * 1e-4
                sc[chunk_sz >= cap] = 1e18
                k = int(np.argmin(sc))
                cnt_dk[ds, k] += 1
            chunk_of[c] = k
            chunk_sz[k] += 1
    return chunk_of, cnt_dk


def prep(x, edge_index, cfg: Cfg):
    """Host-side data prep. Returns (plan, per-core inputs, unpermute info)."""
    N, E, NC = cfg.N, cfg.E, cfg.n_cores
    CR, NK, T = cfg.chunk_real, cfg.n_chunks, cfg.tiles
    row = np.asarray(edge_index[0], dtype=np.int64)
    col = np.asarray(edge_index[1], dtype=np.int64)

    deg = np.bincount(row, minlength=N)
    x_np0 = np.asarray(x, dtype=np.float32)
    if cfg.balanced:
        chunk_of, cnt_dk = _color_chunks(row, col, deg, N, NK, CR)
        # rank within chunk
        oc = np.argsort(chunk_of, kind="stable")
        rank_of = np.empty(N, np.int64)
        csz = np.bincount(chunk_of, minlength=NK)
        cstart = np.concatenate([[0], np.cumsum(csz)])
        rank_of[oc] = np.arange(N) - cstart[chunk_of[oc]]
        # dest order: group similar per-chunk count vectors into tiles
        order = np.arange(N)
        for k in range(NK):
            order = order[np.argsort(-cnt_dk[order, k], kind="stable")]
        # x permuted into table layout [NK*CR, CH]
        x_perm = np.zeros((NK * CR, x_np0.shape[1]), np.float32)
        x_perm[chunk_of * CR + rank_of] = x_np0
    else:
        order = np.argsort(-deg, kind="stable")      # node ids by desc degree
        x_perm = x_np0
    core_of = np.empty(N, np.int64)
    pos_of = np.empty(N, np.int64)
    r = np.arange(N)
    core_of[order] = r % NC
    pos_of[order] = r // NC

    ec = core_of[row]
    ep = pos_of[row]
    if cfg.balanced:
        ek = chunk_of[col]
        elocal = rank_of[col].astype(np.int16)
    else:
        ek = col // CR
        elocal = (col - ek * CR).astype(np.int16)
    et = ep // 128
    ed = ep % 128

    # per-(core,tile,chunk,node) counts and within-group slot index j
    key = ((ec * T + et) * NK + ek) * 128 + ed
    o = np.argsort(key, kind="stable")
    ks = key[o]
    first = np.r_[True, ks[1:] != ks[:-1]]
    run_id = np.cumsum(first) - 1
    run_start = np.flatnonzero(first)
    j = np.arange(E) - run_start[run_id]

    cnt = np.bincount(key, minlength=NC * T * NK * 128).reshape(NC, T, NK, 128)
    B = cnt.max(axis=(0, 3)).astype(np.int64)        # [T, NK] shared structure

    Bf = B.reshape(-1)
    off = np.concatenate([[0], np.cumsum(Bf * 128)])  # slot offset per (t,k)
    total_slots = int(off[-1])

    idx_all = np.full((NC, total_slots), CR, np.int16)   # sentinel local idx
    tk = et[o] * NK + ek[o]
    pos_in = off[tk] + j * 128 + ed[o]
    idx_all[ec[o], pos_in] = elocal[o]

    # split into gather instructions and build wrapped idx input
    blk_per_inst = cfg.idx_per_inst // 128
    insts = []           # (t, k, g0blk, nblk, col_off)
    tile_cols = []       # per tile: (col_start, col_end)
    wsegs = []
    col_off = 0
    for t in range(T):
        t_start = col_off
        for k in range(NK):
            btk = int(B[t, k])
            base = int(off[t * NK + k])
            for g0 in range(0, btk, blk_per_inst):
                nb = min(blk_per_inst, btk - g0)
                n_i = nb * 128
                seg = idx_all[:, base + g0 * 128: base + g0 * 128 + n_i]
                wsegs.append(_wrap16(seg))
                insts.append((t, k, g0, nb, col_off))
                col_off += n_i // 16
        tile_cols.append((t_start, col_off))
    idxw = np.concatenate(wsegs, axis=2) if wsegs else np.zeros((NC, 128, 0), np.int16)
    W_total = idxw.shape[2]

    # per-core x_own in pos order, padded to T*128 rows
    own_nodes = np.empty((NC, cfg.npc), np.int64)
    own_nodes[core_of[order], pos_of[order]] = order  # own_nodes[c, p] = node id
    x_np = np.asarray(x, dtype=np.float32)
    x_own = np.zeros((NC, T * 128, CH), np.float32)
    x_own[:, : cfg.npc] = x_np[own_nodes]

    plan = dict(cfg=cfg, B=B, insts=insts, tile_cols=tile_cols, W_total=W_total)
    return plan, idxw, x_own, own_nodes, deg, x_perm


def build_program(plan, reps=1, phases="abc", exp=None):
    exp = {**dict(gather="on", folds="on", queue="rot", fold_mode="reduce"),
           **(exp or {})}
    cfg: Cfg = plan["cfg"]
    N, NK, CR, T = cfg.N, cfg.n_chunks, cfg.chunk_real, cfg.tiles
    CS = cfg.chunk_stride
    B, insts, tile_cols, W_total = (
        plan["B"], plan["insts"], plan["tile_cols"], plan["W_total"])

    nc = bacc.Bacc(None, target_bir_lowering=False, num_swdge_queues=4)
    NA = NK * CR if cfg.balanced else N      # phase-A row count (table layout)
    x_full = nc.declare_dram_parameter("x_full", [NA, CH], F32, isOutput=False)
    x_own = nc.declare_dram_parameter("x_own", [T * 128, CH], F32, isOutput=False)
    wth = nc.declare_dram_parameter("w_theta_t", [CH, CH], F32, isOutput=False)
    wph = nc.declare_dram_parameter("w_phi_t", [CH, CH], F32, isOutput=False)
    idxw = nc.declare_dram_parameter("idxw", [128, max(W_total, 16)], I16, isOutput=False)
    out = nc.declare_dram_parameter("out", [T * 128, CH], F32, isOutput=True)

    qc = [0]  # gather queue rotation

    with tile.TileContext(nc) as tc:
        with ExitStack() as ctx:
            consts = ctx.enter_context(tc.tile_pool(name="consts", bufs=1))
            dram = ctx.enter_context(tc.tile_pool(name="dram", bufs=1, space="DRAM"))
            ax = ctx.enter_context(tc.tile_pool(name="ax", bufs=2))
            axT = ctx.enter_context(tc.tile_pool(name="axT", bufs=2))
            ay = ctx.enter_context(tc.tile_pool(name="ay", bufs=2))
            ps_t = ctx.enter_context(tc.tile_pool(name="ps_t", bufs=2, space="PSUM"))
            ps_y = ctx.enter_context(tc.tile_pool(name="ps_y", bufs=2, space="PSUM"))
            ps_c = ctx.enter_context(tc.tile_pool(name="ps_c", bufs=2, space="PSUM"))
            gidx = ctx.enter_context(tc.tile_pool(name="gidx", bufs=3))
            gdst = ctx.enter_context(tc.tile_pool(name="gdst", bufs=2))
            fold = ctx.enter_context(tc.tile_pool(name="fold", bufs=2))
            fin = ctx.enter_context(tc.tile_pool(name="fin", bufs=2))

            y_aug = dram.tile([NK * CS, CH], BF16)

            ident = consts.tile([128, 128], F32)
            make_identity(nc, ident[:])
            wth_sb = consts.tile([CH, CH], F32)
            nc.sync.dma_start(out=wth_sb[:], in_=wth[:])
            wph_sb = consts.tile([CH, CH], F32)
            nc.sync.dma_start(out=wph_sb[:], in_=wph[:])
            y_own_sb = consts.tile([128, T * 128], F32)
            probe = consts.tile([128, CH], BF16)
            nc.gpsimd.memset(probe[:], 0.0)
            cst128 = consts.tile([128, CH], BF16)
            nc.gpsimd.memset(cst128[:], 1.0)
            sent = consts.tile([1, CH], BF16)
            nc.gpsimd.memset(sent[:], SENT_VAL)
            for k in range(NK):
                nc.sync.dma_start(out=y_aug[k * CS + CR: k * CS + CR + 1, :], in_=sent[:])

            A_MODE = os.environ.get("A_MODE", "full")
            PS_BUFS = int(os.environ.get("PS_BUFS", "2"))
            # ---------------- Phase A: y_aug = (x @ W_theta.T).bf16 ----------
            def emit_group(src, n0, gn, dst):
                """Process rows [n0, n0+gn) of src -> y into dst.
                dst = ("aug",) writes y_aug rows (with chunk-boundary split),
                dst = ("own",) writes y_own_sb cols."""
                nt = (gn + 127) // 128
                xg = ax.tile([128, nt * 128], F32, tag="xg",
                             bufs=int(os.environ.get("XG_BUFS", "2")))
                xg3 = xg[:].rearrange("p (i c) -> p i c", c=CH)
                load_eng = nc.gpsimd if A_MODE == "dma3" else nc.sync
                if gn % 128 == 0:
                    load_eng.dma_start(
                        out=xg3[:, :nt, :],
                        in_=src[n0: n0 + gn, :].rearrange("(i p) c -> p i c", p=128))
                else:
                    for i in range(nt):
                        rn = min(128, gn - i * 128)
                        nc.sync.dma_start(
                            out=xg3[:rn, i, :],
                            in_=src[n0 + i * 128: n0 + i * 128 + rn, :])
                if A_MODE.startswith("dma") and dst == "aug":
                    # dma : load->store dep, both on sync
                    # dma2: stores only dep-free (loads still emitted)
                    # dma3: load on gpsimd, stores dep on load, on sync
                    # dma5: loads only (no stores)
                    if gn % 128 == 0:
                        for i in range(nt):
                            r0 = n0 + i * 128
                            kb = r0 // CR
                            if A_MODE == "dma5":
                                continue
                            src_ap = (cst128[:] if A_MODE == "dma2"
                                      else xg3[:, i, :CH // 2].bitcast(BF16))
                            nc.sync.dma_start(
                                out=y_aug[r0 + kb: r0 + kb + 128, :], in_=src_ap)
                    return
                pt = ps_t.tile([128, nt * 128], F32, tag="pt", bufs=PS_BUFS)
                for i in range(nt):
                    rn = min(128, gn - i * 128)
                    nc.tensor.transpose(
                        out=pt[:, i * 128: i * 128 + rn],
                        in_=xg3[:rn, i, :],
                        identity=ident[:rn, :rn])
                xT = axT.tile([128, nt * 128], F32, tag="xT")
                if exp.get("fold_mode") == "reduce":
                    nc.scalar.copy(out=xT[:, : nt * 128], in_=pt[:, : nt * 128])
                else:
                    nc.vector.tensor_copy(out=xT[:, : nt * 128], in_=pt[:, : nt * 128])
                if A_MODE == "nomm" and dst == "aug":
                    for i in range(nt):
                        r0 = n0 + i * 128
                        kb = r0 // CR
                        nc.sync.dma_start(
                            out=y_aug[r0 + kb: r0 + kb + 128, :],
                            in_=xT[:, i * 128: i * 128 + 128][:, :CH // 2].bitcast(BF16))
                    return
                py = ps_y.tile([128, nt * 128], F32, tag="py", bufs=PS_BUFS)
                for i in range(nt):
                    rn = min(128, gn - i * 128)
                    nc.tensor.matmul(
                        out=py[:rn, i * 128: (i + 1) * 128],
                        lhsT=xT[:, i * 128: i * 128 + rn],
                        rhs=wth_sb[:],
                        start=True, stop=True)
                if dst == "own":
                    nc.scalar.copy(
                        out=y_own_sb[:, n0: n0 + nt * 128], in_=py[:, : nt * 128])
                    return
                yg = ay.tile([128, nt * 128], BF16, tag="yg")
                copy2 = nc.vector.tensor_copy if A_MODE == "dvecopy" else nc.scalar.copy
                if gn % 128 == 0:
                    copy2(out=yg[:, : gn], in_=py[:, : gn])
                else:
                    for i in range(nt):
                        rn = min(128, gn - i * 128)
                        copy2(
                            out=yg[:rn, i * 128: (i + 1) * 128],
                            in_=py[:rn, i * 128: (i + 1) * 128])
                yg3 = yg[:].rearrange("p (i c) -> p i c", c=CH)
                # write y rows n -> aug rows n + n // CR, splitting at tile level
                for i in range(nt):
                    r0 = n0 + i * 128
                    rn = min(128, gn - i * 128)
                    kb = r0 // CR
                    ke = (r0 + rn - 1) // CR
                    if kb == ke:
                        nc.sync.dma_start(
                            out=y_aug[r0 + kb: r0 + kb + rn, :], in_=yg3[:rn, i, :])
                    else:
                        split = (kb + 1) * CR - r0       # rows before boundary
                        nc.sync.dma_start(
                            out=y_aug[r0 + kb: r0 + kb + split, :],
                            in_=yg3[:split, i, :])
                        nc.sync.dma_start(
                            out=y_aug[r0 + split + ke: r0 + ke + rn, :],
                            in_=yg3[split:rn, i, :])

            by_tile = {}
            for (t, k, g0, nb, coff) in insts:
                by_tile.setdefault(t, []).append((k, g0, nb, coff))

            for _rep in range(reps):
              for n0 in range(0, NA, 512):
                emit_group(x_full, n0, min(512, NA - n0), "aug")
              for n0 in range(0, T * 128, 512):
                emit_group(x_own, n0, min(512, T * 128 - n0), "own")

              # ---------------- Phase B + C per tile ---------------------------
              for t in range(T):
                 c0, c1 = tile_cols[t]
                 it = gidx.tile([128, max(c1 - c0, 16)], I16, tag="it")
                 if c1 > c0:
                     nc.sync.dma_start(out=it[:, : c1 - c0], in_=idxw[:, c0:c1])
                 if exp.get("fold_mode") == "reduce":
                     kws = [k for k in range(NK) if int(B[t, k]) > 0]
                     koff = {}
                     wt = 0
                     for k in kws:
                         koff[k] = wt
                         wt += int(B[t, k])
                     if wt > 0:
                         dk = gdst.tile([128, wt * CH], BF16, tag="gr")
                         dk3 = dk[:].rearrange("p (b c) -> p b c", c=CH)
                     for (k, g0, nb, coff) in by_tile.get(t, []):
                         if exp["gather"] == "off":
                             break
                         n_i = nb * 128
                         col0 = koff[k] + g0
                         nc.gpsimd.dma_gather(
                             out_ap=dk3[:, col0: col0 + nb, :],
                             in_ap=y_aug[k * CS: (k + 1) * CS, :],
                             idxs_ap=it[:, coff - c0: coff - c0 + n_i // 16],
                             num_idxs=n_i,
                             num_idxs_reg=n_i,
                             elem_size=CH,
                             queue_num=(qc[0] % 4) if exp["queue"] == "rot" else 0,
                         )
                         qc[0] += 1
                     m = fin.tile([128, CH], F32, tag="m")
                     if wt == 0:
                         nc.gpsimd.memset(m[:], SENT_VAL)
                     else:
                         dkT = dk[:].rearrange("p (b c) -> p c b", c=CH)
                         nc.vector.tensor_reduce(
                             out=m[:], in_=dkT, axis=mybir.AxisListType.X,
                             op=mybir.AluOpType.min)
                     aggr = fin.tile([128, CH], F32, tag="aggr")
                     nc.vector.tensor_sub(
                         out=aggr[:], in0=y_own_sb[:, t * 128: (t + 1) * 128],
                         in1=m[:])
                     ptr = ps_c.tile([128, CH], F32, tag="ctr")
                     nc.tensor.transpose(out=ptr[:], in_=aggr[:], identity=ident[:])
                     aggrT = fin.tile([128, CH], F32, tag="aggrT")
                     nc.scalar.copy(out=aggrT[:], in_=ptr[:])
                     po = ps_c.tile([128, CH], F32, tag="cmm")
                     nc.tensor.matmul(out=po[:], lhsT=aggrT[:], rhs=wph_sb[:],
                                      start=True, stop=True)
                     osb = fin.tile([128, CH], F32, tag="osb")
                     nc.scalar.copy(out=osb[:], in_=po[:])
                     nc.sync.dma_start(out=out[t * 128: (t + 1) * 128, :],
                                       in_=osb[:])
                     continue
                 dks = {}
                 for k in range(NK):
                     btk = int(B[t, k])
                     if btk == 0:
                         continue
                     dks[k] = gdst.tile([128, btk * CH], BF16, tag=f"g{k}", name=f"dk{k}")
                 for (k, g0, nb, coff) in by_tile.get(t, []):
                     if exp["gather"] == "off":
                         break
                     dk3 = dks[k][:].rearrange("p (b c) -> p b c", c=CH)
                     n_i = nb * 128
                     ndup = 2 if exp["gather"] == "dup" else 1
                     for di in range(ndup):
                         if di == 0:
                             dst = dk3[:, g0: g0 + nb, :]
                         else:
                             ddup = gdst.tile(
                                 [128, (cfg.idx_per_inst // 128) * CH], BF16,
                                 tag="gdup", bufs=2)
                             dst = ddup[:].rearrange(
                                 "p (b c) -> p b c", c=CH)[:, :nb, :]
                         nc.gpsimd.dma_gather(
                             out_ap=dst,
                             in_ap=y_aug[k * CS: (k + 1) * CS, :],
                             idxs_ap=it[:, coff - c0: coff - c0 + n_i // 16],
                             num_idxs=n_i,
                             num_idxs_reg=n_i,
                             elem_size=CH,
                             queue_num=(qc[0] % 4) if exp["queue"] == "rot" else 0,
                         )
                         qc[0] += 1
                 # fold each chunk's rect down to one [128, CH] min
                 mks = []
                 if exp["folds"] == "off":
                     m = fin.tile([128, CH], F32, tag="m")
                     nc.gpsimd.memset(m[:], SENT_VAL)
                     aggr = fin.tile([128, CH], F32, tag="aggr")
                     nc.vector.tensor_sub(
                         out=aggr[:], in0=y_own_sb[:, t * 128: (t + 1) * 128],
                         in1=m[:])
                     ptr = ps_c.tile([128, CH], F32, tag="ctr")
                     nc.tensor.transpose(out=ptr[:], in_=aggr[:], identity=ident[:])
                     aggrT = fin.tile([128, CH], F32, tag="aggrT")
                     nc.vector.tensor_copy(out=aggrT[:], in_=ptr[:])
                     po = ps_c.tile([128, CH], F32, tag="cmm")
                     nc.tensor.matmul(out=po[:], lhsT=aggrT[:], rhs=wph_sb[:],
                                      start=True, stop=True)
                     osb = fin.tile([128, CH], F32, tag="osb")
                     nc.scalar.copy(out=osb[:], in_=po[:])
                     nc.sync.dma_start(out=out[t * 128: (t + 1) * 128, :], in_=osb[:])
                     continue
                 for k in range(NK):
                     if k not in dks:
                         continue
                     cur = dks[k]
                     nb = int(B[t, k])
                     while nb > 1:
                         half = (nb + 1) // 2
                         nxt = fold.tile([128, half * CH], BF16, tag=f"f{k}", bufs=3)
                         nc.vector.tensor_tensor(
                             out=nxt[:, : half * CH],
                             in0=cur[:, : half * CH],
                             in1=cur[:, (nb - half) * CH: nb * CH],
                             op=mybir.AluOpType.min)
                         cur, nb = nxt, half
                     mks.append(cur)
                 m = fin.tile([128, CH], F32, tag="m")
                 if len(mks) == 0:
                     nc.gpsimd.memset(m[:], SENT_VAL)
                 elif len(mks) == 1:
                     nc.vector.tensor_copy(out=m[:], in_=mks[0][:, :CH])
                 else:
                     # sequential accumulate with alternating tags (max 2 live)
                     acc = mks[0]
                     for i in range(1, len(mks) - 1):
                         mm = fold.tile([128, CH], BF16, tag=f"mrg{i % 2}")
                         nc.vector.tensor_tensor(
                             out=mm[:], in0=acc[:, :CH], in1=mks[i][:, :CH],
                             op=mybir.AluOpType.min)
                         acc = mm
                     nc.vector.tensor_tensor(
                         out=m[:], in0=acc[:, :CH], in1=mks[-1][:, :CH],
                         op=mybir.AluOpType.min)
                 # aggr = y_own - m ; out_tile = aggr @ W_phi.T
                 aggr = fin.tile([128, CH], F32, tag="aggr")
                 nc.vector.tensor_sub(
                     out=aggr[:], in0=y_own_sb[:, t * 128: (t + 1) * 128], in1=m[:])
                 ptr = ps_c.tile([128, CH], F32, tag="ctr")
                 nc.tensor.transpose(out=ptr[:], in_=aggr[:], identity=ident[:])
                 aggrT = fin.tile([128, CH], F32, tag="aggrT")
                 nc.vector.tensor_copy(out=aggrT[:], in_=ptr[:])
                 po = ps_c.tile([128, CH], F32, tag="cmm")
                 nc.tensor.matmul(out=po[:], lhsT=aggrT[:], rhs=wph_sb[:],
                                  start=True, stop=True)
                 osb = fin.tile([128, CH], F32, tag="osb")
                 nc.scalar.copy(out=osb[:], in_=po[:])
                 nc.sync.dma_start(out=out[t * 128: (t + 1) * 128, :], in_=osb[:])

            if phases != "abc":
                fillz = consts.tile([128, CH], F32)
                nc.vector.tensor_copy(out=fillz[:], in_=probe[:])
                for t in range(T):
                    nc.sync.dma_start(out=out[t * 128: (t + 1) * 128, :], in_=fillz[:])
    nc.compile()
    return nc


_CACHE = {}


def _get_runner_and_plan(x, edge_index, cfg: Cfg, reps=1, phases="abc", exp=None):
    plan, idxw, x_own, own_nodes, deg, x_perm = prep(x, edge_index, cfg)
    skey = (cfg, reps, phases, tuple(sorted((exp or {}).items())),
            tuple(plan["B"].reshape(-1).tolist()))
    if skey not in _CACHE:
        nc = build_program(plan, reps=reps, phases=phases, exp=exp)
        _CACHE[skey] = BassRunner(nc, cfg.n_cores)
    return _CACHE[skey], plan, idxw, x_own, own_nodes, deg, x_perm


def run_cfg(x, edge_index, W_theta, W_phi, cfg: Cfg, time_iters=0, reps=1, phases="abc", exp=None):
    runner, plan, idxw, x_own, own_nodes, deg, x_perm = _get_runner_and_plan(x, edge_index, cfg, reps=reps, phases=phases, exp=exp)
    if exp and exp.get("sent_idx"):
        idxw = np.full_like(idxw, cfg.chunk_real)
    wtt = np.ascontiguousarray(np.asarray(W_theta, np.float32).T)
    wpt = np.ascontiguousarray(np.asarray(W_phi, np.float32).T)
    in_maps = [
        dict(x_full=x_perm, x_own=x_own[c], w_theta_t=wtt, w_phi_t=wpt,
             idxw=np.ascontiguousarray(idxw[c]) if plan["W_total"] > 0
             else np.zeros((128, 16), np.int16))
        for c in range(cfg.n_cores)
    ]
    runner.prepare(in_maps)
    outs = runner.run()
    t_ns = runner.time_ns(iters=time_iters) if time_iters else None
    res = runner.results(outs)
    out_full = np.empty((cfg.N, CH), np.float32)
    for c in range(cfg.n_cores):
        out_full[own_nodes[c]] = res[c]["out"][: cfg.npc]
    out_full[deg == 0] = 0.0
    return out_full, t_ns


def kernel(x, edge_index, W_theta, W_phi):
    out, _ = run_cfg(x, edge_index, W_theta, W_phi, Cfg())
    return out



# revision 10
# speedup vs baseline: 1.7965x; 1.7965x over previous
"""Trainium2 Bass kernel for DevConv-style GNN message passing.

Reference computation:
    rel_t = (x[row] - x[col]) @ W_theta.T          # [E, 128]
    aggr  = segment_max(rel_t, row, N)             # [N, 128], empty -> 0
    out   = aggr @ W_phi.T                         # [N, 128]

Key reformulation: with y = x @ W_theta.T, within a segment (fixed row d)
    max_e (y[d] - y[col_e]) = y[d] - min_e y[col_e]     (per channel)
so the per-edge matmul disappears and only ONE gather per edge (y[col]) is
needed, followed by a segmented min.

The gather is bound by SWDGE descriptor generation on the Pool engine
(~one descriptor per gathered row), so runtime ~ padded-slot count. Two
host-side layout optimizations minimize it:
  * balanced chunk coloring: dma_gather indices are int16, so the y table
    is split into 4 chunks of 25088 rows. A greedy quota-balanced coloring
    assigns each node to a chunk so every destination's edges spread
    ~deg/4 per chunk (vs Binomial(deg,1/4) for a range split).
  * count-vector tiling: destinations are grouped into 128-row tiles (and
    striped across the 8 cores) sorted by their per-chunk count vectors,
    so the per-tile-per-chunk max count (the padded rect width) is tight.
Together padding inflation drops ~1.95x -> ~1.37x.

Per core:
  Phase A: y = x @ W_theta.T for ALL nodes (bf16) into the chunked HBM
           table (+1 sentinel row of +3e38 per chunk); x arrives already
           permuted into table order so stores are contiguous.
  Phase B: per 128-dest tile, all chunks' dma_gathers land in ONE SBUF
           rect [128 x sum_k B[t,k] slots] (pad slots point at the chunk
           sentinel); a single strided DVE tensor_reduce(min) folds the
           rect -> m[d] = min_e y[col_e]. (One 1-input reduce per tile
           instead of a 2-input fold tree keeps the DVE out of 2-port
           perf mode, which would lock GpSimd out of the shared SBUF port
           and stall SWDGE descriptor generation.)
  Phase C: aggr = y_own - m  (y_own computed on-chip from x_own),
           out_tile = aggr @ W_phi.T via PE transpose + matmul.
Host un-permutes the concatenated core outputs and zeroes empty nodes.
"""
import sys
import os

sys.path.insert(0, "/opt/trn_rl_repo")

from contextlib import ExitStack
from dataclasses import dataclass

import numpy as np
import ml_dtypes

import concourse.bass as bass
import concourse.tile as tile
from concourse import bacc, mybir
from concourse.masks import make_identity

import time

import jax
from jax.sharding import Mesh, PartitionSpec
from jax.experimental.shard_map import shard_map

from concourse.bass2jax import (
    _bass_exec_p, install_neuronx_cc_hook, partition_id_tensor)


class BassRunner:
    """Keeps a jitted PJRT executable for a Bass program so it can be run
    repeatedly on device-resident inputs (for wall-clock timing)."""

    def __init__(self, nc, n_cores: int):
        install_neuronx_cc_hook()
        self.nc = nc
        self.n_cores = n_cores
        partition_name = nc.partition_id_tensor.name if nc.partition_id_tensor else None
        in_names, out_names, out_avals = [], [], []
        for alloc in nc.m.functions[0].allocations:
            if not isinstance(alloc, mybir.MemoryLocationSet):
                continue
            name = alloc.memorylocations[0].name
            if alloc.kind == "ExternalInput":
                if name != partition_name:
                    in_names.append(name)
            elif alloc.kind == "ExternalOutput":
                out_names.append(name)
                out_avals.append(jax.core.ShapedArray(
                    tuple(alloc.tensor_shape), mybir.dt.np(alloc.dtype)))
        self.in_names, self.out_names, self.out_avals = in_names, out_names, out_avals
        self.n_params = len(in_names)
        all_in_names = list(in_names) + list(out_names)
        if partition_name is not None:
            all_in_names.append(partition_name)

        def _body(*args):
            operands = list(args)
            if partition_name is not None:
                operands.append(partition_id_tensor())
            outs = _bass_exec_p.bind(
                *operands,
                out_avals=tuple(out_avals),
                in_names=tuple(all_in_names),
                out_names=tuple(out_names),
                lowering_input_output_aliases=(),
                sim_require_finite=True,
                sim_require_nnan=True,
                nc=nc,
            )
            return tuple(outs)

        devices = jax.devices()[:n_cores]
        self.mesh = Mesh(np.asarray(devices), ("core",))
        n_outs = len(out_names)
        in_specs = (PartitionSpec("core"),) * (self.n_params + n_outs)
        out_specs = (PartitionSpec("core"),) * n_outs
        self.fn = jax.jit(
            shard_map(_body, mesh=self.mesh, in_specs=in_specs,
                      out_specs=out_specs, check_rep=False),
            keep_unused=True,
        )
        self._dev_args = None

    def prepare(self, in_maps):
        assert len(in_maps) == self.n_cores
        concat_in = [
            np.concatenate([np.asarray(in_maps[c][n]) for c in range(self.n_cores)],
                           axis=0)
            for n in self.in_names
        ]
        concat_zeros = [
            np.zeros((self.n_cores * a.shape[0], *a.shape[1:]), a.dtype)
            for a in self.out_avals
        ]
        sharding = jax.sharding.NamedSharding(self.mesh, PartitionSpec("core"))
        self._dev_args = [jax.device_put(v, sharding) for v in concat_in + concat_zeros]
        return self

    def run(self):
        outs = self.fn(*self._dev_args)
        jax.block_until_ready(outs)
        return outs

    def results(self, outs):
        return [
            {n: np.asarray(outs[i]).reshape(self.n_cores, *self.out_avals[i].shape)[c]
             for i, n in enumerate(self.out_names)}
            for c in range(self.n_cores)
        ]

    def time_ns(self, iters=5, warmup=2):
        for _ in range(warmup):
            self.run()
        ts = []
        for _ in range(iters):
            t0 = time.perf_counter()
            self.run()
            ts.append((time.perf_counter() - t0) * 1e9)
        return min(ts)


CH = 128
F32 = mybir.dt.float32
BF16 = mybir.dt.bfloat16
I16 = mybir.dt.int16
SENT_VAL = 3.0e38
IDX_PER_INST = 1024          # dma_gather crashes >= 2048 idx/instruction
BLK_PER_INST = IDX_PER_INST // 128


@dataclass(frozen=True)
class Cfg:
    N: int = 100_000
    E: int = 3_200_000
    n_cores: int = 8
    chunk_real: int = 25_088     # table rows per chunk (512-aligned, < int16 max)
    idx_per_inst: int = 1024
    balanced: bool = True        # balanced chunk coloring + count-vector tiling
    n_sent: int = 128            # sentinel rows per chunk (pad reads spread
                                 # over HBM banks instead of hammering one row)

    @property
    def n_chunks(self):
        return (self.N + self.chunk_real - 1) // self.chunk_real

    @property
    def chunk_stride(self):
        return self.chunk_real + self.n_sent

    @property
    def npc(self):
        assert self.N % self.n_cores == 0
        return self.N // self.n_cores

    @property
    def tiles(self):
        return (self.npc + 127) // 128


def _wrap16(seg: np.ndarray) -> np.ndarray:
    """Per-instruction idx wrap: flat [n] -> [128, n//16]; idx i at
    (partition i%16, col i//16), replicated across the 8 gpsimd groups."""
    n = seg.shape[-1]
    w = seg.reshape(*seg.shape[:-1], n // 16, 16)
    w = np.swapaxes(w, -1, -2)                       # [..., 16, n//16]
    return np.tile(w, (1,) * (seg.ndim - 1) + (8, 1))


def _color_chunks(row, col, deg, N, NK, cap):
    """Greedy quota-balanced assignment of cols to NK chunks; returns
    (chunk_of, cnt_dk) where cnt_dk[d,k] = per-dest per-chunk edge count."""
    o = np.argsort(col, kind="stable")
    dest_s = row[o]
    starts = np.searchsorted(col[o], np.arange(N + 1))
    quota = -(-deg // NK)
    cnt_dk = np.zeros((N, NK), np.int32)
    chunk_of = np.full(N, -1, np.int64)
    chunk_sz = np.zeros(NK, np.int64)
    col_order = np.argsort(-(starts[1:] - starts[:-1]), kind="stable")
    for _ in range(2):
        for c in col_order:
            ds = dest_s[starts[c]: starts[c + 1]]
            kprev = chunk_of[c]
            if kprev >= 0:
                if len(ds):
                    cnt_dk[ds, kprev] -= 1
                chunk_sz[kprev] -= 1
            if len(ds) == 0:
                k = int(np.argmin(chunk_sz))
            else:
                cc = cnt_dk[ds]
                over = np.maximum(0, cc + 1 - quota[ds][:, None])
                sc = (over * 1000.0 + cc).sum(axis=0).astype(np.float64)
                sc += chunk_sz * 1e-4
                sc[chunk_sz >= cap] = 1e18
                k = int(np.argmin(sc))
                cnt_dk[ds, k] += 1
            chunk_of[c] = k
            chunk_sz[k] += 1
    return chunk_of, cnt_dk


def prep(x, edge_index, cfg: Cfg):
    """Host-side data prep. Returns (plan, per-core inputs, unpermute info)."""
    N, E, NC = cfg.N, cfg.E, cfg.n_cores
    CR, NK, T = cfg.chunk_real, cfg.n_chunks, cfg.tiles
    row = np.asarray(edge_index[0], dtype=np.int64)
    col = np.asarray(edge_index[1], dtype=np.int64)

    deg = np.bincount(row, minlength=N)
    x_np0 = np.asarray(x, dtype=np.float32)
    if cfg.balanced:
        chunk_of, cnt_dk = _color_chunks(row, col, deg, N, NK, CR)
        # rank within chunk
        oc = np.argsort(chunk_of, kind="stable")
        rank_of = np.empty(N, np.int64)
        csz = np.bincount(chunk_of, minlength=NK)
        cstart = np.concatenate([[0], np.cumsum(csz)])
        rank_of[oc] = np.arange(N) - cstart[chunk_of[oc]]
        # dest order: group similar per-chunk count vectors into tiles
        order = np.arange(N)
        for k in range(NK):
            order = order[np.argsort(-cnt_dk[order, k], kind="stable")]
        # x permuted into table layout [NK*CR, CH]
        x_perm = np.zeros((NK * CR, x_np0.shape[1]), np.float32)
        x_perm[chunk_of * CR + rank_of] = x_np0
    else:
        order = np.argsort(-deg, kind="stable")      # node ids by desc degree
        x_perm = x_np0
    core_of = np.empty(N, np.int64)
    pos_of = np.empty(N, np.int64)
    r = np.arange(N)
    core_of[order] = r % NC
    pos_of[order] = r // NC

    ec = core_of[row]
    ep = pos_of[row]
    if cfg.balanced:
        ek = chunk_of[col]
        elocal = rank_of[col].astype(np.int16)
    else:
        ek = col // CR
        elocal = (col - ek * CR).astype(np.int16)
    et = ep // 128
    ed = ep % 128

    # per-(core,tile,chunk,node) counts and within-group slot index j
    key = ((ec * T + et) * NK + ek) * 128 + ed
    o = np.argsort(key, kind="stable")
    ks = key[o]
    first = np.r_[True, ks[1:] != ks[:-1]]
    run_id = np.cumsum(first) - 1
    run_start = np.flatnonzero(first)
    j = np.arange(E) - run_start[run_id]

    cnt = np.bincount(key, minlength=NC * T * NK * 128).reshape(NC, T, NK, 128)
    B = cnt.max(axis=(0, 3)).astype(np.int64)        # [T, NK] shared structure

    Bf = B.reshape(-1)
    off = np.concatenate([[0], np.cumsum(Bf * 128)])  # slot offset per (t,k)
    total_slots = int(off[-1])

    # pad slots point at one of n_sent sentinel rows (spread over HBM banks)
    pad_vals = (CR + (np.arange(total_slots) % cfg.n_sent)).astype(np.int16)
    idx_all = np.broadcast_to(pad_vals, (NC, total_slots)).copy()
    tk = et[o] * NK + ek[o]
    pos_in = off[tk] + j * 128 + ed[o]
    idx_all[ec[o], pos_in] = elocal[o]

    # split into gather instructions and build wrapped idx input
    blk_per_inst = cfg.idx_per_inst // 128
    insts = []           # (t, k, g0blk, nblk, col_off)
    tile_cols = []       # per tile: (col_start, col_end)
    wsegs = []
    col_off = 0
    for t in range(T):
        t_start = col_off
        for k in range(NK):
            btk = int(B[t, k])
            base = int(off[t * NK + k])
            for g0 in range(0, btk, blk_per_inst):
                nb = min(blk_per_inst, btk - g0)
                n_i = nb * 128
                seg = idx_all[:, base + g0 * 128: base + g0 * 128 + n_i]
                wsegs.append(_wrap16(seg))
                insts.append((t, k, g0, nb, col_off))
                col_off += n_i // 16
        tile_cols.append((t_start, col_off))
    idxw = np.concatenate(wsegs, axis=2) if wsegs else np.zeros((NC, 128, 0), np.int16)
    W_total = idxw.shape[2]

    # per-core x_own in pos order, padded to T*128 rows
    own_nodes = np.empty((NC, cfg.npc), np.int64)
    own_nodes[core_of[order], pos_of[order]] = order  # own_nodes[c, p] = node id
    x_np = np.asarray(x, dtype=np.float32)
    x_own = np.zeros((NC, T * 128, CH), np.float32)
    x_own[:, : cfg.npc] = x_np[own_nodes]

    plan = dict(cfg=cfg, B=B, insts=insts, tile_cols=tile_cols, W_total=W_total)
    return plan, idxw, x_own, own_nodes, deg, x_perm


def build_program(plan, reps=1, phases="abc", exp=None):
    exp = {**dict(gather="on", folds="on", queue="rot", fold_mode="reduce"),
           **(exp or {})}
    cfg: Cfg = plan["cfg"]
    N, NK, CR, T = cfg.N, cfg.n_chunks, cfg.chunk_real, cfg.tiles
    CS = cfg.chunk_stride
    B, insts, tile_cols, W_total = (
        plan["B"], plan["insts"], plan["tile_cols"], plan["W_total"])

    nc = bacc.Bacc(None, target_bir_lowering=False, num_swdge_queues=4,
                   dynamic_dma_scratch_size=int(exp.get("scratch", 16384)))
    NA = NK * CR if cfg.balanced else N      # phase-A row count (table layout)
    x_full = nc.declare_dram_parameter("x_full", [NA, CH], F32, isOutput=False)
    x_own = nc.declare_dram_parameter("x_own", [T * 128, CH], F32, isOutput=False)
    wth = nc.declare_dram_parameter("w_theta_t", [CH, CH], F32, isOutput=False)
    wph = nc.declare_dram_parameter("w_phi_t", [CH, CH], F32, isOutput=False)
    idxw = nc.declare_dram_parameter("idxw", [128, max(W_total, 16)], I16, isOutput=False)
    out = nc.declare_dram_parameter("out", [T * 128, CH], F32, isOutput=True)

    qc = [0]  # gather queue rotation

    with tile.TileContext(nc) as tc:
        with ExitStack() as ctx:
            consts = ctx.enter_context(tc.tile_pool(name="consts", bufs=1))
            dram = ctx.enter_context(tc.tile_pool(name="dram", bufs=1, space="DRAM"))
            ax = ctx.enter_context(tc.tile_pool(name="ax", bufs=2))
            axT = ctx.enter_context(tc.tile_pool(name="axT", bufs=2))
            ay = ctx.enter_context(tc.tile_pool(name="ay", bufs=2))
            ps_t = ctx.enter_context(tc.tile_pool(name="ps_t", bufs=2, space="PSUM"))
            ps_y = ctx.enter_context(tc.tile_pool(name="ps_y", bufs=2, space="PSUM"))
            ps_c = ctx.enter_context(tc.tile_pool(name="ps_c", bufs=2, space="PSUM"))
            gidx = ctx.enter_context(tc.tile_pool(name="gidx", bufs=4))
            gdst = ctx.enter_context(tc.tile_pool(name="gdst", bufs=2))
            fold = ctx.enter_context(tc.tile_pool(name="fold", bufs=2))
            fin = ctx.enter_context(tc.tile_pool(name="fin", bufs=2))

            y_aug = dram.tile([NK * CS, CH], BF16)

            ident = consts.tile([128, 128], F32)
            make_identity(nc, ident[:])
            wth_sb = consts.tile([CH, CH], F32)
            nc.sync.dma_start(out=wth_sb[:], in_=wth[:])
            wph_sb = consts.tile([CH, CH], F32)
            nc.sync.dma_start(out=wph_sb[:], in_=wph[:])
            y_own_sb = consts.tile([128, T * 128], F32)
            probe = consts.tile([128, CH], BF16)
            nc.gpsimd.memset(probe[:], 0.0)
            cst128 = consts.tile([128, CH], BF16)
            nc.gpsimd.memset(cst128[:], 1.0)
            sent = consts.tile([cfg.n_sent, CH], BF16)
            nc.gpsimd.memset(sent[:], SENT_VAL)
            for k in range(NK):
                nc.sync.dma_start(
                    out=y_aug[k * CS + CR: k * CS + CR + cfg.n_sent, :],
                    in_=sent[:])

            A_MODE = os.environ.get("A_MODE", "full")
            PS_BUFS = int(os.environ.get("PS_BUFS", "2"))
            # ---------------- Phase A: y_aug = (x @ W_theta.T).bf16 ----------
            def emit_group(src, n0, gn, dst):
                """Process rows [n0, n0+gn) of src -> y into dst.
                dst = ("aug",) writes y_aug rows (with chunk-boundary split),
                dst = ("own",) writes y_own_sb cols."""
                nt = (gn + 127) // 128
                xg = ax.tile([128, nt * 128], F32, tag="xg",
                             bufs=int(os.environ.get("XG_BUFS", "2")))
                xg3 = xg[:].rearrange("p (i c) -> p i c", c=CH)
                load_eng = nc.gpsimd if A_MODE == "dma3" else nc.sync
                if gn % 128 == 0:
                    load_eng.dma_start(
                        out=xg3[:, :nt, :],
                        in_=src[n0: n0 + gn, :].rearrange("(i p) c -> p i c", p=128))
                else:
                    for i in range(nt):
                        rn = min(128, gn - i * 128)
                        nc.sync.dma_start(
                            out=xg3[:rn, i, :],
                            in_=src[n0 + i * 128: n0 + i * 128 + rn, :])
                if A_MODE.startswith("dma") and dst == "aug":
                    # dma : load->store dep, both on sync
                    # dma2: stores only dep-free (loads still emitted)
                    # dma3: load on gpsimd, stores dep on load, on sync
                    # dma5: loads only (no stores)
                    if gn % 128 == 0:
                        for i in range(nt):
                            r0 = n0 + i * 128
                            kb = (r0 // CR) * cfg.n_sent
                            if A_MODE == "dma5":
                                continue
                            src_ap = (cst128[:] if A_MODE == "dma2"
                                      else xg3[:, i, :CH // 2].bitcast(BF16))
                            nc.sync.dma_start(
                                out=y_aug[r0 + kb: r0 + kb + 128, :], in_=src_ap)
                    return
                pt = ps_t.tile([128, nt * 128], F32, tag="pt", bufs=PS_BUFS)
                for i in range(nt):
                    rn = min(128, gn - i * 128)
                    nc.tensor.transpose(
                        out=pt[:, i * 128: i * 128 + rn],
                        in_=xg3[:rn, i, :],
                        identity=ident[:rn, :rn])
                xT = axT.tile([128, nt * 128], F32, tag="xT")
                if exp.get("fold_mode") == "reduce":
                    nc.scalar.copy(out=xT[:, : nt * 128], in_=pt[:, : nt * 128])
                else:
                    nc.vector.tensor_copy(out=xT[:, : nt * 128], in_=pt[:, : nt * 128])
                if A_MODE == "nomm" and dst == "aug":
                    for i in range(nt):
                        r0 = n0 + i * 128
                        kb = (r0 // CR) * cfg.n_sent
                        nc.sync.dma_start(
                            out=y_aug[r0 + kb: r0 + kb + 128, :],
                            in_=xT[:, i * 128: i * 128 + 128][:, :CH // 2].bitcast(BF16))
                    return
                py = ps_y.tile([128, nt * 128], F32, tag="py", bufs=PS_BUFS)
                for i in range(nt):
                    rn = min(128, gn - i * 128)
                    nc.tensor.matmul(
                        out=py[:rn, i * 128: (i + 1) * 128],
                        lhsT=xT[:, i * 128: i * 128 + rn],
                        rhs=wth_sb[:],
                        start=True, stop=True)
                if dst == "own":
                    nc.scalar.copy(
                        out=y_own_sb[:, n0: n0 + nt * 128], in_=py[:, : nt * 128])
                    return
                yg = ay.tile([128, nt * 128], BF16, tag="yg")
                copy2 = nc.vector.tensor_copy if A_MODE == "dvecopy" else nc.scalar.copy
                if gn % 128 == 0:
                    copy2(out=yg[:, : gn], in_=py[:, : gn])
                else:
                    for i in range(nt):
                        rn = min(128, gn - i * 128)
                        copy2(
                            out=yg[:rn, i * 128: (i + 1) * 128],
                            in_=py[:rn, i * 128: (i + 1) * 128])
                yg3 = yg[:].rearrange("p (i c) -> p i c", c=CH)
                # write y rows n -> aug rows n + n // CR, splitting at tile level
                for i in range(nt):
                    r0 = n0 + i * 128
                    rn = min(128, gn - i * 128)
                    kc = r0 // CR
                    ec = (r0 + rn - 1) // CR
                    kb = kc * cfg.n_sent
                    ke = ec * cfg.n_sent
                    if kc == ec:
                        nc.sync.dma_start(
                            out=y_aug[r0 + kb: r0 + kb + rn, :], in_=yg3[:rn, i, :])
                    else:
                        split = (kc + 1) * CR - r0       # rows before boundary
                        nc.sync.dma_start(
                            out=y_aug[r0 + kb: r0 + kb + split, :],
                            in_=yg3[:split, i, :])
                        nc.sync.dma_start(
                            out=y_aug[r0 + split + ke: r0 + ke + rn, :],
                            in_=yg3[split:rn, i, :])

            by_tile = {}
            for (t, k, g0, nb, coff) in insts:
                by_tile.setdefault(t, []).append((k, g0, nb, coff))

            for _rep in range(reps):
              for n0 in range(0, NA, 512):
                emit_group(x_full, n0, min(512, NA - n0), "aug")
              for n0 in range(0, T * 128, 512):
                emit_group(x_own, n0, min(512, T * 128 - n0), "own")

              # ---------------- Phase B + C per tile ---------------------------
              for t in range(T):
                 c0, c1 = tile_cols[t]
                 it = gidx.tile([128, max(c1 - c0, 16)], I16, tag="it")
                 if c1 > c0:
                     nc.sync.dma_start(out=it[:, : c1 - c0], in_=idxw[:, c0:c1])
                 if exp.get("fold_mode") == "reduce":
                     kws = [k for k in range(NK) if int(B[t, k]) > 0]
                     koff = {}
                     wt = 0
                     for k in kws:
                         koff[k] = wt
                         wt += int(B[t, k])
                     if wt > 0:
                         dk = gdst.tile([128, wt * CH], BF16, tag="gr",
                                        bufs=int(exp.get("gbufs", 3)))
                         dk3 = dk[:].rearrange("p (b c) -> p b c", c=CH)
                     for (k, g0, nb, coff) in by_tile.get(t, []):
                         n_i = nb * 128
                         col0 = koff[k] + g0
                         if exp["gather"] == "seq":
                             r0 = min(k * CS + g0 * 128, NK * CS - n_i)
                             nc.sync.dma_start(
                                 out=dk3[:, col0: col0 + nb, :],
                                 in_=y_aug[r0: r0 + n_i, :].rearrange(
                                     "(i p) c -> p i c", p=128))
                             continue
                         nc.gpsimd.dma_gather(
                             out_ap=dk3[:, col0: col0 + nb, :],
                             in_ap=y_aug[k * CS: (k + 1) * CS, :],
                             idxs_ap=it[:, coff - c0: coff - c0 + n_i // 16],
                             num_idxs=n_i,
                             num_idxs_reg=n_i,
                             elem_size=CH,
                             queue_num=(qc[0] % 4) if exp["queue"] == "rot" else 0,
                         )
                         qc[0] += 1
                     m = fin.tile([128, CH], F32, tag="m")
                     if wt == 0 or exp.get("reduce") == "off":
                         nc.gpsimd.memset(m[:], SENT_VAL)
                     else:
                         dkT = dk[:].rearrange("p (b c) -> p c b", c=CH)
                         nc.vector.tensor_reduce(
                             out=m[:], in_=dkT, axis=mybir.AxisListType.X,
                             op=mybir.AluOpType.min)
                     aggr = fin.tile([128, CH], F32, tag="aggr")
                     nc.vector.tensor_sub(
                         out=aggr[:], in0=y_own_sb[:, t * 128: (t + 1) * 128],
                         in1=m[:])
                     ptr = ps_c.tile([128, CH], F32, tag="ctr")
                     nc.tensor.transpose(out=ptr[:], in_=aggr[:], identity=ident[:])
                     aggrT = fin.tile([128, CH], F32, tag="aggrT")
                     nc.scalar.copy(out=aggrT[:], in_=ptr[:])
                     po = ps_c.tile([128, CH], F32, tag="cmm")
                     nc.tensor.matmul(out=po[:], lhsT=aggrT[:], rhs=wph_sb[:],
                                      start=True, stop=True)
                     osb = fin.tile([128, CH], F32, tag="osb")
                     nc.scalar.copy(out=osb[:], in_=po[:])
                     nc.sync.dma_start(out=out[t * 128: (t + 1) * 128, :],
                                       in_=osb[:])
                     continue
                 dks = {}
                 for k in range(NK):
                     btk = int(B[t, k])
                     if btk == 0:
                         continue
                     dks[k] = gdst.tile([128, btk * CH], BF16, tag=f"g{k}", name=f"dk{k}")
                 for (k, g0, nb, coff) in by_tile.get(t, []):
                     if exp["gather"] == "off":
                         break
                     dk3 = dks[k][:].rearrange("p (b c) -> p b c", c=CH)
                     n_i = nb * 128
                     ndup = 2 if exp["gather"] == "dup" else 1
                     for di in range(ndup):
                         if di == 0:
                             dst = dk3[:, g0: g0 + nb, :]
                         else:
                             ddup = gdst.tile(
                                 [128, (cfg.idx_per_inst // 128) * CH], BF16,
                                 tag="gdup", bufs=2)
                             dst = ddup[:].rearrange(
                                 "p (b c) -> p b c", c=CH)[:, :nb, :]
                         nc.gpsimd.dma_gather(
                             out_ap=dst,
                             in_ap=y_aug[k * CS: (k + 1) * CS, :],
                             idxs_ap=it[:, coff - c0: coff - c0 + n_i // 16],
                             num_idxs=n_i,
                             num_idxs_reg=n_i,
                             elem_size=CH,
                             queue_num=(qc[0] % 4) if exp["queue"] == "rot" else 0,
                         )
                         qc[0] += 1
                 # fold each chunk's rect down to one [128, CH] min
                 mks = []
                 if exp["folds"] == "off":
                     m = fin.tile([128, CH], F32, tag="m")
                     nc.gpsimd.memset(m[:], SENT_VAL)
                     aggr = fin.tile([128, CH], F32, tag="aggr")
                     nc.vector.tensor_sub(
                         out=aggr[:], in0=y_own_sb[:, t * 128: (t + 1) * 128],
                         in1=m[:])
                     ptr = ps_c.tile([128, CH], F32, tag="ctr")
                     nc.tensor.transpose(out=ptr[:], in_=aggr[:], identity=ident[:])
                     aggrT = fin.tile([128, CH], F32, tag="aggrT")
                     nc.vector.tensor_copy(out=aggrT[:], in_=ptr[:])
                     po = ps_c.tile([128, CH], F32, tag="cmm")
                     nc.tensor.matmul(out=po[:], lhsT=aggrT[:], rhs=wph_sb[:],
                                      start=True, stop=True)
                     osb = fin.tile([128, CH], F32, tag="osb")
                     nc.scalar.copy(out=osb[:], in_=po[:])
                     nc.sync.dma_start(out=out[t * 128: (t + 1) * 128, :], in_=osb[:])
                     continue
                 for k in range(NK):
                     if k not in dks:
                         continue
                     cur = dks[k]
                     nb = int(B[t, k])
                     while nb > 1:
                         half = (nb + 1) // 2
                         nxt = fold.tile([128, half * CH], BF16, tag=f"f{k}", bufs=3)
                         nc.vector.tensor_tensor(
                             out=nxt[:, : half * CH],
                             in0=cur[:, : half * CH],
                             in1=cur[:, (nb - half) * CH: nb * CH],
                             op=mybir.AluOpType.min)
                         cur, nb = nxt, half
                     mks.append(cur)
                 m = fin.tile([128, CH], F32, tag="m")
                 if len(mks) == 0:
                     nc.gpsimd.memset(m[:], SENT_VAL)
                 elif len(mks) == 1:
                     nc.vector.tensor_copy(out=m[:], in_=mks[0][:, :CH])
                 else:
                     # sequential accumulate with alternating tags (max 2 live)
                     acc = mks[0]
                     for i in range(1, len(mks) - 1):
                         mm = fold.tile([128, CH], BF16, tag=f"mrg{i % 2}")
                         nc.vector.tensor_tensor(
                             out=mm[:], in0=acc[:, :CH], in1=mks[i][:, :CH],
                             op=mybir.AluOpType.min)
                         acc = mm
                     nc.vector.tensor_tensor(
                         out=m[:], in0=acc[:, :CH], in1=mks[-1][:, :CH],
                         op=mybir.AluOpType.min)
                 # aggr = y_own - m ; out_tile = aggr @ W_phi.T
                 aggr = fin.tile([128, CH], F32, tag="aggr")
                 nc.vector.tensor_sub(
                     out=aggr[:], in0=y_own_sb[:, t * 128: (t + 1) * 128], in1=m[:])
                 ptr = ps_c.tile([128, CH], F32, tag="ctr")
                 nc.tensor.transpose(out=ptr[:], in_=aggr[:], identity=ident[:])
                 aggrT = fin.tile([128, CH], F32, tag="aggrT")
                 nc.vector.tensor_copy(out=aggrT[:], in_=ptr[:])
                 po = ps_c.tile([128, CH], F32, tag="cmm")
                 nc.tensor.matmul(out=po[:], lhsT=aggrT[:], rhs=wph_sb[:],
                                  start=True, stop=True)
                 osb = fin.tile([128, CH], F32, tag="osb")
                 nc.scalar.copy(out=osb[:], in_=po[:])
                 nc.sync.dma_start(out=out[t * 128: (t + 1) * 128, :], in_=osb[:])

            if phases != "abc":
                fillz = consts.tile([128, CH], F32)
                nc.vector.tensor_copy(out=fillz[:], in_=probe[:])
                for t in range(T):
                    nc.sync.dma_start(out=out[t * 128: (t + 1) * 128, :], in_=fillz[:])
    nc.compile()
    return nc


_CACHE = {}


def _get_runner_and_plan(x, edge_index, cfg: Cfg, reps=1, phases="abc", exp=None):
    plan, idxw, x_own, own_nodes, deg, x_perm = prep(x, edge_index, cfg)
    skey = (cfg, reps, phases, tuple(sorted((exp or {}).items())),
            tuple(plan["B"].reshape(-1).tolist()))
    if skey not in _CACHE:
        nc = build_program(plan, reps=reps, phases=phases, exp=exp)
        _CACHE[skey] = BassRunner(nc, cfg.n_cores)
    return _CACHE[skey], plan, idxw, x_own, own_nodes, deg, x_perm


def run_cfg(x, edge_index, W_theta, W_phi, cfg: Cfg, time_iters=0, reps=1, phases="abc", exp=None):
    runner, plan, idxw, x_own, own_nodes, deg, x_perm = _get_runner_and_plan(x, edge_index, cfg, reps=reps, phases=phases, exp=exp)
    if exp and exp.get("sent_idx"):
        idxw = np.full_like(idxw, cfg.chunk_real)
    wtt = np.ascontiguousarray(np.asarray(W_theta, np.float32).T)
    wpt = np.ascontiguousarray(np.asarray(W_phi, np.float32).T)
    in_maps = [
        dict(x_full=x_perm, x_own=x_own[c], w_theta_t=wtt, w_phi_t=wpt,
             idxw=np.ascontiguousarray(idxw[c]) if plan["W_total"] > 0
             else np.zeros((128, 16), np.int16))
        for c in range(cfg.n_cores)
    ]
    runner.prepare(in_maps)
    outs = runner.run()
    t_ns = runner.time_ns(iters=time_iters) if time_iters else None
    res = runner.results(outs)
    out_full = np.empty((cfg.N, CH), np.float32)
    for c in range(cfg.n_cores):
        out_full[own_nodes[c]] = res[c]["out"][: cfg.npc]
    out_full[deg == 0] = 0.0
    return out_full, t_ns


def kernel(x, edge_index, W_theta, W_phi):
    out, _ = run_cfg(x, edge_index, W_theta, W_phi, Cfg())
    return out



# revision 20
# speedup vs baseline: 2.4466x; 1.3619x over previous
"""Trainium2 Bass kernel for DevConv-style GNN message passing.

Reference computation:
    rel_t = (x[row] - x[col]) @ W_theta.T          # [E, 128]
    aggr  = segment_max(rel_t, row, N)             # [N, 128], empty -> 0
    out   = aggr @ W_phi.T                         # [N, 128]

Key reformulation: with y = x @ W_theta.T, within a segment (fixed row d)
    max_e (y[d] - y[col_e]) = y[d] - min_e y[col_e]     (per channel)
so the per-edge matmul disappears and only ONE gather per edge (y[col]) is
needed, followed by a segmented min.

The gather is bound by SWDGE descriptor generation on the Pool engine
(~one descriptor per gathered row), so runtime ~ padded-slot count. Two
host-side layout optimizations minimize it:
  * balanced chunk coloring: dma_gather indices are int16, so the y table
    is split into 4 chunks of 25088 rows. A greedy quota-balanced coloring
    assigns each node to a chunk so every destination's edges spread
    ~deg/4 per chunk (vs Binomial(deg,1/4) for a range split).
  * count-vector tiling: destinations are grouped into 128-row tiles (and
    striped across the 8 cores) sorted by their per-chunk count vectors,
    so the per-tile-per-chunk max count (the padded rect width) is tight.
Together padding inflation drops ~1.95x -> ~1.37x.

Per core:
  Phase A: y = x @ W_theta.T for ALL nodes (bf16) into the chunked HBM
           table (+1 sentinel row of +3e38 per chunk); x arrives already
           permuted into table order so stores are contiguous.
  Phase B: per 128-dest tile, all chunks' dma_gathers land in ONE SBUF
           rect [128 x sum_k B[t,k] slots] (pad slots point at the chunk
           sentinel); a single strided DVE tensor_reduce(min) folds the
           rect -> m[d] = min_e y[col_e]. (One 1-input reduce per tile
           instead of a 2-input fold tree keeps the DVE out of 2-port
           perf mode, which would lock GpSimd out of the shared SBUF port
           and stall SWDGE descriptor generation.)
  Phase C: aggr = y_own - m  (y_own computed on-chip from x_own),
           out_tile = aggr @ W_phi.T via PE transpose + matmul.
Host un-permutes the concatenated core outputs and zeroes empty nodes.
"""
import sys
import os

sys.path.insert(0, "/opt/trn_rl_repo")

from contextlib import ExitStack
from dataclasses import dataclass

import numpy as np
import ml_dtypes

import concourse.bass as bass
import concourse.tile as tile
from concourse import bacc, mybir
from concourse.masks import make_identity

import time

import jax
from jax.sharding import Mesh, PartitionSpec
from jax.experimental.shard_map import shard_map

from concourse.bass2jax import (
    _bass_exec_p, install_neuronx_cc_hook, partition_id_tensor)


class BassRunner:
    """Keeps a jitted PJRT executable for a Bass program so it can be run
    repeatedly on device-resident inputs (for wall-clock timing)."""

    def __init__(self, nc, n_cores: int):
        install_neuronx_cc_hook()
        self.nc = nc
        self.n_cores = n_cores
        partition_name = nc.partition_id_tensor.name if nc.partition_id_tensor else None
        in_names, out_names, out_avals = [], [], []
        for alloc in nc.m.functions[0].allocations:
            if not isinstance(alloc, mybir.MemoryLocationSet):
                continue
            name = alloc.memorylocations[0].name
            if alloc.kind == "ExternalInput":
                if name != partition_name:
                    in_names.append(name)
            elif alloc.kind == "ExternalOutput":
                out_names.append(name)
                out_avals.append(jax.core.ShapedArray(
                    tuple(alloc.tensor_shape), mybir.dt.np(alloc.dtype)))
        self.in_names, self.out_names, self.out_avals = in_names, out_names, out_avals
        self.n_params = len(in_names)
        all_in_names = list(in_names) + list(out_names)
        if partition_name is not None:
            all_in_names.append(partition_name)

        def _body(*args):
            operands = list(args)
            if partition_name is not None:
                operands.append(partition_id_tensor())
            outs = _bass_exec_p.bind(
                *operands,
                out_avals=tuple(out_avals),
                in_names=tuple(all_in_names),
                out_names=tuple(out_names),
                lowering_input_output_aliases=(),
                sim_require_finite=True,
                sim_require_nnan=True,
                nc=nc,
            )
            return tuple(outs)

        devices = jax.devices()[:n_cores]
        self.mesh = Mesh(np.asarray(devices), ("core",))
        n_outs = len(out_names)
        in_specs = (PartitionSpec("core"),) * (self.n_params + n_outs)
        out_specs = (PartitionSpec("core"),) * n_outs
        self.fn = jax.jit(
            shard_map(_body, mesh=self.mesh, in_specs=in_specs,
                      out_specs=out_specs, check_rep=False),
            keep_unused=True,
        )
        self._dev_args = None

    def prepare(self, in_maps):
        assert len(in_maps) == self.n_cores
        concat_in = [
            np.concatenate([np.asarray(in_maps[c][n]) for c in range(self.n_cores)],
                           axis=0)
            for n in self.in_names
        ]
        concat_zeros = [
            np.zeros((self.n_cores * a.shape[0], *a.shape[1:]), a.dtype)
            for a in self.out_avals
        ]
        sharding = jax.sharding.NamedSharding(self.mesh, PartitionSpec("core"))
        self._dev_args = [jax.device_put(v, sharding) for v in concat_in + concat_zeros]
        return self

    def run(self):
        outs = self.fn(*self._dev_args)
        jax.block_until_ready(outs)
        return outs

    def results(self, outs):
        return [
            {n: np.asarray(outs[i]).reshape(self.n_cores, *self.out_avals[i].shape)[c]
             for i, n in enumerate(self.out_names)}
            for c in range(self.n_cores)
        ]

    def time_ns(self, iters=5, warmup=2):
        for _ in range(warmup):
            self.run()
        ts = []
        for _ in range(iters):
            t0 = time.perf_counter()
            self.run()
            ts.append((time.perf_counter() - t0) * 1e9)
        return min(ts)


CH = 128
F32 = mybir.dt.float32
BF16 = mybir.dt.bfloat16
I16 = mybir.dt.int16
SENT_VAL = 3.0e38
IDX_PER_INST = 1024          # dma_gather crashes >= 2048 idx/instruction
BLK_PER_INST = IDX_PER_INST // 128


@dataclass(frozen=True)
class Cfg:
    N: int = 100_000
    E: int = 3_200_000
    n_cores: int = 8
    chunk_real: int = 25_088     # table rows per chunk (512-aligned, < int16 max)
    idx_per_inst: int = 1024
    balanced: bool = True        # balanced chunk coloring + count-vector tiling
    n_sent: int = 128            # sentinel rows per chunk (pad reads spread
                                 # over HBM banks instead of hammering one row)
    layout: str = "v3"           # "v3": chunk-major windowed phase B
                                 # "tile": original per-tile rects
    win_tiles: int = 8           # tiles per gather window (v3)

    @property
    def n_chunks(self):
        return (self.N + self.chunk_real - 1) // self.chunk_real

    @property
    def chunk_stride(self):
        return self.chunk_real + self.n_sent

    @property
    def npc(self):
        assert self.N % self.n_cores == 0
        return self.N // self.n_cores

    @property
    def tiles(self):
        return (self.npc + 127) // 128


def _wrap16(seg: np.ndarray) -> np.ndarray:
    """Per-instruction idx wrap: flat [n] -> [128, n//16]; idx i at
    (partition i%16, col i//16), replicated across the 8 gpsimd groups."""
    n = seg.shape[-1]
    w = seg.reshape(*seg.shape[:-1], n // 16, 16)
    w = np.swapaxes(w, -1, -2)                       # [..., 16, n//16]
    return np.tile(w, (1,) * (seg.ndim - 1) + (8, 1))


def _color_chunks(row, col, deg, N, NK, cap):
    """Greedy quota-balanced assignment of cols to NK chunks; returns
    (chunk_of, cnt_dk) where cnt_dk[d,k] = per-dest per-chunk edge count."""
    o = np.argsort(col, kind="stable")
    dest_s = row[o]
    starts = np.searchsorted(col[o], np.arange(N + 1))
    quota = -(-deg // NK)
    cnt_dk = np.zeros((N, NK), np.int32)
    chunk_of = np.full(N, -1, np.int64)
    chunk_sz = np.zeros(NK, np.int64)
    col_order = np.argsort(-(starts[1:] - starts[:-1]), kind="stable")
    for _ in range(2):
        for c in col_order:
            ds = dest_s[starts[c]: starts[c + 1]]
            kprev = chunk_of[c]
            if kprev >= 0:
                if len(ds):
                    cnt_dk[ds, kprev] -= 1
                chunk_sz[kprev] -= 1
            if len(ds) == 0:
                k = int(np.argmin(chunk_sz))
            else:
                cc = cnt_dk[ds]
                over = np.maximum(0, cc + 1 - quota[ds][:, None])
                sc = (over * 1000.0 + cc).sum(axis=0).astype(np.float64)
                sc += chunk_sz * 1e-4
                sc[chunk_sz >= cap] = 1e18
                k = int(np.argmin(sc))
                cnt_dk[ds, k] += 1
            chunk_of[c] = k
            chunk_sz[k] += 1
    return chunk_of, cnt_dk


def prep(x, edge_index, cfg: Cfg):
    """Host-side data prep. Returns (plan, per-core inputs, unpermute info)."""
    N, E, NC = cfg.N, cfg.E, cfg.n_cores
    CR, NK, T = cfg.chunk_real, cfg.n_chunks, cfg.tiles
    row = np.asarray(edge_index[0], dtype=np.int64)
    col = np.asarray(edge_index[1], dtype=np.int64)

    deg = np.bincount(row, minlength=N)
    x_np0 = np.asarray(x, dtype=np.float32)
    if cfg.balanced:
        chunk_of, cnt_dk = _color_chunks(row, col, deg, N, NK, CR)
        # rank within chunk
        oc = np.argsort(chunk_of, kind="stable")
        rank_of = np.empty(N, np.int64)
        csz = np.bincount(chunk_of, minlength=NK)
        cstart = np.concatenate([[0], np.cumsum(csz)])
        rank_of[oc] = np.arange(N) - cstart[chunk_of[oc]]
        # dest order: group by max per-chunk count (the rect width driver),
        # then by the full count vector — tiles get near-identical vectors
        order = np.lexsort(tuple(cnt_dk[:, k] for k in range(NK - 1, -1, -1))
                           + (-cnt_dk.max(axis=1),))
        # x permuted into table layout [NK*CR, CH]
        x_perm = np.zeros((NK * CR, x_np0.shape[1]), np.float32)
        x_perm[chunk_of * CR + rank_of] = x_np0
    else:
        order = np.argsort(-deg, kind="stable")      # node ids by desc degree
        x_perm = x_np0
    core_of = np.empty(N, np.int64)
    pos_of = np.empty(N, np.int64)
    r = np.arange(N)
    core_of[order] = r % NC
    pos_of[order] = r // NC

    ec = core_of[row]
    ep = pos_of[row]
    if cfg.balanced:
        ek = chunk_of[col]
        elocal = rank_of[col].astype(np.int16)
    else:
        ek = col // CR
        elocal = (col - ek * CR).astype(np.int16)
    et = ep // 128
    ed = ep % 128

    # per-(core,tile,chunk,node) counts and within-group slot index j
    key = ((ec * T + et) * NK + ek) * 128 + ed
    o = np.argsort(key, kind="stable")
    ks = key[o]
    first = np.r_[True, ks[1:] != ks[:-1]]
    run_id = np.cumsum(first) - 1
    run_start = np.flatnonzero(first)
    j = np.arange(E) - run_start[run_id]

    cnt = np.bincount(key, minlength=NC * T * NK * 128).reshape(NC, T, NK, 128)
    B = cnt.max(axis=(0, 3)).astype(np.int64)        # [T, NK] shared structure
    B = np.maximum(B, 1)         # every (t,k) rect non-empty (simplifies accum)

    Bf = B.reshape(-1)
    off = np.concatenate([[0], np.cumsum(Bf * 128)])  # slot offset per (t,k)
    total_slots = int(off[-1])

    # pad slots point at one of n_sent sentinel rows (spread over HBM banks)
    pad_vals = (CR + (np.arange(total_slots) % cfg.n_sent)).astype(np.int16)
    idx_all = np.broadcast_to(pad_vals, (NC, total_slots)).copy()
    tk = et[o] * NK + ek[o]
    pos_in = off[tk] + j * 128 + ed[o]
    idx_all[ec[o], pos_in] = elocal[o]

    # split into gather instructions and build wrapped idx input
    blk_per_inst = cfg.idx_per_inst // 128
    insts = []           # (t, k, g0blk, nblk, col_off)
    tile_cols = []       # per tile: (col_start, col_end)
    windows = None
    wsegs = []
    col_off = 0
    if cfg.layout == "v3":
        # chunk-major layout: per chunk, all tiles' rects back-to-back;
        # instructions span tile boundaries within a window of win_tiles.
        Bc = np.ascontiguousarray(B.T)               # [NK, T]
        cb = np.concatenate([[0], np.cumsum(Bc.sum(axis=1) * 128)])
        off3 = np.zeros((NK, T), np.int64)
        for k in range(NK):
            off3[k] = cb[k] + np.concatenate(
                [[0], np.cumsum(Bc[k][:-1] * 128)])
        total3 = int(cb[-1])
        pad_vals = (CR + (np.arange(total3) % cfg.n_sent)).astype(np.int16)
        idx_all = np.broadcast_to(pad_vals, (NC, total3)).copy()
        pos_in = off3[ek[o], et[o]] + j * 128 + ed[o]
        idx_all[ec[o], pos_in] = elocal[o]
        windows = []
        for k in range(NK):
            for t0 in range(0, T, cfg.win_tiles):
                t1 = min(t0 + cfg.win_tiles, T)
                WB = int(Bc[k, t0:t1].sum())
                base_slot = int(off3[k, t0])
                inst_list = []
                for g0 in range(0, WB, blk_per_inst):
                    nb = min(blk_per_inst, WB - g0)
                    seg = idx_all[:, base_slot + g0 * 128:
                                  base_slot + (g0 + nb) * 128]
                    wsegs.append(_wrap16(seg))
                    inst_list.append((g0, nb, col_off))
                    col_off += nb * 128 // 16
                reduces = []
                acc = 0
                for t in range(t0, t1):
                    reduces.append((t, acc, int(Bc[k, t])))
                    acc += int(Bc[k, t])
                windows.append(dict(k=k, WB=WB, insts=inst_list,
                                    reduces=reduces))
    else:
        for t in range(T):
            t_start = col_off
            for k in range(NK):
                btk = int(B[t, k])
                base = int(off[t * NK + k])
                for g0 in range(0, btk, blk_per_inst):
                    nb = min(blk_per_inst, btk - g0)
                    n_i = nb * 128
                    seg = idx_all[:, base + g0 * 128: base + g0 * 128 + n_i]
                    wsegs.append(_wrap16(seg))
                    insts.append((t, k, g0, nb, col_off))
                    col_off += n_i // 16
            tile_cols.append((t_start, col_off))
    idxw = np.concatenate(wsegs, axis=2) if wsegs else np.zeros((NC, 128, 0), np.int16)
    W_total = idxw.shape[2]

    # per-core x_own in pos order, padded to T*128 rows
    own_nodes = np.empty((NC, cfg.npc), np.int64)
    own_nodes[core_of[order], pos_of[order]] = order  # own_nodes[c, p] = node id
    x_np = np.asarray(x, dtype=np.float32)
    x_own = np.zeros((NC, T * 128, CH), np.float32)
    x_own[:, : cfg.npc] = x_np[own_nodes]

    plan = dict(cfg=cfg, B=B, insts=insts, tile_cols=tile_cols, W_total=W_total,
                windows=windows)
    return plan, idxw, x_own, own_nodes, deg, x_perm


def build_program(plan, reps=1, phases="abc", exp=None):
    exp = {**dict(gather="on", folds="on", queue="rot", fold_mode="reduce"),
           **(exp or {})}
    cfg: Cfg = plan["cfg"]
    N, NK, CR, T = cfg.N, cfg.n_chunks, cfg.chunk_real, cfg.tiles
    CS = cfg.chunk_stride
    B, insts, tile_cols, W_total = (
        plan["B"], plan["insts"], plan["tile_cols"], plan["W_total"])

    windows = plan.get("windows")
    nc = bacc.Bacc(None, target_bir_lowering=False, num_swdge_queues=4,
                   dynamic_dma_scratch_size=int(exp.get("scratch", 16384)))
    NA = NK * CR if cfg.balanced else N      # phase-A row count (table layout)
    x_full = nc.declare_dram_parameter("x_full", [NA, CH], F32, isOutput=False)
    x_own = nc.declare_dram_parameter("x_own", [T * 128, CH], F32, isOutput=False)
    wth = nc.declare_dram_parameter("w_theta_t", [CH, CH], F32, isOutput=False)
    wph = nc.declare_dram_parameter("w_phi_t", [CH, CH], F32, isOutput=False)
    idxw = nc.declare_dram_parameter("idxw", [128, max(W_total, 16)], I16, isOutput=False)
    out = nc.declare_dram_parameter("out", [T * 128, CH], F32, isOutput=True)

    qc = [0]  # gather queue rotation

    with tile.TileContext(nc) as tc:
        with ExitStack() as ctx:
            consts = ctx.enter_context(tc.tile_pool(name="consts", bufs=1))
            dram = ctx.enter_context(tc.tile_pool(name="dram", bufs=1, space="DRAM"))
            ax = ctx.enter_context(tc.tile_pool(name="ax", bufs=2))
            axT = ctx.enter_context(tc.tile_pool(name="axT", bufs=2))
            ay = ctx.enter_context(tc.tile_pool(name="ay", bufs=2))
            ps_t = ctx.enter_context(tc.tile_pool(name="ps_t", bufs=2, space="PSUM"))
            ps_y = ctx.enter_context(tc.tile_pool(name="ps_y", bufs=2, space="PSUM"))
            ps_c = ctx.enter_context(tc.tile_pool(name="ps_c", bufs=2, space="PSUM"))
            gidx = ctx.enter_context(tc.tile_pool(name="gidx", bufs=4))
            gdst = ctx.enter_context(tc.tile_pool(name="gdst", bufs=2))
            fold = ctx.enter_context(tc.tile_pool(name="fold", bufs=2))
            fin = ctx.enter_context(tc.tile_pool(name="fin", bufs=2))

            y_aug = dram.tile([NK * CS, CH], BF16)

            ident = consts.tile([128, 128], F32)
            make_identity(nc, ident[:])
            wth_sb = consts.tile([CH, CH], F32)
            nc.sync.dma_start(out=wth_sb[:], in_=wth[:])
            wph_sb = consts.tile([CH, CH], F32)
            nc.sync.dma_start(out=wph_sb[:], in_=wph[:])
            y_own_sb = consts.tile([128, T * 128],
                                   BF16 if windows is not None else F32)
            probe = consts.tile([128, CH], BF16)
            nc.gpsimd.memset(probe[:], 0.0)
            cst128 = consts.tile([128, CH], BF16)
            nc.gpsimd.memset(cst128[:], 1.0)
            sent = consts.tile([cfg.n_sent, CH], BF16)
            nc.gpsimd.memset(sent[:], SENT_VAL)
            for k in range(NK):
                nc.sync.dma_start(
                    out=y_aug[k * CS + CR: k * CS + CR + cfg.n_sent, :],
                    in_=sent[:])

            A_MODE = os.environ.get("A_MODE", "full")
            PS_BUFS = int(os.environ.get("PS_BUFS", "2"))
            # ---------------- Phase A: y_aug = (x @ W_theta.T).bf16 ----------
            def emit_group(src, n0, gn, dst):
                """Process rows [n0, n0+gn) of src -> y into dst.
                dst = ("aug",) writes y_aug rows (with chunk-boundary split),
                dst = ("own",) writes y_own_sb cols."""
                nt = (gn + 127) // 128
                xg = ax.tile([128, nt * 128], F32, tag="xg",
                             bufs=int(os.environ.get("XG_BUFS", "2")))
                xg3 = xg[:].rearrange("p (i c) -> p i c", c=CH)
                load_eng = nc.gpsimd if A_MODE == "dma3" else nc.sync
                store_eng = nc.scalar if exp.get("a_store") == "act" else nc.sync
                if gn % 128 == 0:
                    load_eng.dma_start(
                        out=xg3[:, :nt, :],
                        in_=src[n0: n0 + gn, :].rearrange("(i p) c -> p i c", p=128))
                else:
                    for i in range(nt):
                        rn = min(128, gn - i * 128)
                        nc.sync.dma_start(
                            out=xg3[:rn, i, :],
                            in_=src[n0 + i * 128: n0 + i * 128 + rn, :])
                if A_MODE.startswith("dma") and dst == "aug":
                    # dma : load->store dep, both on sync
                    # dma2: stores only dep-free (loads still emitted)
                    # dma3: load on gpsimd, stores dep on load, on sync
                    # dma5: loads only (no stores)
                    if gn % 128 == 0:
                        for i in range(nt):
                            r0 = n0 + i * 128
                            kb = (r0 // CR) * cfg.n_sent
                            if A_MODE == "dma5":
                                continue
                            src_ap = (cst128[:] if A_MODE == "dma2"
                                      else xg3[:, i, :CH // 2].bitcast(BF16))
                            nc.sync.dma_start(
                                out=y_aug[r0 + kb: r0 + kb + 128, :], in_=src_ap)
                    return
                pt = ps_t.tile([128, nt * 128], F32, tag="pt", bufs=PS_BUFS)
                for i in range(nt):
                    rn = min(128, gn - i * 128)
                    nc.tensor.transpose(
                        out=pt[:, i * 128: i * 128 + rn],
                        in_=xg3[:rn, i, :],
                        identity=ident[:rn, :rn])
                xT = axT.tile([128, nt * 128], F32, tag="xT")
                if exp.get("fold_mode") == "reduce":
                    nc.scalar.copy(out=xT[:, : nt * 128], in_=pt[:, : nt * 128])
                else:
                    nc.vector.tensor_copy(out=xT[:, : nt * 128], in_=pt[:, : nt * 128])
                if A_MODE == "nomm" and dst == "aug":
                    for i in range(nt):
                        r0 = n0 + i * 128
                        kb = (r0 // CR) * cfg.n_sent
                        nc.sync.dma_start(
                            out=y_aug[r0 + kb: r0 + kb + 128, :],
                            in_=xT[:, i * 128: i * 128 + 128][:, :CH // 2].bitcast(BF16))
                    return
                py = ps_y.tile([128, nt * 128], F32, tag="py", bufs=PS_BUFS)
                for i in range(nt):
                    rn = min(128, gn - i * 128)
                    nc.tensor.matmul(
                        out=py[:rn, i * 128: (i + 1) * 128],
                        lhsT=xT[:, i * 128: i * 128 + rn],
                        rhs=wth_sb[:],
                        start=True, stop=True)
                if dst == "own":
                    nc.scalar.copy(
                        out=y_own_sb[:, n0: n0 + nt * 128], in_=py[:, : nt * 128])
                    return
                yg = ay.tile([128, nt * 128], BF16, tag="yg")
                copy2 = nc.vector.tensor_copy if A_MODE == "dvecopy" else nc.scalar.copy
                if gn % 128 == 0:
                    copy2(out=yg[:, : gn], in_=py[:, : gn])
                else:
                    for i in range(nt):
                        rn = min(128, gn - i * 128)
                        copy2(
                            out=yg[:rn, i * 128: (i + 1) * 128],
                            in_=py[:rn, i * 128: (i + 1) * 128])
                yg3 = yg[:].rearrange("p (i c) -> p i c", c=CH)
                # write y rows n -> aug rows n + n // CR, splitting at tile level
                for i in range(nt):
                    r0 = n0 + i * 128
                    rn = min(128, gn - i * 128)
                    kc = r0 // CR
                    ec = (r0 + rn - 1) // CR
                    kb = kc * cfg.n_sent
                    ke = ec * cfg.n_sent
                    if kc == ec:
                        store_eng.dma_start(
                            out=y_aug[r0 + kb: r0 + kb + rn, :], in_=yg3[:rn, i, :])
                    else:
                        split = (kc + 1) * CR - r0       # rows before boundary
                        store_eng.dma_start(
                            out=y_aug[r0 + kb: r0 + kb + split, :],
                            in_=yg3[:split, i, :])
                        store_eng.dma_start(
                            out=y_aug[r0 + split + ke: r0 + ke + rn, :],
                            in_=yg3[split:rn, i, :])

            by_tile = {}
            for (t, k, g0, nb, coff) in insts:
                by_tile.setdefault(t, []).append((k, g0, nb, coff))

            for _rep in range(reps):
              for n0 in range(0, NA, 512):
                emit_group(x_full, n0, min(512, NA - n0), "aug")
              for n0 in range(0, T * 128, 512):
                emit_group(x_own, n0, min(512, T * 128 - n0), "own")

              if windows is not None:
                 # ---------- Phase B + C, chunk-major windows (v3) ----------
                 maccA = consts.tile([128, T * 128], BF16)
                 maccB = consts.tile([128, T * 128], BF16)

                 def emit_phase_c(t, m_ap):
                     aggr = fin.tile([128, CH], F32, tag="aggr")
                     nc.vector.tensor_sub(
                         out=aggr[:],
                         in0=y_own_sb[:, t * 128: (t + 1) * 128], in1=m_ap)
                     ptr = ps_c.tile([128, CH], F32, tag="ctr")
                     nc.tensor.transpose(out=ptr[:], in_=aggr[:],
                                         identity=ident[:])
                     aggrT = fin.tile([128, CH], F32, tag="aggrT")
                     nc.scalar.copy(out=aggrT[:], in_=ptr[:])
                     po = ps_c.tile([128, CH], F32, tag="cmm")
                     nc.tensor.matmul(out=po[:], lhsT=aggrT[:], rhs=wph_sb[:],
                                      start=True, stop=True)
                     osb = fin.tile([128, CH], F32, tag="osb")
                     nc.scalar.copy(out=osb[:], in_=po[:])
                     nc.sync.dma_start(out=out[t * 128: (t + 1) * 128, :],
                                       in_=osb[:])

                 for w in windows:
                     k, WB = w["k"], w["WB"]
                     wc0 = w["insts"][0][2]
                     wc1 = w["insts"][-1][2] + w["insts"][-1][1] * 128 // 16
                     it = gidx.tile([128, max(wc1 - wc0, 16)], I16, tag="it")
                     nc.sync.dma_start(out=it[:, : wc1 - wc0],
                                       in_=idxw[:, wc0:wc1])
                     win = gdst.tile([128, WB * CH], BF16, tag="win",
                                     bufs=int(exp.get("gbufs", 3)))
                     win3 = win[:].rearrange("p (b c) -> p b c", c=CH)
                     for (g0, nb, coff) in w["insts"]:
                         nc.gpsimd.dma_gather(
                             out_ap=win3[:, g0: g0 + nb, :],
                             in_ap=y_aug[k * CS: (k + 1) * CS, :],
                             idxs_ap=it[:, coff - wc0:
                                        coff - wc0 + nb * 128 // 16],
                             num_idxs=nb * 128,
                             num_idxs_reg=nb * 128,
                             elem_size=CH,
                             queue_num=(qc[0] % 4) if exp["queue"] == "rot"
                             else 0,
                         )
                         qc[0] += 1
                     winT = win[:].rearrange("p (b c) -> p c b", c=CH)
                     for (t, boff, btk) in w["reduces"]:
                         sl = winT[:, :, boff: boff + btk]
                         if k == 0:
                             nc.vector.tensor_reduce(
                                 out=maccA[:, t * 128: (t + 1) * 128],
                                 in_=sl, axis=mybir.AxisListType.X,
                                 op=mybir.AluOpType.min)
                         else:
                             mk = fin.tile([128, CH], BF16, tag="mk", bufs=2)
                             nc.vector.tensor_reduce(
                                 out=mk[:], in_=sl, axis=mybir.AxisListType.X,
                                 op=mybir.AluOpType.min)
                             src = maccA if k % 2 == 1 else maccB
                             dst = maccB if k % 2 == 1 else maccA
                             nc.vector.tensor_tensor(
                                 out=dst[:, t * 128: (t + 1) * 128],
                                 in0=src[:, t * 128: (t + 1) * 128],
                                 in1=mk[:], op=mybir.AluOpType.min)
                             if k == NK - 1:
                                 emit_phase_c(
                                     t, dst[:, t * 128: (t + 1) * 128])
                 continue

              # ---------------- Phase B + C per tile ---------------------------
              for t in range(T):
                 c0, c1 = tile_cols[t]
                 it = gidx.tile([128, max(c1 - c0, 16)], I16, tag="it")
                 if c1 > c0:
                     nc.sync.dma_start(out=it[:, : c1 - c0], in_=idxw[:, c0:c1])
                 if exp.get("fold_mode") == "reduce":
                     kws = [k for k in range(NK) if int(B[t, k]) > 0]
                     koff = {}
                     wt = 0
                     for k in kws:
                         koff[k] = wt
                         wt += int(B[t, k])
                     if wt > 0:
                         dk = gdst.tile([128, wt * CH], BF16, tag="gr",
                                        bufs=int(exp.get("gbufs", 3)))
                         dk3 = dk[:].rearrange("p (b c) -> p b c", c=CH)
                     for (k, g0, nb, coff) in by_tile.get(t, []):
                         n_i = nb * 128
                         col0 = koff[k] + g0
                         if exp["gather"] == "seq":
                             r0 = min(k * CS + g0 * 128, NK * CS - n_i)
                             nc.sync.dma_start(
                                 out=dk3[:, col0: col0 + nb, :],
                                 in_=y_aug[r0: r0 + n_i, :].rearrange(
                                     "(i p) c -> p i c", p=128))
                             continue
                         nc.gpsimd.dma_gather(
                             out_ap=dk3[:, col0: col0 + nb, :],
                             in_ap=y_aug[k * CS: (k + 1) * CS, :],
                             idxs_ap=it[:, coff - c0: coff - c0 + n_i // 16],
                             num_idxs=n_i,
                             num_idxs_reg=n_i,
                             elem_size=CH,
                             queue_num=(qc[0] % 4) if exp["queue"] == "rot" else 0,
                         )
                         qc[0] += 1
                     m = fin.tile([128, CH], F32, tag="m")
                     if wt == 0 or exp.get("reduce") == "off":
                         nc.gpsimd.memset(m[:], SENT_VAL)
                     else:
                         dkT = dk[:].rearrange("p (b c) -> p c b", c=CH)
                         nc.vector.tensor_reduce(
                             out=m[:], in_=dkT, axis=mybir.AxisListType.X,
                             op=mybir.AluOpType.min)
                     aggr = fin.tile([128, CH], F32, tag="aggr")
                     nc.vector.tensor_sub(
                         out=aggr[:], in0=y_own_sb[:, t * 128: (t + 1) * 128],
                         in1=m[:])
                     ptr = ps_c.tile([128, CH], F32, tag="ctr")
                     nc.tensor.transpose(out=ptr[:], in_=aggr[:], identity=ident[:])
                     aggrT = fin.tile([128, CH], F32, tag="aggrT")
                     nc.scalar.copy(out=aggrT[:], in_=ptr[:])
                     po = ps_c.tile([128, CH], F32, tag="cmm")
                     nc.tensor.matmul(out=po[:], lhsT=aggrT[:], rhs=wph_sb[:],
                                      start=True, stop=True)
                     osb = fin.tile([128, CH], F32, tag="osb")
                     nc.scalar.copy(out=osb[:], in_=po[:])
                     nc.sync.dma_start(out=out[t * 128: (t + 1) * 128, :],
                                       in_=osb[:])
                     continue
                 dks = {}
                 for k in range(NK):
                     btk = int(B[t, k])
                     if btk == 0:
                         continue
                     dks[k] = gdst.tile([128, btk * CH], BF16, tag=f"g{k}", name=f"dk{k}")
                 for (k, g0, nb, coff) in by_tile.get(t, []):
                     if exp["gather"] == "off":
                         break
                     dk3 = dks[k][:].rearrange("p (b c) -> p b c", c=CH)
                     n_i = nb * 128
                     ndup = 2 if exp["gather"] == "dup" else 1
                     for di in range(ndup):
                         if di == 0:
                             dst = dk3[:, g0: g0 + nb, :]
                         else:
                             ddup = gdst.tile(
                                 [128, (cfg.idx_per_inst // 128) * CH], BF16,
                                 tag="gdup", bufs=2)
                             dst = ddup[:].rearrange(
                                 "p (b c) -> p b c", c=CH)[:, :nb, :]
                         nc.gpsimd.dma_gather(
                             out_ap=dst,
                             in_ap=y_aug[k * CS: (k + 1) * CS, :],
                             idxs_ap=it[:, coff - c0: coff - c0 + n_i // 16],
                             num_idxs=n_i,
                             num_idxs_reg=n_i,
                             elem_size=CH,
                             queue_num=(qc[0] % 4) if exp["queue"] == "rot" else 0,
                         )
                         qc[0] += 1
                 # fold each chunk's rect down to one [128, CH] min
                 mks = []
                 if exp["folds"] == "off":
                     m = fin.tile([128, CH], F32, tag="m")
                     nc.gpsimd.memset(m[:], SENT_VAL)
                     aggr = fin.tile([128, CH], F32, tag="aggr")
                     nc.vector.tensor_sub(
                         out=aggr[:], in0=y_own_sb[:, t * 128: (t + 1) * 128],
                         in1=m[:])
                     ptr = ps_c.tile([128, CH], F32, tag="ctr")
                     nc.tensor.transpose(out=ptr[:], in_=aggr[:], identity=ident[:])
                     aggrT = fin.tile([128, CH], F32, tag="aggrT")
                     nc.vector.tensor_copy(out=aggrT[:], in_=ptr[:])
                     po = ps_c.tile([128, CH], F32, tag="cmm")
                     nc.tensor.matmul(out=po[:], lhsT=aggrT[:], rhs=wph_sb[:],
                                      start=True, stop=True)
                     osb = fin.tile([128, CH], F32, tag="osb")
                     nc.scalar.copy(out=osb[:], in_=po[:])
                     nc.sync.dma_start(out=out[t * 128: (t + 1) * 128, :], in_=osb[:])
                     continue
                 for k in range(NK):
                     if k not in dks:
                         continue
                     cur = dks[k]
                     nb = int(B[t, k])
                     while nb > 1:
                         half = (nb + 1) // 2
                         nxt = fold.tile([128, half * CH], BF16, tag=f"f{k}", bufs=3)
                         nc.vector.tensor_tensor(
                             out=nxt[:, : half * CH],
                             in0=cur[:, : half * CH],
                             in1=cur[:, (nb - half) * CH: nb * CH],
                             op=mybir.AluOpType.min)
                         cur, nb = nxt, half
                     mks.append(cur)
                 m = fin.tile([128, CH], F32, tag="m")
                 if len(mks) == 0:
                     nc.gpsimd.memset(m[:], SENT_VAL)
                 elif len(mks) == 1:
                     nc.vector.tensor_copy(out=m[:], in_=mks[0][:, :CH])
                 else:
                     # sequential accumulate with alternating tags (max 2 live)
                     acc = mks[0]
                     for i in range(1, len(mks) - 1):
                         mm = fold.tile([128, CH], BF16, tag=f"mrg{i % 2}")
                         nc.vector.tensor_tensor(
                             out=mm[:], in0=acc[:, :CH], in1=mks[i][:, :CH],
                             op=mybir.AluOpType.min)
                         acc = mm
                     nc.vector.tensor_tensor(
                         out=m[:], in0=acc[:, :CH], in1=mks[-1][:, :CH],
                         op=mybir.AluOpType.min)
                 # aggr = y_own - m ; out_tile = aggr @ W_phi.T
                 aggr = fin.tile([128, CH], F32, tag="aggr")
                 nc.vector.tensor_sub(
                     out=aggr[:], in0=y_own_sb[:, t * 128: (t + 1) * 128], in1=m[:])
                 ptr = ps_c.tile([128, CH], F32, tag="ctr")
                 nc.tensor.transpose(out=ptr[:], in_=aggr[:], identity=ident[:])
                 aggrT = fin.tile([128, CH], F32, tag="aggrT")
                 nc.vector.tensor_copy(out=aggrT[:], in_=ptr[:])
                 po = ps_c.tile([128, CH], F32, tag="cmm")
                 nc.tensor.matmul(out=po[:], lhsT=aggrT[:], rhs=wph_sb[:],
                                  start=True, stop=True)
                 osb = fin.tile([128, CH], F32, tag="osb")
                 nc.scalar.copy(out=osb[:], in_=po[:])
                 nc.sync.dma_start(out=out[t * 128: (t + 1) * 128, :], in_=osb[:])

            if phases != "abc":
                fillz = consts.tile([128, CH], F32)
                nc.vector.tensor_copy(out=fillz[:], in_=probe[:])
                for t in range(T):
                    nc.sync.dma_start(out=out[t * 128: (t + 1) * 128, :], in_=fillz[:])
    nc.compile()
    return nc


_CACHE = {}


def _get_runner_and_plan(x, edge_index, cfg: Cfg, reps=1, phases="abc", exp=None):
    plan, idxw, x_own, own_nodes, deg, x_perm = prep(x, edge_index, cfg)
    skey = (cfg, reps, phases, tuple(sorted((exp or {}).items())),
            tuple(plan["B"].reshape(-1).tolist()))
    if skey not in _CACHE:
        nc = build_program(plan, reps=reps, phases=phases, exp=exp)
        _CACHE[skey] = BassRunner(nc, cfg.n_cores)
    return _CACHE[skey], plan, idxw, x_own, own_nodes, deg, x_perm


def run_cfg(x, edge_index, W_theta, W_phi, cfg: Cfg, time_iters=0, reps=1, phases="abc", exp=None):
    runner, plan, idxw, x_own, own_nodes, deg, x_perm = _get_runner_and_plan(x, edge_index, cfg, reps=reps, phases=phases, exp=exp)
    if exp and exp.get("sent_idx"):
        idxw = np.full_like(idxw, cfg.chunk_real)
    wtt = np.ascontiguousarray(np.asarray(W_theta, np.float32).T)
    wpt = np.ascontiguousarray(np.asarray(W_phi, np.float32).T)
    in_maps = [
        dict(x_full=x_perm, x_own=x_own[c], w_theta_t=wtt, w_phi_t=wpt,
             idxw=np.ascontiguousarray(idxw[c]) if plan["W_total"] > 0
             else np.zeros((128, 16), np.int16))
        for c in range(cfg.n_cores)
    ]
    runner.prepare(in_maps)
    outs = runner.run()
    t_ns = runner.time_ns(iters=time_iters) if time_iters else None
    res = runner.results(outs)
    out_full = np.empty((cfg.N, CH), np.float32)
    for c in range(cfg.n_cores):
        out_full[own_nodes[c]] = res[c]["out"][: cfg.npc]
    out_full[deg == 0] = 0.0
    return out_full, t_ns


def kernel(x, edge_index, W_theta, W_phi):
    out, _ = run_cfg(x, edge_index, W_theta, W_phi, Cfg())
    return out



# revision 29
# speedup vs baseline: 3.7438x; 1.5302x over previous
"""Trainium2 Bass kernel for DevConv-style GNN message passing.

Reference computation:
    rel_t = (x[row] - x[col]) @ W_theta.T          # [E, 128]
    aggr  = segment_max(rel_t, row, N)             # [N, 128], empty -> 0
    out   = aggr @ W_phi.T                         # [N, 128]

Key reformulation: with y = x @ W_theta.T, within a segment (fixed row d)
    max_e (y[d] - y[col_e]) = y[d] - min_e y[col_e]     (per channel)
so the per-edge matmul disappears and only ONE gather per edge (y[col]) is
needed, followed by a segmented min.

The gather is bound by SWDGE descriptor generation on the Pool engine
(one descriptor pair per gathered row, generated by the Q7 core pair that
queue_num selects), so runtime ~ padded-slot count x per-idx gen cost.
What makes it fast (measured on HW, baseline 3.69 ms -> ~1.0-1.8 ms
depending on machine load):
  * queue rotation over all 4 SWDGE queues overlaps descriptor generation
    across the 4 Q7 core pairs (~3x aggregate gen rate);
  * 2048-idx instructions with single_packet=False halve the ~1 us/inst
    fixed cost (single_packet=True crashes the mesh at >=2048 idx);
  * pad slots point at 128 sentinel rows per chunk, not one — a single
    sentinel row serializes 27% of the drain on one HBM bank (+4.4 ms!);
  * balanced chunk coloring (int16 idx -> 4 chunks of 25088 rows) spreads
    each destination's edges ~deg/4 per chunk;
  * tiles sorted by max per-chunk count then count vector: padding
    inflation 1.95x -> 1.16x;
  * chunk-major windowed phase B (v3): per chunk, win_tiles=12 tiles'
    rects are one contiguous SBUF window, so gather instructions span
    tile boundaries at ~full 2048-idx occupancy, and phase A (chunks
    1..3) overlaps with phase-B gathers of earlier chunks.

Per core:
  Phase A: host supplies x^T (bf16, table order); y^T tiles come from
           direct PE matmuls (no transposes), are PE-transposed once and
           stored as bf16 rows into the chunked HBM table y_aug; stores
           issue on the Act HWDGE so phase-B idx loads don't queue behind
           them on SP. y_own is computed into SBUF the same way (no
           transpose needed).
  Phase B: per (chunk, window): dma_gathers fill the window rect; per
           tile a strided 1-input DVE tensor_reduce(min) gives the chunk-
           partial m, min-accumulated across chunks into ping-pong bf16
           accumulators (1-input reduce keeps the DVE out of 2-port perf
           mode, which would lock GpSimd off the shared SBUF port and
           stall SWDGE descriptor generation).
  Phase C: aggr = y_own - m; out_tile = aggr @ W_phi.T via PE transpose
           + matmul, emitted as soon as a tile's last chunk is folded.
Host un-permutes the concatenated core outputs and zeroes empty nodes.
"""
import sys
import os

sys.path.insert(0, "/opt/trn_rl_repo")

from contextlib import ExitStack
from dataclasses import dataclass

import numpy as np
import ml_dtypes

import concourse.bass as bass
import concourse.tile as tile
from concourse import bacc, mybir
from concourse.masks import make_identity

import time

import jax
from jax.sharding import Mesh, PartitionSpec
from jax.experimental.shard_map import shard_map

from concourse.bass2jax import (
    _bass_exec_p, install_neuronx_cc_hook, partition_id_tensor)


class BassRunner:
    """Keeps a jitted PJRT executable for a Bass program so it can be run
    repeatedly on device-resident inputs (for wall-clock timing)."""

    def __init__(self, nc, n_cores: int):
        install_neuronx_cc_hook()
        self.nc = nc
        self.n_cores = n_cores
        partition_name = nc.partition_id_tensor.name if nc.partition_id_tensor else None
        in_names, out_names, out_avals = [], [], []
        for alloc in nc.m.functions[0].allocations:
            if not isinstance(alloc, mybir.MemoryLocationSet):
                continue
            name = alloc.memorylocations[0].name
            if alloc.kind == "ExternalInput":
                if name != partition_name:
                    in_names.append(name)
            elif alloc.kind == "ExternalOutput":
                out_names.append(name)
                out_avals.append(jax.core.ShapedArray(
                    tuple(alloc.tensor_shape), mybir.dt.np(alloc.dtype)))
        self.in_names, self.out_names, self.out_avals = in_names, out_names, out_avals
        self.n_params = len(in_names)
        all_in_names = list(in_names) + list(out_names)
        if partition_name is not None:
            all_in_names.append(partition_name)

        def _body(*args):
            operands = list(args)
            if partition_name is not None:
                operands.append(partition_id_tensor())
            outs = _bass_exec_p.bind(
                *operands,
                out_avals=tuple(out_avals),
                in_names=tuple(all_in_names),
                out_names=tuple(out_names),
                lowering_input_output_aliases=(),
                sim_require_finite=True,
                sim_require_nnan=True,
                nc=nc,
            )
            return tuple(outs)

        devices = jax.devices()[:n_cores]
        self.mesh = Mesh(np.asarray(devices), ("core",))
        n_outs = len(out_names)
        in_specs = (PartitionSpec("core"),) * (self.n_params + n_outs)
        out_specs = (PartitionSpec("core"),) * n_outs
        self.fn = jax.jit(
            shard_map(_body, mesh=self.mesh, in_specs=in_specs,
                      out_specs=out_specs, check_rep=False),
            keep_unused=True,
        )
        self._dev_args = None

    def prepare(self, in_maps):
        assert len(in_maps) == self.n_cores
        concat_in = [
            np.concatenate([np.asarray(in_maps[c][n]) for c in range(self.n_cores)],
                           axis=0)
            for n in self.in_names
        ]
        concat_zeros = [
            np.zeros((self.n_cores * a.shape[0], *a.shape[1:]), a.dtype)
            for a in self.out_avals
        ]
        sharding = jax.sharding.NamedSharding(self.mesh, PartitionSpec("core"))
        self._dev_args = [jax.device_put(v, sharding) for v in concat_in + concat_zeros]
        return self

    def run(self):
        outs = self.fn(*self._dev_args)
        jax.block_until_ready(outs)
        return outs

    def results(self, outs):
        return [
            {n: np.asarray(outs[i]).reshape(self.n_cores, *self.out_avals[i].shape)[c]
             for i, n in enumerate(self.out_names)}
            for c in range(self.n_cores)
        ]

    def time_ns(self, iters=5, warmup=2):
        for _ in range(warmup):
            self.run()
        ts = []
        for _ in range(iters):
            t0 = time.perf_counter()
            self.run()
            ts.append((time.perf_counter() - t0) * 1e9)
        return min(ts)


CH = 128
F32 = mybir.dt.float32
BF16 = mybir.dt.bfloat16
I16 = mybir.dt.int16
SENT_VAL = 3.0e38
IDX_PER_INST = 1024          # dma_gather crashes >= 2048 idx/instruction
BLK_PER_INST = IDX_PER_INST // 128


@dataclass(frozen=True)
class Cfg:
    N: int = 100_000
    E: int = 3_200_000
    n_cores: int = 8
    chunk_real: int = 25_088     # table rows per chunk (512-aligned, < int16 max)
    idx_per_inst: int = 2048     # needs single_packet=False (spf) to not crash
    balanced: bool = True        # balanced chunk coloring + count-vector tiling
    n_sent: int = 128            # sentinel rows per chunk (pad reads spread
                                 # over HBM banks instead of hammering one row)
    layout: str = "v3"           # "v3": chunk-major windowed phase B
                                 # "tile": original per-tile rects
    win_tiles: int = 12          # tiles per gather window (v3)

    @property
    def n_chunks(self):
        return (self.N + self.chunk_real - 1) // self.chunk_real

    @property
    def chunk_stride(self):
        return self.chunk_real + self.n_sent

    @property
    def npc(self):
        assert self.N % self.n_cores == 0
        return self.N // self.n_cores

    @property
    def tiles(self):
        return (self.npc + 127) // 128


def _wrap16(seg: np.ndarray) -> np.ndarray:
    """Per-instruction idx wrap: flat [n] -> [128, n//16]; idx i at
    (partition i%16, col i//16), replicated across the 8 gpsimd groups."""
    n = seg.shape[-1]
    w = seg.reshape(*seg.shape[:-1], n // 16, 16)
    w = np.swapaxes(w, -1, -2)                       # [..., 16, n//16]
    return np.tile(w, (1,) * (seg.ndim - 1) + (8, 1))


def _color_chunks(row, col, deg, N, NK, cap):
    """Greedy quota-balanced assignment of cols to NK chunks; returns
    (chunk_of, cnt_dk) where cnt_dk[d,k] = per-dest per-chunk edge count."""
    o = np.argsort(col, kind="stable")
    dest_s = row[o]
    starts = np.searchsorted(col[o], np.arange(N + 1))
    quota = -(-deg // NK)
    cnt_dk = np.zeros((N, NK), np.int32)
    chunk_of = np.full(N, -1, np.int64)
    chunk_sz = np.zeros(NK, np.int64)
    col_order = np.argsort(-(starts[1:] - starts[:-1]), kind="stable")
    for _ in range(2):
        for c in col_order:
            ds = dest_s[starts[c]: starts[c + 1]]
            kprev = chunk_of[c]
            if kprev >= 0:
                if len(ds):
                    cnt_dk[ds, kprev] -= 1
                chunk_sz[kprev] -= 1
            if len(ds) == 0:
                k = int(np.argmin(chunk_sz))
            else:
                cc = cnt_dk[ds]
                over = np.maximum(0, cc + 1 - quota[ds][:, None])
                sc = (over * 1000.0 + cc).sum(axis=0).astype(np.float64)
                sc += chunk_sz * 1e-4
                sc[chunk_sz >= cap] = 1e18
                k = int(np.argmin(sc))
                cnt_dk[ds, k] += 1
            chunk_of[c] = k
            chunk_sz[k] += 1
    return chunk_of, cnt_dk


def prep(x, edge_index, cfg: Cfg):
    """Host-side data prep. Returns (plan, per-core inputs, unpermute info)."""
    N, E, NC = cfg.N, cfg.E, cfg.n_cores
    CR, NK, T = cfg.chunk_real, cfg.n_chunks, cfg.tiles
    row = np.asarray(edge_index[0], dtype=np.int64)
    col = np.asarray(edge_index[1], dtype=np.int64)

    deg = np.bincount(row, minlength=N)
    x_np0 = np.asarray(x, dtype=np.float32)
    if cfg.balanced:
        chunk_of, cnt_dk = _color_chunks(row, col, deg, N, NK, CR)
        # rank within chunk
        oc = np.argsort(chunk_of, kind="stable")
        rank_of = np.empty(N, np.int64)
        csz = np.bincount(chunk_of, minlength=NK)
        cstart = np.concatenate([[0], np.cumsum(csz)])
        rank_of[oc] = np.arange(N) - cstart[chunk_of[oc]]
        # dest order: group by max per-chunk count (the rect width driver),
        # then by the full count vector — tiles get near-identical vectors
        order = np.lexsort(tuple(cnt_dk[:, k] for k in range(NK - 1, -1, -1))
                           + (-cnt_dk.max(axis=1),))
        # x permuted into table layout [NK*CR, CH]
        x_perm = np.zeros((NK * CR, x_np0.shape[1]), np.float32)
        x_perm[chunk_of * CR + rank_of] = x_np0
    else:
        order = np.argsort(-deg, kind="stable")      # node ids by desc degree
        x_perm = x_np0
    core_of = np.empty(N, np.int64)
    pos_of = np.empty(N, np.int64)
    r = np.arange(N)
    core_of[order] = r % NC
    pos_of[order] = r // NC

    ec = core_of[row]
    ep = pos_of[row]
    if cfg.balanced:
        ek = chunk_of[col]
        elocal = rank_of[col].astype(np.int16)
    else:
        ek = col // CR
        elocal = (col - ek * CR).astype(np.int16)
    et = ep // 128
    ed = ep % 128

    # per-(core,tile,chunk,node) counts and within-group slot index j
    key = ((ec * T + et) * NK + ek) * 128 + ed
    o = np.argsort(key, kind="stable")
    ks = key[o]
    first = np.r_[True, ks[1:] != ks[:-1]]
    run_id = np.cumsum(first) - 1
    run_start = np.flatnonzero(first)
    j = np.arange(E) - run_start[run_id]

    cnt = np.bincount(key, minlength=NC * T * NK * 128).reshape(NC, T, NK, 128)
    B = cnt.max(axis=(0, 3)).astype(np.int64)        # [T, NK] shared structure
    B = np.maximum(B, 1)         # every (t,k) rect non-empty (simplifies accum)

    Bf = B.reshape(-1)
    off = np.concatenate([[0], np.cumsum(Bf * 128)])  # slot offset per (t,k)
    total_slots = int(off[-1])

    # pad slots point at one of n_sent sentinel rows (spread over HBM banks)
    pad_vals = (CR + (np.arange(total_slots) % cfg.n_sent)).astype(np.int16)
    idx_all = np.broadcast_to(pad_vals, (NC, total_slots)).copy()
    tk = et[o] * NK + ek[o]
    pos_in = off[tk] + j * 128 + ed[o]
    idx_all[ec[o], pos_in] = elocal[o]

    # split into gather instructions and build wrapped idx input
    blk_per_inst = cfg.idx_per_inst // 128
    insts = []           # (t, k, g0blk, nblk, col_off)
    tile_cols = []       # per tile: (col_start, col_end)
    windows = None
    wsegs = []
    col_off = 0
    if cfg.layout == "v3":
        # chunk-major layout: per chunk, all tiles' rects back-to-back;
        # instructions span tile boundaries within a window of win_tiles.
        Bc = np.ascontiguousarray(B.T)               # [NK, T]
        cb = np.concatenate([[0], np.cumsum(Bc.sum(axis=1) * 128)])
        off3 = np.zeros((NK, T), np.int64)
        for k in range(NK):
            off3[k] = cb[k] + np.concatenate(
                [[0], np.cumsum(Bc[k][:-1] * 128)])
        total3 = int(cb[-1])
        pad_vals = (CR + (np.arange(total3) % cfg.n_sent)).astype(np.int16)
        idx_all = np.broadcast_to(pad_vals, (NC, total3)).copy()
        pos_in = off3[ek[o], et[o]] + j * 128 + ed[o]
        idx_all[ec[o], pos_in] = elocal[o]
        windows = []
        for k in range(NK):
            for t0 in range(0, T, cfg.win_tiles):
                t1 = min(t0 + cfg.win_tiles, T)
                WB = int(Bc[k, t0:t1].sum())
                base_slot = int(off3[k, t0])
                inst_list = []
                for g0 in range(0, WB, blk_per_inst):
                    nb = min(blk_per_inst, WB - g0)
                    seg = idx_all[:, base_slot + g0 * 128:
                                  base_slot + (g0 + nb) * 128]
                    wsegs.append(_wrap16(seg))
                    inst_list.append((g0, nb, col_off))
                    col_off += nb * 128 // 16
                reduces = []
                acc = 0
                for t in range(t0, t1):
                    reduces.append((t, acc, int(Bc[k, t])))
                    acc += int(Bc[k, t])
                windows.append(dict(k=k, WB=WB, insts=inst_list,
                                    reduces=reduces))
    else:
        for t in range(T):
            t_start = col_off
            for k in range(NK):
                btk = int(B[t, k])
                base = int(off[t * NK + k])
                for g0 in range(0, btk, blk_per_inst):
                    nb = min(blk_per_inst, btk - g0)
                    n_i = nb * 128
                    seg = idx_all[:, base + g0 * 128: base + g0 * 128 + n_i]
                    wsegs.append(_wrap16(seg))
                    insts.append((t, k, g0, nb, col_off))
                    col_off += n_i // 16
            tile_cols.append((t_start, col_off))
    idxw = np.concatenate(wsegs, axis=2) if wsegs else np.zeros((NC, 128, 0), np.int16)
    W_total = idxw.shape[2]

    # per-core x_own in pos order, padded to T*128 rows
    own_nodes = np.empty((NC, cfg.npc), np.int64)
    own_nodes[core_of[order], pos_of[order]] = order  # own_nodes[c, p] = node id
    x_np = np.asarray(x, dtype=np.float32)
    x_own = np.zeros((NC, T * 128, CH), np.float32)
    x_own[:, : cfg.npc] = x_np[own_nodes]

    x_permT = np.ascontiguousarray(x_perm.T).astype(ml_dtypes.bfloat16)
    x_ownT = np.ascontiguousarray(np.transpose(x_own, (0, 2, 1))).astype(
        ml_dtypes.bfloat16)
    plan = dict(cfg=cfg, B=B, insts=insts, tile_cols=tile_cols, W_total=W_total,
                windows=windows)
    return plan, idxw, x_own, own_nodes, deg, x_perm, x_permT, x_ownT


def build_program(plan, reps=1, phases="abc", exp=None):
    exp = {**dict(gather="on", folds="on", queue="rot", fold_mode="reduce",
                  a_store="act", spf=1),
           **(exp or {})}
    cfg: Cfg = plan["cfg"]
    N, NK, CR, T = cfg.N, cfg.n_chunks, cfg.chunk_real, cfg.tiles
    CS = cfg.chunk_stride
    B, insts, tile_cols, W_total = (
        plan["B"], plan["insts"], plan["tile_cols"], plan["W_total"])

    windows = plan.get("windows")
    nc = bacc.Bacc(None, target_bir_lowering=False, num_swdge_queues=4,
                   dynamic_dma_scratch_size=int(exp.get("scratch", 16384)))
    NA = NK * CR if cfg.balanced else N      # phase-A row count (table layout)
    axt = windows is not None and exp.get("axt", 1)
    if axt:
        x_fullT = nc.declare_dram_parameter("x_fullT", [CH, NA], BF16,
                                            isOutput=False)
        x_ownT = nc.declare_dram_parameter("x_ownT", [CH, T * 128], BF16,
                                           isOutput=False)
        wthb = nc.declare_dram_parameter("w_theta_b", [CH, CH], BF16,
                                         isOutput=False)
    else:
        x_full = nc.declare_dram_parameter("x_full", [NA, CH], F32, isOutput=False)
        x_own = nc.declare_dram_parameter("x_own", [T * 128, CH], F32, isOutput=False)
        wth = nc.declare_dram_parameter("w_theta_t", [CH, CH], F32, isOutput=False)
    wph = nc.declare_dram_parameter("w_phi_t", [CH, CH], F32, isOutput=False)
    idxw = nc.declare_dram_parameter("idxw", [128, max(W_total, 16)], I16, isOutput=False)
    out = nc.declare_dram_parameter("out", [T * 128, CH], F32, isOutput=True)

    qc = [0]  # gather queue rotation

    with tile.TileContext(nc) as tc:
        with ExitStack() as ctx:
            consts = ctx.enter_context(tc.tile_pool(name="consts", bufs=1))
            dram = ctx.enter_context(tc.tile_pool(name="dram", bufs=1, space="DRAM"))
            ax = ctx.enter_context(tc.tile_pool(name="ax", bufs=2))
            axT = ctx.enter_context(tc.tile_pool(name="axT", bufs=2))
            ay = ctx.enter_context(tc.tile_pool(name="ay", bufs=2))
            ps_t = ctx.enter_context(tc.tile_pool(name="ps_t", bufs=2, space="PSUM"))
            ps_y = ctx.enter_context(tc.tile_pool(name="ps_y", bufs=2, space="PSUM"))
            ps_c = ctx.enter_context(tc.tile_pool(name="ps_c", bufs=2, space="PSUM"))
            gidx = ctx.enter_context(tc.tile_pool(name="gidx", bufs=4))
            gdst = ctx.enter_context(tc.tile_pool(name="gdst", bufs=2))
            fold = ctx.enter_context(tc.tile_pool(name="fold", bufs=2))
            fin = ctx.enter_context(tc.tile_pool(name="fin", bufs=2))

            y_aug = dram.tile([NK * CS, CH], BF16)

            ident = consts.tile([128, 128], F32)
            make_identity(nc, ident[:])
            if axt:
                wth_sb = consts.tile([CH, CH], BF16)
                nc.sync.dma_start(out=wth_sb[:], in_=wthb[:])
            else:
                wth_sb = consts.tile([CH, CH], F32)
                nc.sync.dma_start(out=wth_sb[:], in_=wth[:])
            wph_sb = consts.tile([CH, CH], F32)
            nc.sync.dma_start(out=wph_sb[:], in_=wph[:])
            y_own_sb = consts.tile([128, T * 128],
                                   BF16 if windows is not None else F32)
            probe = consts.tile([128, CH], BF16)
            nc.gpsimd.memset(probe[:], 0.0)
            cst128 = consts.tile([128, CH], BF16)
            nc.gpsimd.memset(cst128[:], 1.0)
            sent = consts.tile([cfg.n_sent, CH], BF16)
            nc.gpsimd.memset(sent[:], SENT_VAL)
            for k in range(NK):
                nc.sync.dma_start(
                    out=y_aug[k * CS + CR: k * CS + CR + cfg.n_sent, :],
                    in_=sent[:])

            A_MODE = os.environ.get("A_MODE", "full")
            PS_BUFS = int(os.environ.get("PS_BUFS", "2"))
            # ---------------- Phase A: y_aug = (x @ W_theta.T).bf16 ----------
            def emit_group(src, n0, gn, dst):
                """Process rows [n0, n0+gn) of src -> y into dst.
                dst = ("aug",) writes y_aug rows (with chunk-boundary split),
                dst = ("own",) writes y_own_sb cols."""
                nt = (gn + 127) // 128
                xg = ax.tile([128, nt * 128], F32, tag="xg",
                             bufs=int(os.environ.get("XG_BUFS", "2")))
                xg3 = xg[:].rearrange("p (i c) -> p i c", c=CH)
                load_eng = nc.gpsimd if A_MODE == "dma3" else nc.sync
                store_eng = nc.scalar if exp.get("a_store") == "act" else nc.sync
                if gn % 128 == 0:
                    load_eng.dma_start(
                        out=xg3[:, :nt, :],
                        in_=src[n0: n0 + gn, :].rearrange("(i p) c -> p i c", p=128))
                else:
                    for i in range(nt):
                        rn = min(128, gn - i * 128)
                        nc.sync.dma_start(
                            out=xg3[:rn, i, :],
                            in_=src[n0 + i * 128: n0 + i * 128 + rn, :])
                if A_MODE.startswith("dma") and dst == "aug":
                    # dma : load->store dep, both on sync
                    # dma2: stores only dep-free (loads still emitted)
                    # dma3: load on gpsimd, stores dep on load, on sync
                    # dma5: loads only (no stores)
                    if gn % 128 == 0:
                        for i in range(nt):
                            r0 = n0 + i * 128
                            kb = (r0 // CR) * cfg.n_sent
                            if A_MODE == "dma5":
                                continue
                            src_ap = (cst128[:] if A_MODE == "dma2"
                                      else xg3[:, i, :CH // 2].bitcast(BF16))
                            nc.sync.dma_start(
                                out=y_aug[r0 + kb: r0 + kb + 128, :], in_=src_ap)
                    return
                pt = ps_t.tile([128, nt * 128], F32, tag="pt", bufs=PS_BUFS)
                for i in range(nt):
                    rn = min(128, gn - i * 128)
                    nc.tensor.transpose(
                        out=pt[:, i * 128: i * 128 + rn],
                        in_=xg3[:rn, i, :],
                        identity=ident[:rn, :rn])
                xT = axT.tile([128, nt * 128], F32, tag="xT")
                if exp.get("fold_mode") == "reduce":
                    nc.scalar.copy(out=xT[:, : nt * 128], in_=pt[:, : nt * 128])
                else:
                    nc.vector.tensor_copy(out=xT[:, : nt * 128], in_=pt[:, : nt * 128])
                if A_MODE == "nomm" and dst == "aug":
                    for i in range(nt):
                        r0 = n0 + i * 128
                        kb = (r0 // CR) * cfg.n_sent
                        nc.sync.dma_start(
                            out=y_aug[r0 + kb: r0 + kb + 128, :],
                            in_=xT[:, i * 128: i * 128 + 128][:, :CH // 2].bitcast(BF16))
                    return
                py = ps_y.tile([128, nt * 128], F32, tag="py", bufs=PS_BUFS)
                for i in range(nt):
                    rn = min(128, gn - i * 128)
                    nc.tensor.matmul(
                        out=py[:rn, i * 128: (i + 1) * 128],
                        lhsT=xT[:, i * 128: i * 128 + rn],
                        rhs=wth_sb[:],
                        start=True, stop=True)
                if dst == "own":
                    nc.scalar.copy(
                        out=y_own_sb[:, n0: n0 + nt * 128], in_=py[:, : nt * 128])
                    return
                yg = ay.tile([128, nt * 128], BF16, tag="yg")
                copy2 = nc.vector.tensor_copy if A_MODE == "dvecopy" else nc.scalar.copy
                if gn % 128 == 0:
                    copy2(out=yg[:, : gn], in_=py[:, : gn])
                else:
                    for i in range(nt):
                        rn = min(128, gn - i * 128)
                        copy2(
                            out=yg[:rn, i * 128: (i + 1) * 128],
                            in_=py[:rn, i * 128: (i + 1) * 128])
                yg3 = yg[:].rearrange("p (i c) -> p i c", c=CH)
                # write y rows n -> aug rows n + n // CR, splitting at tile level
                for i in range(nt):
                    r0 = n0 + i * 128
                    rn = min(128, gn - i * 128)
                    kc = r0 // CR
                    ec = (r0 + rn - 1) // CR
                    kb = kc * cfg.n_sent
                    ke = ec * cfg.n_sent
                    if kc == ec:
                        store_eng.dma_start(
                            out=y_aug[r0 + kb: r0 + kb + rn, :], in_=yg3[:rn, i, :])
                    else:
                        split = (kc + 1) * CR - r0       # rows before boundary
                        store_eng.dma_start(
                            out=y_aug[r0 + kb: r0 + kb + split, :],
                            in_=yg3[:split, i, :])
                        store_eng.dma_start(
                            out=y_aug[r0 + split + ke: r0 + ke + rn, :],
                            in_=yg3[split:rn, i, :])

            by_tile = {}
            for (t, k, g0, nb, coff) in insts:
                by_tile.setdefault(t, []).append((k, g0, nb, coff))

            def emit_group_xt(srcT, n0, gn, dst):
                """Transpose-free phase A: srcT is [128 in_ch, nodes] bf16;
                y tile = (xT_tile)^T @ W_theta^T via direct PE matmuls."""
                nt = (gn + 127) // 128
                xg = ax.tile([128, nt * 128], BF16, tag="xgt")
                nc.sync.dma_start(out=xg[:, :gn], in_=srcT[:, n0: n0 + gn])
                py = ps_y.tile([128, nt * 128], F32, tag="pyt", bufs=PS_BUFS)
                for i in range(nt):
                    rn = min(128, gn - i * 128)
                    nc.tensor.matmul(
                        out=py[:rn, i * 128: (i + 1) * 128],
                        lhsT=xg[:, i * 128: i * 128 + rn],
                        rhs=wth_sb[:], start=True, stop=True)
                if dst == "own":
                    nc.scalar.copy(out=y_own_sb[:, n0: n0 + nt * 128],
                                   in_=py[:, : nt * 128])
                    return
                yg = ay.tile([128, nt * 128], BF16, tag="ygt")
                nc.scalar.copy(out=yg[:, : nt * 128], in_=py[:, : nt * 128])
                yg3 = yg[:].rearrange("p (i c) -> p i c", c=CH)
                store_eng = nc.scalar if exp.get("a_store") == "act" else nc.sync
                for i in range(nt):
                    r0 = n0 + i * 128
                    rn = min(128, gn - i * 128)
                    kc = r0 // CR
                    ec = (r0 + rn - 1) // CR
                    kb = kc * cfg.n_sent
                    ke = ec * cfg.n_sent
                    if kc == ec:
                        store_eng.dma_start(
                            out=y_aug[r0 + kb: r0 + kb + rn, :],
                            in_=yg3[:rn, i, :])
                    else:
                        split = (kc + 1) * CR - r0
                        store_eng.dma_start(
                            out=y_aug[r0 + kb: r0 + kb + split, :],
                            in_=yg3[:split, i, :])
                        store_eng.dma_start(
                            out=y_aug[r0 + split + ke: r0 + ke + rn, :],
                            in_=yg3[split:rn, i, :])

            for _rep in range(reps):
              if axt:
                for n0 in range(0, NA, 512):
                    emit_group_xt(x_fullT, n0, min(512, NA - n0), "aug")
                for n0 in range(0, T * 128, 512):
                    emit_group_xt(x_ownT, n0, min(512, T * 128 - n0), "own")
              else:
                for n0 in range(0, NA, 512):
                    emit_group(x_full, n0, min(512, NA - n0), "aug")
                for n0 in range(0, T * 128, 512):
                    emit_group(x_own, n0, min(512, T * 128 - n0), "own")

              if windows is not None:
                 # ---------- Phase B + C, chunk-major windows (v3) ----------
                 maccA = consts.tile([128, T * 128], BF16)
                 maccB = consts.tile([128, T * 128], BF16)

                 def emit_phase_c(t, m_ap):
                     aggr = fin.tile([128, CH], F32, tag="aggr")
                     nc.vector.tensor_sub(
                         out=aggr[:],
                         in0=y_own_sb[:, t * 128: (t + 1) * 128], in1=m_ap)
                     ptr = ps_c.tile([128, CH], F32, tag="ctr")
                     nc.tensor.transpose(out=ptr[:], in_=aggr[:],
                                         identity=ident[:])
                     aggrT = fin.tile([128, CH], F32, tag="aggrT")
                     nc.scalar.copy(out=aggrT[:], in_=ptr[:])
                     po = ps_c.tile([128, CH], F32, tag="cmm")
                     nc.tensor.matmul(out=po[:], lhsT=aggrT[:], rhs=wph_sb[:],
                                      start=True, stop=True)
                     osb = fin.tile([128, CH], F32, tag="osb")
                     nc.scalar.copy(out=osb[:], in_=po[:])
                     nc.sync.dma_start(out=out[t * 128: (t + 1) * 128, :],
                                       in_=osb[:])

                 for w in windows:
                     k, WB = w["k"], w["WB"]
                     wc0 = w["insts"][0][2]
                     wc1 = w["insts"][-1][2] + w["insts"][-1][1] * 128 // 16
                     it = gidx.tile([128, max(wc1 - wc0, 16)], I16, tag="it")
                     nc.sync.dma_start(out=it[:, : wc1 - wc0],
                                       in_=idxw[:, wc0:wc1])
                     win = gdst.tile([128, WB * CH], BF16, tag="win",
                                     bufs=int(exp.get("gbufs", 2)))
                     win3 = win[:].rearrange("p (b c) -> p b c", c=CH)
                     for (g0, nb, coff) in w["insts"]:
                         if exp["gather"] == "seq":
                             r0 = min(k * CS + g0 * 128, NK * CS - nb * 128)
                             nc.sync.dma_start(
                                 out=win3[:, g0: g0 + nb, :],
                                 in_=y_aug[r0: r0 + nb * 128, :].rearrange(
                                     "(i p) c -> p i c", p=128))
                             continue
                         nc.gpsimd.dma_gather(
                             out_ap=win3[:, g0: g0 + nb, :],
                             in_ap=y_aug[k * CS: (k + 1) * CS, :],
                             idxs_ap=it[:, coff - wc0:
                                        coff - wc0 + nb * 128 // 16],
                             num_idxs=nb * 128,
                             num_idxs_reg=nb * 128,
                             elem_size=CH,
                             single_packet=not bool(exp.get("spf")),
                             queue_num=(qc[0] % 4) if exp["queue"] == "rot"
                             else 0,
                         )
                         qc[0] += 1
                     winT = win[:].rearrange("p (b c) -> p c b", c=CH)
                     for (t, boff, btk) in w["reduces"]:
                         if exp.get("reduce") == "off":
                             sl = winT[:, :, 0:1]
                         else:
                             sl = winT[:, :, boff: boff + btk]
                         if k == 0:
                             nc.vector.tensor_reduce(
                                 out=maccA[:, t * 128: (t + 1) * 128],
                                 in_=sl, axis=mybir.AxisListType.X,
                                 op=mybir.AluOpType.min)
                         else:
                             mk = fin.tile([128, CH], BF16, tag="mk", bufs=2)
                             nc.vector.tensor_reduce(
                                 out=mk[:], in_=sl, axis=mybir.AxisListType.X,
                                 op=mybir.AluOpType.min)
                             src = maccA if k % 2 == 1 else maccB
                             dst = maccB if k % 2 == 1 else maccA
                             nc.vector.tensor_tensor(
                                 out=dst[:, t * 128: (t + 1) * 128],
                                 in0=src[:, t * 128: (t + 1) * 128],
                                 in1=mk[:], op=mybir.AluOpType.min)
                             if k == NK - 1:
                                 emit_phase_c(
                                     t, dst[:, t * 128: (t + 1) * 128])
                 continue

              # ---------------- Phase B + C per tile ---------------------------
              for t in range(T):
                 c0, c1 = tile_cols[t]
                 it = gidx.tile([128, max(c1 - c0, 16)], I16, tag="it")
                 if c1 > c0:
                     nc.sync.dma_start(out=it[:, : c1 - c0], in_=idxw[:, c0:c1])
                 if exp.get("fold_mode") == "reduce":
                     kws = [k for k in range(NK) if int(B[t, k]) > 0]
                     koff = {}
                     wt = 0
                     for k in kws:
                         koff[k] = wt
                         wt += int(B[t, k])
                     if wt > 0:
                         dk = gdst.tile([128, wt * CH], BF16, tag="gr",
                                        bufs=int(exp.get("gbufs", 3)))
                         dk3 = dk[:].rearrange("p (b c) -> p b c", c=CH)
                     for (k, g0, nb, coff) in by_tile.get(t, []):
                         n_i = nb * 128
                         col0 = koff[k] + g0
                         if exp["gather"] == "seq":
                             r0 = min(k * CS + g0 * 128, NK * CS - n_i)
                             nc.sync.dma_start(
                                 out=dk3[:, col0: col0 + nb, :],
                                 in_=y_aug[r0: r0 + n_i, :].rearrange(
                                     "(i p) c -> p i c", p=128))
                             continue
                         nc.gpsimd.dma_gather(
                             out_ap=dk3[:, col0: col0 + nb, :],
                             in_ap=y_aug[k * CS: (k + 1) * CS, :],
                             idxs_ap=it[:, coff - c0: coff - c0 + n_i // 16],
                             num_idxs=n_i,
                             num_idxs_reg=n_i,
                             elem_size=CH,
                             queue_num=(qc[0] % 4) if exp["queue"] == "rot" else 0,
                         )
                         qc[0] += 1
                     m = fin.tile([128, CH], F32, tag="m")
                     if wt == 0 or exp.get("reduce") == "off":
                         nc.gpsimd.memset(m[:], SENT_VAL)
                     else:
                         dkT = dk[:].rearrange("p (b c) -> p c b", c=CH)
                         nc.vector.tensor_reduce(
                             out=m[:], in_=dkT, axis=mybir.AxisListType.X,
                             op=mybir.AluOpType.min)
                     aggr = fin.tile([128, CH], F32, tag="aggr")
                     nc.vector.tensor_sub(
                         out=aggr[:], in0=y_own_sb[:, t * 128: (t + 1) * 128],
                         in1=m[:])
                     ptr = ps_c.tile([128, CH], F32, tag="ctr")
                     nc.tensor.transpose(out=ptr[:], in_=aggr[:], identity=ident[:])
                     aggrT = fin.tile([128, CH], F32, tag="aggrT")
                     nc.scalar.copy(out=aggrT[:], in_=ptr[:])
                     po = ps_c.tile([128, CH], F32, tag="cmm")
                     nc.tensor.matmul(out=po[:], lhsT=aggrT[:], rhs=wph_sb[:],
                                      start=True, stop=True)
                     osb = fin.tile([128, CH], F32, tag="osb")
                     nc.scalar.copy(out=osb[:], in_=po[:])
                     nc.sync.dma_start(out=out[t * 128: (t + 1) * 128, :],
                                       in_=osb[:])
                     continue
                 dks = {}
                 for k in range(NK):
                     btk = int(B[t, k])
                     if btk == 0:
                         continue
                     dks[k] = gdst.tile([128, btk * CH], BF16, tag=f"g{k}", name=f"dk{k}")
                 for (k, g0, nb, coff) in by_tile.get(t, []):
                     if exp["gather"] == "off":
                         break
                     dk3 = dks[k][:].rearrange("p (b c) -> p b c", c=CH)
                     n_i = nb * 128
                     ndup = 2 if exp["gather"] == "dup" else 1
                     for di in range(ndup):
                         if di == 0:
                             dst = dk3[:, g0: g0 + nb, :]
                         else:
                             ddup = gdst.tile(
                                 [128, (cfg.idx_per_inst // 128) * CH], BF16,
                                 tag="gdup", bufs=2)
                             dst = ddup[:].rearrange(
                                 "p (b c) -> p b c", c=CH)[:, :nb, :]
                         nc.gpsimd.dma_gather(
                             out_ap=dst,
                             in_ap=y_aug[k * CS: (k + 1) * CS, :],
                             idxs_ap=it[:, coff - c0: coff - c0 + n_i // 16],
                             num_idxs=n_i,
                             num_idxs_reg=n_i,
                             elem_size=CH,
                             queue_num=(qc[0] % 4) if exp["queue"] == "rot" else 0,
                         )
                         qc[0] += 1
                 # fold each chunk's rect down to one [128, CH] min
                 mks = []
                 if exp["folds"] == "off":
                     m = fin.tile([128, CH], F32, tag="m")
                     nc.gpsimd.memset(m[:], SENT_VAL)
                     aggr = fin.tile([128, CH], F32, tag="aggr")
                     nc.vector.tensor_sub(
                         out=aggr[:], in0=y_own_sb[:, t * 128: (t + 1) * 128],
                         in1=m[:])
                     ptr = ps_c.tile([128, CH], F32, tag="ctr")
                     nc.tensor.transpose(out=ptr[:], in_=aggr[:], identity=ident[:])
                     aggrT = fin.tile([128, CH], F32, tag="aggrT")
                     nc.vector.tensor_copy(out=aggrT[:], in_=ptr[:])
                     po = ps_c.tile([128, CH], F32, tag="cmm")
                     nc.tensor.matmul(out=po[:], lhsT=aggrT[:], rhs=wph_sb[:],
                                      start=True, stop=True)
                     osb = fin.tile([128, CH], F32, tag="osb")
                     nc.scalar.copy(out=osb[:], in_=po[:])
                     nc.sync.dma_start(out=out[t * 128: (t + 1) * 128, :], in_=osb[:])
                     continue
                 for k in range(NK):
                     if k not in dks:
                         continue
                     cur = dks[k]
                     nb = int(B[t, k])
                     while nb > 1:
                         half = (nb + 1) // 2
                         nxt = fold.tile([128, half * CH], BF16, tag=f"f{k}", bufs=3)
                         nc.vector.tensor_tensor(
                             out=nxt[:, : half * CH],
                             in0=cur[:, : half * CH],
                             in1=cur[:, (nb - half) * CH: nb * CH],
                             op=mybir.AluOpType.min)
                         cur, nb = nxt, half
                     mks.append(cur)
                 m = fin.tile([128, CH], F32, tag="m")
                 if len(mks) == 0:
                     nc.gpsimd.memset(m[:], SENT_VAL)
                 elif len(mks) == 1:
                     nc.vector.tensor_copy(out=m[:], in_=mks[0][:, :CH])
                 else:
                     # sequential accumulate with alternating tags (max 2 live)
                     acc = mks[0]
                     for i in range(1, len(mks) - 1):
                         mm = fold.tile([128, CH], BF16, tag=f"mrg{i % 2}")
                         nc.vector.tensor_tensor(
                             out=mm[:], in0=acc[:, :CH], in1=mks[i][:, :CH],
                             op=mybir.AluOpType.min)
                         acc = mm
                     nc.vector.tensor_tensor(
                         out=m[:], in0=acc[:, :CH], in1=mks[-1][:, :CH],
                         op=mybir.AluOpType.min)
                 # aggr = y_own - m ; out_tile = aggr @ W_phi.T
                 aggr = fin.tile([128, CH], F32, tag="aggr")
                 nc.vector.tensor_sub(
                     out=aggr[:], in0=y_own_sb[:, t * 128: (t + 1) * 128], in1=m[:])
                 ptr = ps_c.tile([128, CH], F32, tag="ctr")
                 nc.tensor.transpose(out=ptr[:], in_=aggr[:], identity=ident[:])
                 aggrT = fin.tile([128, CH], F32, tag="aggrT")
                 nc.vector.tensor_copy(out=aggrT[:], in_=ptr[:])
                 po = ps_c.tile([128, CH], F32, tag="cmm")
                 nc.tensor.matmul(out=po[:], lhsT=aggrT[:], rhs=wph_sb[:],
                                  start=True, stop=True)
                 osb = fin.tile([128, CH], F32, tag="osb")
                 nc.scalar.copy(out=osb[:], in_=po[:])
                 nc.sync.dma_start(out=out[t * 128: (t + 1) * 128, :], in_=osb[:])

            if phases != "abc":
                fillz = consts.tile([128, CH], F32)
                nc.vector.tensor_copy(out=fillz[:], in_=probe[:])
                for t in range(T):
                    nc.sync.dma_start(out=out[t * 128: (t + 1) * 128, :], in_=fillz[:])
    nc.compile()
    return nc


_CACHE = {}


def _get_runner_and_plan(x, edge_index, cfg: Cfg, reps=1, phases="abc", exp=None):
    prepped = prep(x, edge_index, cfg)
    plan = prepped[0]
    skey = (cfg, reps, phases, tuple(sorted((exp or {}).items())),
            tuple(plan["B"].reshape(-1).tolist()))
    if skey not in _CACHE:
        nc = build_program(plan, reps=reps, phases=phases, exp=exp)
        _CACHE[skey] = BassRunner(nc, cfg.n_cores)
    return (_CACHE[skey],) + prepped


def run_cfg(x, edge_index, W_theta, W_phi, cfg: Cfg, time_iters=0, reps=1, phases="abc", exp=None):
    (runner, plan, idxw, x_own, own_nodes, deg, x_perm, x_permT,
     x_ownT) = _get_runner_and_plan(
        x, edge_index, cfg, reps=reps, phases=phases, exp=exp)
    if exp and exp.get("sent_idx"):
        idxw = np.full_like(idxw, cfg.chunk_real)
    wtt = np.ascontiguousarray(np.asarray(W_theta, np.float32).T)
    wpt = np.ascontiguousarray(np.asarray(W_phi, np.float32).T)
    wtb = wtt.astype(ml_dtypes.bfloat16)
    in_maps = [
        dict(x_full=x_perm, x_own=x_own[c], w_theta_t=wtt, w_phi_t=wpt,
             x_fullT=x_permT, x_ownT=x_ownT[c], w_theta_b=wtb,
             idxw=np.ascontiguousarray(idxw[c]) if plan["W_total"] > 0
             else np.zeros((128, 16), np.int16))
        for c in range(cfg.n_cores)
    ]
    runner.prepare(in_maps)
    outs = runner.run()
    t_ns = runner.time_ns(iters=time_iters) if time_iters else None
    res = runner.results(outs)
    out_full = np.empty((cfg.N, CH), np.float32)
    for c in range(cfg.n_cores):
        out_full[own_nodes[c]] = res[c]["out"][: cfg.npc]
    out_full[deg == 0] = 0.0
    return out_full, t_ns


def kernel(x, edge_index, W_theta, W_phi):
    out, _ = run_cfg(x, edge_index, W_theta, W_phi, Cfg())
    return out



# revision 30
# speedup vs baseline: 4.3443x; 1.1604x over previous
"""Trainium2 Bass kernel for DevConv-style GNN message passing.

Reference computation:
    rel_t = (x[row] - x[col]) @ W_theta.T          # [E, 128]
    aggr  = segment_max(rel_t, row, N)             # [N, 128], empty -> 0
    out   = aggr @ W_phi.T                         # [N, 128]

Key reformulation: with y = x @ W_theta.T, within a segment (fixed row d)
    max_e (y[d] - y[col_e]) = y[d] - min_e y[col_e]     (per channel)
so the per-edge matmul disappears and only ONE gather per edge (y[col]) is
needed, followed by a segmented min.

The gather is bound by SWDGE descriptor generation on the Pool engine
(one descriptor pair per gathered row, generated by the Q7 core pair that
queue_num selects), so runtime ~ padded-slot count x per-idx gen cost.
What makes it fast (measured on HW, baseline 3.69 ms -> ~1.0-1.8 ms
depending on machine load):
  * queue rotation over all 4 SWDGE queues overlaps descriptor generation
    across the 4 Q7 core pairs (~3x aggregate gen rate);
  * 2048-idx instructions with single_packet=False halve the ~1 us/inst
    fixed cost (single_packet=True crashes the mesh at >=2048 idx);
  * pad slots point at 128 sentinel rows per chunk, not one — a single
    sentinel row serializes 27% of the drain on one HBM bank (+4.4 ms!);
  * balanced chunk coloring (int16 idx -> 4 chunks of 25088 rows) spreads
    each destination's edges ~deg/4 per chunk;
  * tiles sorted by max per-chunk count then count vector: padding
    inflation 1.95x -> 1.16x;
  * chunk-major windowed phase B (v3): per chunk, win_tiles=12 tiles'
    rects are one contiguous SBUF window, so gather instructions span
    tile boundaries at ~full 2048-idx occupancy, and phase A (chunks
    1..3) overlaps with phase-B gathers of earlier chunks.

Per core:
  Phase A: host supplies x^T (bf16, table order); y^T tiles come from
           direct PE matmuls (no transposes), are PE-transposed once and
           stored as bf16 rows into the chunked HBM table y_aug; stores
           issue on the Act HWDGE so phase-B idx loads don't queue behind
           them on SP. y_own is computed into SBUF the same way (no
           transpose needed).
  Phase B: per (chunk, window): dma_gathers fill the window rect; per
           tile a strided 1-input DVE tensor_reduce(min) gives the chunk-
           partial m, min-accumulated across chunks into ping-pong bf16
           accumulators (1-input reduce keeps the DVE out of 2-port perf
           mode, which would lock GpSimd off the shared SBUF port and
           stall SWDGE descriptor generation).
  Phase C: aggr = y_own - m; out_tile = aggr @ W_phi.T via PE transpose
           + matmul, emitted as soon as a tile's last chunk is folded.
Host un-permutes the concatenated core outputs and zeroes empty nodes.
"""
import sys
import os

sys.path.insert(0, "/opt/trn_rl_repo")

from contextlib import ExitStack
from dataclasses import dataclass

import numpy as np
import ml_dtypes

import concourse.bass as bass
import concourse.tile as tile
from concourse import bacc, mybir
from concourse.masks import make_identity

import time

import jax
from jax.sharding import Mesh, PartitionSpec
from jax.experimental.shard_map import shard_map

from concourse.bass2jax import (
    _bass_exec_p, install_neuronx_cc_hook, partition_id_tensor)


class BassRunner:
    """Keeps a jitted PJRT executable for a Bass program so it can be run
    repeatedly on device-resident inputs (for wall-clock timing)."""

    def __init__(self, nc, n_cores: int):
        install_neuronx_cc_hook()
        self.nc = nc
        self.n_cores = n_cores
        partition_name = nc.partition_id_tensor.name if nc.partition_id_tensor else None
        in_names, out_names, out_avals = [], [], []
        for alloc in nc.m.functions[0].allocations:
            if not isinstance(alloc, mybir.MemoryLocationSet):
                continue
            name = alloc.memorylocations[0].name
            if alloc.kind == "ExternalInput":
                if name != partition_name:
                    in_names.append(name)
            elif alloc.kind == "ExternalOutput":
                out_names.append(name)
                out_avals.append(jax.core.ShapedArray(
                    tuple(alloc.tensor_shape), mybir.dt.np(alloc.dtype)))
        self.in_names, self.out_names, self.out_avals = in_names, out_names, out_avals
        self.n_params = len(in_names)
        all_in_names = list(in_names) + list(out_names)
        if partition_name is not None:
            all_in_names.append(partition_name)

        def _body(*args):
            operands = list(args)
            if partition_name is not None:
                operands.append(partition_id_tensor())
            outs = _bass_exec_p.bind(
                *operands,
                out_avals=tuple(out_avals),
                in_names=tuple(all_in_names),
                out_names=tuple(out_names),
                lowering_input_output_aliases=(),
                sim_require_finite=True,
                sim_require_nnan=True,
                nc=nc,
            )
            return tuple(outs)

        devices = jax.devices()[:n_cores]
        self.mesh = Mesh(np.asarray(devices), ("core",))
        n_outs = len(out_names)
        in_specs = (PartitionSpec("core"),) * (self.n_params + n_outs)
        out_specs = (PartitionSpec("core"),) * n_outs
        self.fn = jax.jit(
            shard_map(_body, mesh=self.mesh, in_specs=in_specs,
                      out_specs=out_specs, check_rep=False),
            keep_unused=True,
        )
        self._dev_args = None

    def prepare(self, in_maps):
        assert len(in_maps) == self.n_cores
        concat_in = [
            np.concatenate([np.asarray(in_maps[c][n]) for c in range(self.n_cores)],
                           axis=0)
            for n in self.in_names
        ]
        concat_zeros = [
            np.zeros((self.n_cores * a.shape[0], *a.shape[1:]), a.dtype)
            for a in self.out_avals
        ]
        sharding = jax.sharding.NamedSharding(self.mesh, PartitionSpec("core"))
        self._dev_args = [jax.device_put(v, sharding) for v in concat_in + concat_zeros]
        return self

    def run(self):
        outs = self.fn(*self._dev_args)
        jax.block_until_ready(outs)
        return outs

    def results(self, outs):
        return [
            {n: np.asarray(outs[i]).reshape(self.n_cores, *self.out_avals[i].shape)[c]
             for i, n in enumerate(self.out_names)}
            for c in range(self.n_cores)
        ]

    def time_ns(self, iters=5, warmup=2):
        for _ in range(warmup):
            self.run()
        ts = []
        for _ in range(iters):
            t0 = time.perf_counter()
            self.run()
            ts.append((time.perf_counter() - t0) * 1e9)
        return min(ts)


CH = 128
F32 = mybir.dt.float32
BF16 = mybir.dt.bfloat16
I16 = mybir.dt.int16
SENT_VAL = 3.0e38
IDX_PER_INST = 2048          # >=2048 requires single_packet=False
BLK_PER_INST = IDX_PER_INST // 128


@dataclass(frozen=True)
class Cfg:
    N: int = 100_000
    E: int = 3_200_000
    n_cores: int = 8
    chunk_real: int = 25_088     # table rows per chunk (512-aligned, < int16 max)
    idx_per_inst: int = 2048     # needs single_packet=False (spf) to not crash
    balanced: bool = True        # balanced chunk coloring + count-vector tiling
    n_sent: int = 128            # sentinel rows per chunk (pad reads spread
                                 # over HBM banks instead of hammering one row)
    layout: str = "v3"           # "v3": chunk-major windowed phase B
                                 # "tile": original per-tile rects
    win_tiles: int = 12          # tiles per gather window (v3)

    @property
    def n_chunks(self):
        return (self.N + self.chunk_real - 1) // self.chunk_real

    @property
    def chunk_stride(self):
        return self.chunk_real + self.n_sent

    @property
    def npc(self):
        assert self.N % self.n_cores == 0
        return self.N // self.n_cores

    @property
    def tiles(self):
        return (self.npc + 127) // 128


def _wrap16(seg: np.ndarray) -> np.ndarray:
    """Per-instruction idx wrap: flat [n] -> [128, n//16]; idx i at
    (partition i%16, col i//16), replicated across the 8 gpsimd groups."""
    n = seg.shape[-1]
    w = seg.reshape(*seg.shape[:-1], n // 16, 16)
    w = np.swapaxes(w, -1, -2)                       # [..., 16, n//16]
    return np.tile(w, (1,) * (seg.ndim - 1) + (8, 1))


def _color_chunks(row, col, deg, N, NK, cap):
    """Greedy quota-balanced assignment of cols to NK chunks; returns
    (chunk_of, cnt_dk) where cnt_dk[d,k] = per-dest per-chunk edge count."""
    o = np.argsort(col, kind="stable")
    dest_s = row[o]
    starts = np.searchsorted(col[o], np.arange(N + 1))
    quota = -(-deg // NK)
    cnt_dk = np.zeros((N, NK), np.int32)
    chunk_of = np.full(N, -1, np.int64)
    chunk_sz = np.zeros(NK, np.int64)
    col_order = np.argsort(-(starts[1:] - starts[:-1]), kind="stable")
    for _ in range(2):
        for c in col_order:
            ds = dest_s[starts[c]: starts[c + 1]]
            kprev = chunk_of[c]
            if kprev >= 0:
                if len(ds):
                    cnt_dk[ds, kprev] -= 1
                chunk_sz[kprev] -= 1
            if len(ds) == 0:
                k = int(np.argmin(chunk_sz))
            else:
                cc = cnt_dk[ds]
                over = np.maximum(0, cc + 1 - quota[ds][:, None])
                sc = (over * 1000.0 + cc).sum(axis=0).astype(np.float64)
                sc += chunk_sz * 1e-4
                sc[chunk_sz >= cap] = 1e18
                k = int(np.argmin(sc))
                cnt_dk[ds, k] += 1
            chunk_of[c] = k
            chunk_sz[k] += 1
    return chunk_of, cnt_dk


def prep(x, edge_index, cfg: Cfg):
    """Host-side data prep. Returns (plan, per-core inputs, unpermute info)."""
    N, E, NC = cfg.N, cfg.E, cfg.n_cores
    CR, NK, T = cfg.chunk_real, cfg.n_chunks, cfg.tiles
    row = np.asarray(edge_index[0], dtype=np.int64)
    col = np.asarray(edge_index[1], dtype=np.int64)

    deg = np.bincount(row, minlength=N)
    x_np0 = np.asarray(x, dtype=np.float32)
    if cfg.balanced:
        chunk_of, cnt_dk = _color_chunks(row, col, deg, N, NK, CR)
        # rank within chunk
        oc = np.argsort(chunk_of, kind="stable")
        rank_of = np.empty(N, np.int64)
        csz = np.bincount(chunk_of, minlength=NK)
        cstart = np.concatenate([[0], np.cumsum(csz)])
        rank_of[oc] = np.arange(N) - cstart[chunk_of[oc]]
        # dest order: group by max per-chunk count (the rect width driver),
        # then by the full count vector — tiles get near-identical vectors
        order = np.lexsort(tuple(cnt_dk[:, k] for k in range(NK - 1, -1, -1))
                           + (-cnt_dk.max(axis=1),))
        # x permuted into table layout [NK*CR, CH]
        x_perm = np.zeros((NK * CR, x_np0.shape[1]), np.float32)
        x_perm[chunk_of * CR + rank_of] = x_np0
    else:
        order = np.argsort(-deg, kind="stable")      # node ids by desc degree
        x_perm = x_np0
    core_of = np.empty(N, np.int64)
    pos_of = np.empty(N, np.int64)
    r = np.arange(N)
    core_of[order] = r % NC
    pos_of[order] = r // NC

    ec = core_of[row]
    ep = pos_of[row]
    if cfg.balanced:
        ek = chunk_of[col]
        elocal = rank_of[col].astype(np.int16)
    else:
        ek = col // CR
        elocal = (col - ek * CR).astype(np.int16)
    et = ep // 128
    ed = ep % 128

    # per-(core,tile,chunk,node) counts and within-group slot index j
    key = ((ec * T + et) * NK + ek) * 128 + ed
    o = np.argsort(key, kind="stable")
    ks = key[o]
    first = np.r_[True, ks[1:] != ks[:-1]]
    run_id = np.cumsum(first) - 1
    run_start = np.flatnonzero(first)
    j = np.arange(E) - run_start[run_id]

    cnt = np.bincount(key, minlength=NC * T * NK * 128).reshape(NC, T, NK, 128)
    B = cnt.max(axis=(0, 3)).astype(np.int64)        # [T, NK] shared structure
    B = np.maximum(B, 1)         # every (t,k) rect non-empty (simplifies accum)

    Bf = B.reshape(-1)
    off = np.concatenate([[0], np.cumsum(Bf * 128)])  # slot offset per (t,k)
    total_slots = int(off[-1])

    # pad slots point at one of n_sent sentinel rows (spread over HBM banks)
    pad_vals = (CR + (np.arange(total_slots) % cfg.n_sent)).astype(np.int16)
    idx_all = np.broadcast_to(pad_vals, (NC, total_slots)).copy()
    tk = et[o] * NK + ek[o]
    pos_in = off[tk] + j * 128 + ed[o]
    idx_all[ec[o], pos_in] = elocal[o]

    # split into gather instructions and build wrapped idx input
    blk_per_inst = cfg.idx_per_inst // 128
    insts = []           # (t, k, g0blk, nblk, col_off)
    tile_cols = []       # per tile: (col_start, col_end)
    windows = None
    wsegs = []
    col_off = 0
    if cfg.layout == "v3":
        # chunk-major layout: per chunk, all tiles' rects back-to-back;
        # instructions span tile boundaries within a window of win_tiles.
        Bc = np.ascontiguousarray(B.T)               # [NK, T]
        cb = np.concatenate([[0], np.cumsum(Bc.sum(axis=1) * 128)])
        off3 = np.zeros((NK, T), np.int64)
        for k in range(NK):
            off3[k] = cb[k] + np.concatenate(
                [[0], np.cumsum(Bc[k][:-1] * 128)])
        total3 = int(cb[-1])
        pad_vals = (CR + (np.arange(total3) % cfg.n_sent)).astype(np.int16)
        idx_all = np.broadcast_to(pad_vals, (NC, total3)).copy()
        pos_in = off3[ek[o], et[o]] + j * 128 + ed[o]
        idx_all[ec[o], pos_in] = elocal[o]
        windows = []
        for k in range(NK):
            for t0 in range(0, T, cfg.win_tiles):
                t1 = min(t0 + cfg.win_tiles, T)
                WB = int(Bc[k, t0:t1].sum())
                base_slot = int(off3[k, t0])
                inst_list = []
                for g0 in range(0, WB, blk_per_inst):
                    nb = min(blk_per_inst, WB - g0)
                    seg = idx_all[:, base_slot + g0 * 128:
                                  base_slot + (g0 + nb) * 128]
                    wsegs.append(_wrap16(seg))
                    inst_list.append((g0, nb, col_off))
                    col_off += nb * 128 // 16
                reduces = []
                acc = 0
                for t in range(t0, t1):
                    reduces.append((t, acc, int(Bc[k, t])))
                    acc += int(Bc[k, t])
                windows.append(dict(k=k, WB=WB, insts=inst_list,
                                    reduces=reduces))
    else:
        for t in range(T):
            t_start = col_off
            for k in range(NK):
                btk = int(B[t, k])
                base = int(off[t * NK + k])
                for g0 in range(0, btk, blk_per_inst):
                    nb = min(blk_per_inst, btk - g0)
                    n_i = nb * 128
                    seg = idx_all[:, base + g0 * 128: base + g0 * 128 + n_i]
                    wsegs.append(_wrap16(seg))
                    insts.append((t, k, g0, nb, col_off))
                    col_off += n_i // 16
            tile_cols.append((t_start, col_off))
    idxw = np.concatenate(wsegs, axis=2) if wsegs else np.zeros((NC, 128, 0), np.int16)
    W_total = idxw.shape[2]

    # per-core x_own in pos order, padded to T*128 rows
    own_nodes = np.empty((NC, cfg.npc), np.int64)
    own_nodes[core_of[order], pos_of[order]] = order  # own_nodes[c, p] = node id
    x_np = np.asarray(x, dtype=np.float32)
    x_own = np.zeros((NC, T * 128, CH), np.float32)
    x_own[:, : cfg.npc] = x_np[own_nodes]

    x_permT = np.ascontiguousarray(x_perm.T).astype(ml_dtypes.bfloat16)
    x_ownT = np.ascontiguousarray(np.transpose(x_own, (0, 2, 1))).astype(
        ml_dtypes.bfloat16)
    plan = dict(cfg=cfg, B=B, insts=insts, tile_cols=tile_cols, W_total=W_total,
                windows=windows)
    return plan, idxw, x_own, own_nodes, deg, x_perm, x_permT, x_ownT


def build_program(plan, reps=1, phases="abc", exp=None):
    exp = {**dict(gather="on", folds="on", queue="rot", fold_mode="reduce",
                  a_store="act", spf=1),
           **(exp or {})}
    cfg: Cfg = plan["cfg"]
    N, NK, CR, T = cfg.N, cfg.n_chunks, cfg.chunk_real, cfg.tiles
    CS = cfg.chunk_stride
    B, insts, tile_cols, W_total = (
        plan["B"], plan["insts"], plan["tile_cols"], plan["W_total"])

    windows = plan.get("windows")
    nc = bacc.Bacc(None, target_bir_lowering=False, num_swdge_queues=4,
                   dynamic_dma_scratch_size=int(exp.get("scratch", 16384)))
    NA = NK * CR if cfg.balanced else N      # phase-A row count (table layout)
    axt = windows is not None and exp.get("axt", 1)
    if axt:
        x_fullT = nc.declare_dram_parameter("x_fullT", [CH, NA], BF16,
                                            isOutput=False)
        x_ownT = nc.declare_dram_parameter("x_ownT", [CH, T * 128], BF16,
                                           isOutput=False)
        wthb = nc.declare_dram_parameter("w_theta_b", [CH, CH], BF16,
                                         isOutput=False)
    else:
        x_full = nc.declare_dram_parameter("x_full", [NA, CH], F32, isOutput=False)
        x_own = nc.declare_dram_parameter("x_own", [T * 128, CH], F32, isOutput=False)
        wth = nc.declare_dram_parameter("w_theta_t", [CH, CH], F32, isOutput=False)
    wph = nc.declare_dram_parameter("w_phi_t", [CH, CH], F32, isOutput=False)
    idxw = nc.declare_dram_parameter("idxw", [128, max(W_total, 16)], I16, isOutput=False)
    out = nc.declare_dram_parameter("out", [T * 128, CH], F32, isOutput=True)

    qc = [0]  # gather queue rotation

    with tile.TileContext(nc) as tc:
        with ExitStack() as ctx:
            consts = ctx.enter_context(tc.tile_pool(name="consts", bufs=1))
            dram = ctx.enter_context(tc.tile_pool(name="dram", bufs=1, space="DRAM"))
            ax = ctx.enter_context(tc.tile_pool(name="ax", bufs=2))
            axT = ctx.enter_context(tc.tile_pool(name="axT", bufs=2))
            ay = ctx.enter_context(tc.tile_pool(name="ay", bufs=2))
            ps_t = ctx.enter_context(tc.tile_pool(name="ps_t", bufs=2, space="PSUM"))
            ps_y = ctx.enter_context(tc.tile_pool(name="ps_y", bufs=2, space="PSUM"))
            ps_c = ctx.enter_context(tc.tile_pool(name="ps_c", bufs=2, space="PSUM"))
            gidx = ctx.enter_context(tc.tile_pool(name="gidx", bufs=4))
            gdst = ctx.enter_context(tc.tile_pool(name="gdst", bufs=2))
            fold = ctx.enter_context(tc.tile_pool(name="fold", bufs=2))
            fin = ctx.enter_context(tc.tile_pool(name="fin", bufs=2))

            y_aug = dram.tile([NK * CS, CH], BF16)

            ident = consts.tile([128, 128], F32)
            make_identity(nc, ident[:])
            if axt:
                wth_sb = consts.tile([CH, CH], BF16)
                nc.sync.dma_start(out=wth_sb[:], in_=wthb[:])
            else:
                wth_sb = consts.tile([CH, CH], F32)
                nc.sync.dma_start(out=wth_sb[:], in_=wth[:])
            wph_sb = consts.tile([CH, CH], F32)
            nc.sync.dma_start(out=wph_sb[:], in_=wph[:])
            y_own_sb = consts.tile([128, T * 128],
                                   BF16 if windows is not None else F32)
            probe = consts.tile([128, CH], BF16)
            nc.gpsimd.memset(probe[:], 0.0)
            cst128 = consts.tile([128, CH], BF16)
            nc.gpsimd.memset(cst128[:], 1.0)
            sent = consts.tile([cfg.n_sent, CH], BF16)
            nc.gpsimd.memset(sent[:], SENT_VAL)
            for k in range(NK):
                nc.sync.dma_start(
                    out=y_aug[k * CS + CR: k * CS + CR + cfg.n_sent, :],
                    in_=sent[:])

            A_MODE = os.environ.get("A_MODE", "full")
            PS_BUFS = int(os.environ.get("PS_BUFS", "2"))
            # ---------------- Phase A: y_aug = (x @ W_theta.T).bf16 ----------
            def emit_group(src, n0, gn, dst):
                """Process rows [n0, n0+gn) of src -> y into dst.
                dst = ("aug",) writes y_aug rows (with chunk-boundary split),
                dst = ("own",) writes y_own_sb cols."""
                nt = (gn + 127) // 128
                xg = ax.tile([128, nt * 128], F32, tag="xg",
                             bufs=int(os.environ.get("XG_BUFS", "2")))
                xg3 = xg[:].rearrange("p (i c) -> p i c", c=CH)
                load_eng = nc.gpsimd if A_MODE == "dma3" else nc.sync
                store_eng = nc.scalar if exp.get("a_store") == "act" else nc.sync
                if gn % 128 == 0:
                    load_eng.dma_start(
                        out=xg3[:, :nt, :],
                        in_=src[n0: n0 + gn, :].rearrange("(i p) c -> p i c", p=128))
                else:
                    for i in range(nt):
                        rn = min(128, gn - i * 128)
                        nc.sync.dma_start(
                            out=xg3[:rn, i, :],
                            in_=src[n0 + i * 128: n0 + i * 128 + rn, :])
                if A_MODE.startswith("dma") and dst == "aug":
                    # dma : load->store dep, both on sync
                    # dma2: stores only dep-free (loads still emitted)
                    # dma3: load on gpsimd, stores dep on load, on sync
                    # dma5: loads only (no stores)
                    if gn % 128 == 0:
                        for i in range(nt):
                            r0 = n0 + i * 128
                            kb = (r0 // CR) * cfg.n_sent
                            if A_MODE == "dma5":
                                continue
                            src_ap = (cst128[:] if A_MODE == "dma2"
                                      else xg3[:, i, :CH // 2].bitcast(BF16))
                            nc.sync.dma_start(
                                out=y_aug[r0 + kb: r0 + kb + 128, :], in_=src_ap)
                    return
                pt = ps_t.tile([128, nt * 128], F32, tag="pt", bufs=PS_BUFS)
                for i in range(nt):
                    rn = min(128, gn - i * 128)
                    nc.tensor.transpose(
                        out=pt[:, i * 128: i * 128 + rn],
                        in_=xg3[:rn, i, :],
                        identity=ident[:rn, :rn])
                xT = axT.tile([128, nt * 128], F32, tag="xT")
                if exp.get("fold_mode") == "reduce":
                    nc.scalar.copy(out=xT[:, : nt * 128], in_=pt[:, : nt * 128])
                else:
                    nc.vector.tensor_copy(out=xT[:, : nt * 128], in_=pt[:, : nt * 128])
                if A_MODE == "nomm" and dst == "aug":
                    for i in range(nt):
                        r0 = n0 + i * 128
                        kb = (r0 // CR) * cfg.n_sent
                        nc.sync.dma_start(
                            out=y_aug[r0 + kb: r0 + kb + 128, :],
                            in_=xT[:, i * 128: i * 128 + 128][:, :CH // 2].bitcast(BF16))
                    return
                py = ps_y.tile([128, nt * 128], F32, tag="py", bufs=PS_BUFS)
                for i in range(nt):
                    rn = min(128, gn - i * 128)
                    nc.tensor.matmul(
                        out=py[:rn, i * 128: (i + 1) * 128],
                        lhsT=xT[:, i * 128: i * 128 + rn],
                        rhs=wth_sb[:],
                        start=True, stop=True)
                if dst == "own":
                    nc.scalar.copy(
                        out=y_own_sb[:, n0: n0 + nt * 128], in_=py[:, : nt * 128])
                    return
                yg = ay.tile([128, nt * 128], BF16, tag="yg")
                copy2 = nc.vector.tensor_copy if A_MODE == "dvecopy" else nc.scalar.copy
                if gn % 128 == 0:
                    copy2(out=yg[:, : gn], in_=py[:, : gn])
                else:
                    for i in range(nt):
                        rn = min(128, gn - i * 128)
                        copy2(
                            out=yg[:rn, i * 128: (i + 1) * 128],
                            in_=py[:rn, i * 128: (i + 1) * 128])
                yg3 = yg[:].rearrange("p (i c) -> p i c", c=CH)
                # write y rows n -> aug rows n + n // CR, splitting at tile level
                for i in range(nt):
                    r0 = n0 + i * 128
                    rn = min(128, gn - i * 128)
                    kc = r0 // CR
                    ec = (r0 + rn - 1) // CR
                    kb = kc * cfg.n_sent
                    ke = ec * cfg.n_sent
                    if kc == ec:
                        store_eng.dma_start(
                            out=y_aug[r0 + kb: r0 + kb + rn, :], in_=yg3[:rn, i, :])
                    else:
                        split = (kc + 1) * CR - r0       # rows before boundary
                        store_eng.dma_start(
                            out=y_aug[r0 + kb: r0 + kb + split, :],
                            in_=yg3[:split, i, :])
                        store_eng.dma_start(
                            out=y_aug[r0 + split + ke: r0 + ke + rn, :],
                            in_=yg3[split:rn, i, :])

            by_tile = {}
            for (t, k, g0, nb, coff) in insts:
                by_tile.setdefault(t, []).append((k, g0, nb, coff))

            def emit_group_xt(srcT, n0, gn, dst):
                """Transpose-free phase A: srcT is [128 in_ch, nodes] bf16;
                y tile = (xT_tile)^T @ W_theta^T via direct PE matmuls."""
                nt = (gn + 127) // 128
                xg = ax.tile([128, nt * 128], BF16, tag="xgt")
                nc.sync.dma_start(out=xg[:, :gn], in_=srcT[:, n0: n0 + gn])
                py = ps_y.tile([128, nt * 128], F32, tag="pyt", bufs=PS_BUFS)
                for i in range(nt):
                    rn = min(128, gn - i * 128)
                    nc.tensor.matmul(
                        out=py[:rn, i * 128: (i + 1) * 128],
                        lhsT=xg[:, i * 128: i * 128 + rn],
                        rhs=wth_sb[:], start=True, stop=True)
                if dst == "own":
                    nc.scalar.copy(out=y_own_sb[:, n0: n0 + nt * 128],
                                   in_=py[:, : nt * 128])
                    return
                yg = ay.tile([128, nt * 128], BF16, tag="ygt")
                nc.scalar.copy(out=yg[:, : nt * 128], in_=py[:, : nt * 128])
                yg3 = yg[:].rearrange("p (i c) -> p i c", c=CH)
                store_eng = nc.scalar if exp.get("a_store") == "act" else nc.sync
                for i in range(nt):
                    r0 = n0 + i * 128
                    rn = min(128, gn - i * 128)
                    kc = r0 // CR
                    ec = (r0 + rn - 1) // CR
                    kb = kc * cfg.n_sent
                    ke = ec * cfg.n_sent
                    if kc == ec:
                        store_eng.dma_start(
                            out=y_aug[r0 + kb: r0 + kb + rn, :],
                            in_=yg3[:rn, i, :])
                    else:
                        split = (kc + 1) * CR - r0
                        store_eng.dma_start(
                            out=y_aug[r0 + kb: r0 + kb + split, :],
                            in_=yg3[:split, i, :])
                        store_eng.dma_start(
                            out=y_aug[r0 + split + ke: r0 + ke + rn, :],
                            in_=yg3[split:rn, i, :])

            for _rep in range(reps):
              if axt:
                for n0 in range(0, NA, 512):
                    emit_group_xt(x_fullT, n0, min(512, NA - n0), "aug")
                for n0 in range(0, T * 128, 512):
                    emit_group_xt(x_ownT, n0, min(512, T * 128 - n0), "own")
              else:
                for n0 in range(0, NA, 512):
                    emit_group(x_full, n0, min(512, NA - n0), "aug")
                for n0 in range(0, T * 128, 512):
                    emit_group(x_own, n0, min(512, T * 128 - n0), "own")

              if windows is not None:
                 # ---------- Phase B + C, chunk-major windows (v3) ----------
                 maccA = consts.tile([128, T * 128], BF16)
                 maccB = consts.tile([128, T * 128], BF16)

                 def emit_phase_c(t, m_ap):
                     aggr = fin.tile([128, CH], F32, tag="aggr")
                     nc.vector.tensor_sub(
                         out=aggr[:],
                         in0=y_own_sb[:, t * 128: (t + 1) * 128], in1=m_ap)
                     ptr = ps_c.tile([128, CH], F32, tag="ctr")
                     nc.tensor.transpose(out=ptr[:], in_=aggr[:],
                                         identity=ident[:])
                     aggrT = fin.tile([128, CH], F32, tag="aggrT")
                     nc.scalar.copy(out=aggrT[:], in_=ptr[:])
                     po = ps_c.tile([128, CH], F32, tag="cmm")
                     nc.tensor.matmul(out=po[:], lhsT=aggrT[:], rhs=wph_sb[:],
                                      start=True, stop=True)
                     osb = fin.tile([128, CH], F32, tag="osb")
                     nc.scalar.copy(out=osb[:], in_=po[:])
                     nc.sync.dma_start(out=out[t * 128: (t + 1) * 128, :],
                                       in_=osb[:])

                 for w in windows:
                     k, WB = w["k"], w["WB"]
                     wc0 = w["insts"][0][2]
                     wc1 = w["insts"][-1][2] + w["insts"][-1][1] * 128 // 16
                     it = gidx.tile([128, max(wc1 - wc0, 16)], I16, tag="it")
                     nc.sync.dma_start(out=it[:, : wc1 - wc0],
                                       in_=idxw[:, wc0:wc1])
                     win = gdst.tile([128, WB * CH], BF16, tag="win",
                                     bufs=int(exp.get("gbufs", 2)))
                     win3 = win[:].rearrange("p (b c) -> p b c", c=CH)
                     for (g0, nb, coff) in w["insts"]:
                         if exp["gather"] == "seq":
                             r0 = min(k * CS + g0 * 128, NK * CS - nb * 128)
                             nc.sync.dma_start(
                                 out=win3[:, g0: g0 + nb, :],
                                 in_=y_aug[r0: r0 + nb * 128, :].rearrange(
                                     "(i p) c -> p i c", p=128))
                             continue
                         nc.gpsimd.dma_gather(
                             out_ap=win3[:, g0: g0 + nb, :],
                             in_ap=y_aug[k * CS: (k + 1) * CS, :],
                             idxs_ap=it[:, coff - wc0:
                                        coff - wc0 + nb * 128 // 16],
                             num_idxs=nb * 128,
                             num_idxs_reg=nb * 128,
                             elem_size=CH,
                             single_packet=not bool(exp.get("spf")),
                             queue_num=(qc[0] % 4) if exp["queue"] == "rot"
                             else 0,
                         )
                         qc[0] += 1
                     winT = win[:].rearrange("p (b c) -> p c b", c=CH)
                     for (t, boff, btk) in w["reduces"]:
                         if exp.get("reduce") == "off":
                             sl = winT[:, :, 0:1]
                         else:
                             sl = winT[:, :, boff: boff + btk]
                         if k == 0:
                             nc.vector.tensor_reduce(
                                 out=maccA[:, t * 128: (t + 1) * 128],
                                 in_=sl, axis=mybir.AxisListType.X,
                                 op=mybir.AluOpType.min)
                         else:
                             mk = fin.tile([128, CH], BF16, tag="mk", bufs=2)
                             nc.vector.tensor_reduce(
                                 out=mk[:], in_=sl, axis=mybir.AxisListType.X,
                                 op=mybir.AluOpType.min)
                             src = maccA if k % 2 == 1 else maccB
                             dst = maccB if k % 2 == 1 else maccA
                             nc.vector.tensor_tensor(
                                 out=dst[:, t * 128: (t + 1) * 128],
                                 in0=src[:, t * 128: (t + 1) * 128],
                                 in1=mk[:], op=mybir.AluOpType.min)
                             if k == NK - 1:
                                 emit_phase_c(
                                     t, dst[:, t * 128: (t + 1) * 128])
                 continue

              # ---------------- Phase B + C per tile ---------------------------
              for t in range(T):
                 c0, c1 = tile_cols[t]
                 it = gidx.tile([128, max(c1 - c0, 16)], I16, tag="it")
                 if c1 > c0:
                     nc.sync.dma_start(out=it[:, : c1 - c0], in_=idxw[:, c0:c1])
                 if exp.get("fold_mode") == "reduce":
                     kws = [k for k in range(NK) if int(B[t, k]) > 0]
                     koff = {}
                     wt = 0
                     for k in kws:
                         koff[k] = wt
                         wt += int(B[t, k])
                     if wt > 0:
                         dk = gdst.tile([128, wt * CH], BF16, tag="gr",
                                        bufs=int(exp.get("gbufs", 3)))
                         dk3 = dk[:].rearrange("p (b c) -> p b c", c=CH)
                     for (k, g0, nb, coff) in by_tile.get(t, []):
                         n_i = nb * 128
                         col0 = koff[k] + g0
                         if exp["gather"] == "seq":
                             r0 = min(k * CS + g0 * 128, NK * CS - n_i)
                             nc.sync.dma_start(
                                 out=dk3[:, col0: col0 + nb, :],
                                 in_=y_aug[r0: r0 + n_i, :].rearrange(
                                     "(i p) c -> p i c", p=128))
                             continue
                         nc.gpsimd.dma_gather(
                             out_ap=dk3[:, col0: col0 + nb, :],
                             in_ap=y_aug[k * CS: (k + 1) * CS, :],
                             idxs_ap=it[:, coff - c0: coff - c0 + n_i // 16],
                             num_idxs=n_i,
                             num_idxs_reg=n_i,
                             elem_size=CH,
                             queue_num=(qc[0] % 4) if exp["queue"] == "rot" else 0,
                         )
                         qc[0] += 1
                     m = fin.tile([128, CH], F32, tag="m")
                     if wt == 0 or exp.get("reduce") == "off":
                         nc.gpsimd.memset(m[:], SENT_VAL)
                     else:
                         dkT = dk[:].rearrange("p (b c) -> p c b", c=CH)
                         nc.vector.tensor_reduce(
                             out=m[:], in_=dkT, axis=mybir.AxisListType.X,
                             op=mybir.AluOpType.min)
                     aggr = fin.tile([128, CH], F32, tag="aggr")
                     nc.vector.tensor_sub(
                         out=aggr[:], in0=y_own_sb[:, t * 128: (t + 1) * 128],
                         in1=m[:])
                     ptr = ps_c.tile([128, CH], F32, tag="ctr")
                     nc.tensor.transpose(out=ptr[:], in_=aggr[:], identity=ident[:])
                     aggrT = fin.tile([128, CH], F32, tag="aggrT")
                     nc.scalar.copy(out=aggrT[:], in_=ptr[:])
                     po = ps_c.tile([128, CH], F32, tag="cmm")
                     nc.tensor.matmul(out=po[:], lhsT=aggrT[:], rhs=wph_sb[:],
                                      start=True, stop=True)
                     osb = fin.tile([128, CH], F32, tag="osb")
                     nc.scalar.copy(out=osb[:], in_=po[:])
                     nc.sync.dma_start(out=out[t * 128: (t + 1) * 128, :],
                                       in_=osb[:])
                     continue
                 dks = {}
                 for k in range(NK):
                     btk = int(B[t, k])
                     if btk == 0:
                         continue
                     dks[k] = gdst.tile([128, btk * CH], BF16, tag=f"g{k}", name=f"dk{k}")
                 for (k, g0, nb, coff) in by_tile.get(t, []):
                     if exp["gather"] == "off":
                         break
                     dk3 = dks[k][:].rearrange("p (b c) -> p b c", c=CH)
                     n_i = nb * 128
                     ndup = 2 if exp["gather"] == "dup" else 1
                     for di in range(ndup):
                         if di == 0:
                             dst = dk3[:, g0: g0 + nb, :]
                         else:
                             ddup = gdst.tile(
                                 [128, (cfg.idx_per_inst // 128) * CH], BF16,
                                 tag="gdup", bufs=2)
                             dst = ddup[:].rearrange(
                                 "p (b c) -> p b c", c=CH)[:, :nb, :]
                         nc.gpsimd.dma_gather(
                             out_ap=dst,
                             in_ap=y_aug[k * CS: (k + 1) * CS, :],
                             idxs_ap=it[:, coff - c0: coff - c0 + n_i // 16],
                             num_idxs=n_i,
                             num_idxs_reg=n_i,
                             elem_size=CH,
                             queue_num=(qc[0] % 4) if exp["queue"] == "rot" else 0,
                         )
                         qc[0] += 1
                 # fold each chunk's rect down to one [128, CH] min
                 mks = []
                 if exp["folds"] == "off":
                     m = fin.tile([128, CH], F32, tag="m")
                     nc.gpsimd.memset(m[:], SENT_VAL)
                     aggr = fin.tile([128, CH], F32, tag="aggr")
                     nc.vector.tensor_sub(
                         out=aggr[:], in0=y_own_sb[:, t * 128: (t + 1) * 128],
                         in1=m[:])
                     ptr = ps_c.tile([128, CH], F32, tag="ctr")
                     nc.tensor.transpose(out=ptr[:], in_=aggr[:], identity=ident[:])
                     aggrT = fin.tile([128, CH], F32, tag="aggrT")
                     nc.vector.tensor_copy(out=aggrT[:], in_=ptr[:])
                     po = ps_c.tile([128, CH], F32, tag="cmm")
                     nc.tensor.matmul(out=po[:], lhsT=aggrT[:], rhs=wph_sb[:],
                                      start=True, stop=True)
                     osb = fin.tile([128, CH], F32, tag="osb")
                     nc.scalar.copy(out=osb[:], in_=po[:])
                     nc.sync.dma_start(out=out[t * 128: (t + 1) * 128, :], in_=osb[:])
                     continue
                 for k in range(NK):
                     if k not in dks:
                         continue
                     cur = dks[k]
                     nb = int(B[t, k])
                     while nb > 1:
                         half = (nb + 1) // 2
                         nxt = fold.tile([128, half * CH], BF16, tag=f"f{k}", bufs=3)
                         nc.vector.tensor_tensor(
                             out=nxt[:, : half * CH],
                             in0=cur[:, : half * CH],
                             in1=cur[:, (nb - half) * CH: nb * CH],
                             op=mybir.AluOpType.min)
                         cur, nb = nxt, half
                     mks.append(cur)
                 m = fin.tile([128, CH], F32, tag="m")
                 if len(mks) == 0:
                     nc.gpsimd.memset(m[:], SENT_VAL)
                 elif len(mks) == 1:
                     nc.vector.tensor_copy(out=m[:], in_=mks[0][:, :CH])
                 else:
                     # sequential accumulate with alternating tags (max 2 live)
                     acc = mks[0]
                     for i in range(1, len(mks) - 1):
                         mm = fold.tile([128, CH], BF16, tag=f"mrg{i % 2}")
                         nc.vector.tensor_tensor(
                             out=mm[:], in0=acc[:, :CH], in1=mks[i][:, :CH],
                             op=mybir.AluOpType.min)
                         acc = mm
                     nc.vector.tensor_tensor(
                         out=m[:], in0=acc[:, :CH], in1=mks[-1][:, :CH],
                         op=mybir.AluOpType.min)
                 # aggr = y_own - m ; out_tile = aggr @ W_phi.T
                 aggr = fin.tile([128, CH], F32, tag="aggr")
                 nc.vector.tensor_sub(
                     out=aggr[:], in0=y_own_sb[:, t * 128: (t + 1) * 128], in1=m[:])
                 ptr = ps_c.tile([128, CH], F32, tag="ctr")
                 nc.tensor.transpose(out=ptr[:], in_=aggr[:], identity=ident[:])
                 aggrT = fin.tile([128, CH], F32, tag="aggrT")
                 nc.vector.tensor_copy(out=aggrT[:], in_=ptr[:])
                 po = ps_c.tile([128, CH], F32, tag="cmm")
                 nc.tensor.matmul(out=po[:], lhsT=aggrT[:], rhs=wph_sb[:],
                                  start=True, stop=True)
                 osb = fin.tile([128, CH], F32, tag="osb")
                 nc.scalar.copy(out=osb[:], in_=po[:])
                 nc.sync.dma_start(out=out[t * 128: (t + 1) * 128, :], in_=osb[:])

            if phases != "abc":
                fillz = consts.tile([128, CH], F32)
                nc.vector.tensor_copy(out=fillz[:], in_=probe[:])
                for t in range(T):
                    nc.sync.dma_start(out=out[t * 128: (t + 1) * 128, :], in_=fillz[:])
    nc.compile()
    return nc


_CACHE = {}


def _get_runner_and_plan(x, edge_index, cfg: Cfg, reps=1, phases="abc", exp=None):
    prepped = prep(x, edge_index, cfg)
    plan = prepped[0]
    skey = (cfg, reps, phases, tuple(sorted((exp or {}).items())),
            tuple(plan["B"].reshape(-1).tolist()))
    if skey not in _CACHE:
        nc = build_program(plan, reps=reps, phases=phases, exp=exp)
        _CACHE[skey] = BassRunner(nc, cfg.n_cores)
    return (_CACHE[skey],) + prepped


def run_cfg(x, edge_index, W_theta, W_phi, cfg: Cfg, time_iters=0, reps=1, phases="abc", exp=None):
    (runner, plan, idxw, x_own, own_nodes, deg, x_perm, x_permT,
     x_ownT) = _get_runner_and_plan(
        x, edge_index, cfg, reps=reps, phases=phases, exp=exp)
    if exp and exp.get("sent_idx"):
        idxw = np.full_like(idxw, cfg.chunk_real)
    wtt = np.ascontiguousarray(np.asarray(W_theta, np.float32).T)
    wpt = np.ascontiguousarray(np.asarray(W_phi, np.float32).T)
    wtb = wtt.astype(ml_dtypes.bfloat16)
    in_maps = [
        dict(x_full=x_perm, x_own=x_own[c], w_theta_t=wtt, w_phi_t=wpt,
             x_fullT=x_permT, x_ownT=x_ownT[c], w_theta_b=wtb,
             idxw=np.ascontiguousarray(idxw[c]) if plan["W_total"] > 0
             else np.zeros((128, 16), np.int16))
        for c in range(cfg.n_cores)
    ]
    runner.prepare(in_maps)
    outs = runner.run()
    t_ns = runner.time_ns(iters=time_iters) if time_iters else None
    res = runner.results(outs)
    out_full = np.empty((cfg.N, CH), np.float32)
    for c in range(cfg.n_cores):
        out_full[own_nodes[c]] = res[c]["out"][: cfg.npc]
    out_full[deg == 0] = 0.0
    return out_full, t_ns


def kernel(x, edge_index, W_theta, W_phi):
    out, _ = run_cfg(x, edge_index, W_theta, W_phi, Cfg())
    return out



# revision 32
# speedup vs baseline: 4.4588x; 1.0264x over previous
"""Trainium2 Bass kernel for DevConv-style GNN message passing.

Reference computation:
    rel_t = (x[row] - x[col]) @ W_theta.T          # [E, 128]
    aggr  = segment_max(rel_t, row, N)             # [N, 128], empty -> 0
    out   = aggr @ W_phi.T                         # [N, 128]

Key reformulation: with y = x @ W_theta.T, within a segment (fixed row d)
    max_e (y[d] - y[col_e]) = y[d] - min_e y[col_e]     (per channel)
so the per-edge matmul disappears and only ONE gather per edge (y[col]) is
needed, followed by a segmented min.

The gather is bound by SWDGE descriptor generation on the Pool engine
(one descriptor pair per gathered row, generated by the Q7 core pair that
queue_num selects), so runtime ~ padded-slot count x per-idx gen cost.
What makes it fast (measured on HW, baseline 3.69 ms -> ~1.0-1.8 ms
depending on machine load):
  * queue rotation over all 4 SWDGE queues overlaps descriptor generation
    across the 4 Q7 core pairs (~3x aggregate gen rate);
  * 2048-idx instructions with single_packet=False halve the ~1 us/inst
    fixed cost (single_packet=True crashes the mesh at >=2048 idx);
  * pad slots point at 128 sentinel rows per chunk, not one — a single
    sentinel row serializes 27% of the drain on one HBM bank (+4.4 ms!);
  * balanced chunk coloring (int16 idx -> 4 chunks of 25088 rows) spreads
    each destination's edges ~deg/4 per chunk;
  * tiles sorted by max per-chunk count then count vector: padding
    inflation 1.95x -> 1.16x;
  * chunk-major windowed phase B (v3): per chunk, win_tiles=12 tiles'
    rects are one contiguous SBUF window, so gather instructions span
    tile boundaries at ~full 2048-idx occupancy, and phase A (chunks
    1..3) overlaps with phase-B gathers of earlier chunks.

Per core:
  Phase A: host supplies x^T (bf16, table order); y^T tiles come from
           direct PE matmuls (no transposes), are PE-transposed once and
           stored as bf16 rows into the chunked HBM table y_aug; stores
           issue on the Act HWDGE so phase-B idx loads don't queue behind
           them on SP. y_own is computed into SBUF the same way (no
           transpose needed).
  Phase B: per (chunk, window): dma_gathers fill the window rect; per
           tile a strided 1-input DVE tensor_reduce(min) gives the chunk-
           partial m, min-accumulated across chunks into ping-pong bf16
           accumulators (1-input reduce keeps the DVE out of 2-port perf
           mode, which would lock GpSimd off the shared SBUF port and
           stall SWDGE descriptor generation).
  Phase C: aggr = y_own - m; out_tile = aggr @ W_phi.T via PE transpose
           + matmul, emitted as soon as a tile's last chunk is folded.
Host un-permutes the concatenated core outputs and zeroes empty nodes.
"""
import sys
import os

sys.path.insert(0, "/opt/trn_rl_repo")

from contextlib import ExitStack
from dataclasses import dataclass

import numpy as np
import ml_dtypes

import concourse.bass as bass
import concourse.tile as tile
from concourse import bacc, mybir
from concourse.masks import make_identity

import time

import jax
from jax.sharding import Mesh, PartitionSpec
from jax.experimental.shard_map import shard_map

from concourse.bass2jax import (
    _bass_exec_p, install_neuronx_cc_hook, partition_id_tensor)


class BassRunner:
    """Keeps a jitted PJRT executable for a Bass program so it can be run
    repeatedly on device-resident inputs (for wall-clock timing)."""

    def __init__(self, nc, n_cores: int):
        install_neuronx_cc_hook()
        self.nc = nc
        self.n_cores = n_cores
        partition_name = nc.partition_id_tensor.name if nc.partition_id_tensor else None
        in_names, out_names, out_avals = [], [], []
        for alloc in nc.m.functions[0].allocations:
            if not isinstance(alloc, mybir.MemoryLocationSet):
                continue
            name = alloc.memorylocations[0].name
            if alloc.kind == "ExternalInput":
                if name != partition_name:
                    in_names.append(name)
            elif alloc.kind == "ExternalOutput":
                out_names.append(name)
                out_avals.append(jax.core.ShapedArray(
                    tuple(alloc.tensor_shape), mybir.dt.np(alloc.dtype)))
        self.in_names, self.out_names, self.out_avals = in_names, out_names, out_avals
        self.n_params = len(in_names)
        all_in_names = list(in_names) + list(out_names)
        if partition_name is not None:
            all_in_names.append(partition_name)

        def _body(*args):
            operands = list(args)
            if partition_name is not None:
                operands.append(partition_id_tensor())
            outs = _bass_exec_p.bind(
                *operands,
                out_avals=tuple(out_avals),
                in_names=tuple(all_in_names),
                out_names=tuple(out_names),
                lowering_input_output_aliases=(),
                sim_require_finite=True,
                sim_require_nnan=True,
                nc=nc,
            )
            return tuple(outs)

        devices = jax.devices()[:n_cores]
        self.mesh = Mesh(np.asarray(devices), ("core",))
        n_outs = len(out_names)
        in_specs = (PartitionSpec("core"),) * (self.n_params + n_outs)
        out_specs = (PartitionSpec("core"),) * n_outs
        self.fn = jax.jit(
            shard_map(_body, mesh=self.mesh, in_specs=in_specs,
                      out_specs=out_specs, check_rep=False),
            keep_unused=True,
        )
        self._dev_args = None

    def prepare(self, in_maps):
        assert len(in_maps) == self.n_cores
        concat_in = [
            np.concatenate([np.asarray(in_maps[c][n]) for c in range(self.n_cores)],
                           axis=0)
            for n in self.in_names
        ]
        concat_zeros = [
            np.zeros((self.n_cores * a.shape[0], *a.shape[1:]), a.dtype)
            for a in self.out_avals
        ]
        sharding = jax.sharding.NamedSharding(self.mesh, PartitionSpec("core"))
        self._dev_args = [jax.device_put(v, sharding) for v in concat_in + concat_zeros]
        return self

    def run(self):
        outs = self.fn(*self._dev_args)
        jax.block_until_ready(outs)
        return outs

    def results(self, outs):
        return [
            {n: np.asarray(outs[i]).reshape(self.n_cores, *self.out_avals[i].shape)[c]
             for i, n in enumerate(self.out_names)}
            for c in range(self.n_cores)
        ]

    def time_ns(self, iters=5, warmup=2):
        for _ in range(warmup):
            self.run()
        ts = []
        for _ in range(iters):
            t0 = time.perf_counter()
            self.run()
            ts.append((time.perf_counter() - t0) * 1e9)
        return min(ts)


CH = 128
F32 = mybir.dt.float32
BF16 = mybir.dt.bfloat16
I16 = mybir.dt.int16
SENT_VAL = 3.0e38
IDX_PER_INST = 2048          # >=2048 requires single_packet=False
BLK_PER_INST = IDX_PER_INST // 128


@dataclass(frozen=True)
class Cfg:
    N: int = 100_000
    E: int = 3_200_000
    n_cores: int = 8
    chunk_real: int = 25_088     # table rows per chunk (512-aligned, < int16 max)
    idx_per_inst: int = 2048     # needs single_packet=False (spf) to not crash
    balanced: bool = True        # balanced chunk coloring + count-vector tiling
    n_sent: int = 128            # sentinel rows per chunk (pad reads spread
                                 # over HBM banks instead of hammering one row)
    layout: str = "v3"           # "v3": chunk-major windowed phase B
                                 # "tile": original per-tile rects
    win_tiles: int = 12          # tiles per gather window (v3)

    @property
    def n_chunks(self):
        return (self.N + self.chunk_real - 1) // self.chunk_real

    @property
    def chunk_stride(self):
        return self.chunk_real + self.n_sent

    @property
    def npc(self):
        assert self.N % self.n_cores == 0
        return self.N // self.n_cores

    @property
    def tiles(self):
        return (self.npc + 127) // 128


def _wrap16(seg: np.ndarray) -> np.ndarray:
    """Per-instruction idx wrap: flat [n] -> [128, n//16]; idx i at
    (partition i%16, col i//16), replicated across the 8 gpsimd groups."""
    n = seg.shape[-1]
    w = seg.reshape(*seg.shape[:-1], n // 16, 16)
    w = np.swapaxes(w, -1, -2)                       # [..., 16, n//16]
    return np.tile(w, (1,) * (seg.ndim - 1) + (8, 1))


def _color_chunks(row, col, deg, N, NK, cap):
    """Greedy quota-balanced assignment of cols to NK chunks; returns
    (chunk_of, cnt_dk) where cnt_dk[d,k] = per-dest per-chunk edge count."""
    o = np.argsort(col, kind="stable")
    dest_s = row[o]
    starts = np.searchsorted(col[o], np.arange(N + 1))
    quota = -(-deg // NK)
    cnt_dk = np.zeros((N, NK), np.int32)
    chunk_of = np.full(N, -1, np.int64)
    chunk_sz = np.zeros(NK, np.int64)
    col_order = np.argsort(-(starts[1:] - starts[:-1]), kind="stable")
    for _ in range(2):
        for c in col_order:
            ds = dest_s[starts[c]: starts[c + 1]]
            kprev = chunk_of[c]
            if kprev >= 0:
                if len(ds):
                    cnt_dk[ds, kprev] -= 1
                chunk_sz[kprev] -= 1
            if len(ds) == 0:
                k = int(np.argmin(chunk_sz))
            else:
                cc = cnt_dk[ds]
                over = np.maximum(0, cc + 1 - quota[ds][:, None])
                sc = (over * 1000.0 + cc).sum(axis=0).astype(np.float64)
                sc += chunk_sz * 1e-4
                sc[chunk_sz >= cap] = 1e18
                k = int(np.argmin(sc))
                cnt_dk[ds, k] += 1
            chunk_of[c] = k
            chunk_sz[k] += 1
    return chunk_of, cnt_dk


def prep(x, edge_index, cfg: Cfg):
    """Host-side data prep. Returns (plan, per-core inputs, unpermute info)."""
    N, E, NC = cfg.N, cfg.E, cfg.n_cores
    CR, NK, T = cfg.chunk_real, cfg.n_chunks, cfg.tiles
    row = np.asarray(edge_index[0], dtype=np.int64)
    col = np.asarray(edge_index[1], dtype=np.int64)

    deg = np.bincount(row, minlength=N)
    x_np0 = np.asarray(x, dtype=np.float32)
    if cfg.balanced:
        chunk_of, cnt_dk = _color_chunks(row, col, deg, N, NK, CR)
        # rank within chunk
        oc = np.argsort(chunk_of, kind="stable")
        rank_of = np.empty(N, np.int64)
        csz = np.bincount(chunk_of, minlength=NK)
        cstart = np.concatenate([[0], np.cumsum(csz)])
        rank_of[oc] = np.arange(N) - cstart[chunk_of[oc]]
        # dest order: group by max per-chunk count (the rect width driver),
        # then by the full count vector — tiles get near-identical vectors
        order = np.lexsort(tuple(cnt_dk[:, k] for k in range(NK - 1, -1, -1))
                           + (-cnt_dk.max(axis=1),))
        # x permuted into table layout [NK*CR, CH]
        x_perm = np.zeros((NK * CR, x_np0.shape[1]), np.float32)
        x_perm[chunk_of * CR + rank_of] = x_np0
    else:
        order = np.argsort(-deg, kind="stable")      # node ids by desc degree
        x_perm = x_np0
    core_of = np.empty(N, np.int64)
    pos_of = np.empty(N, np.int64)
    r = np.arange(N)
    core_of[order] = r % NC
    pos_of[order] = r // NC

    ec = core_of[row]
    ep = pos_of[row]
    if cfg.balanced:
        ek = chunk_of[col]
        elocal = rank_of[col].astype(np.int16)
    else:
        ek = col // CR
        elocal = (col - ek * CR).astype(np.int16)
    et = ep // 128
    ed = ep % 128

    # per-(core,tile,chunk,node) counts and within-group slot index j
    key = ((ec * T + et) * NK + ek) * 128 + ed
    o = np.argsort(key, kind="stable")
    ks = key[o]
    first = np.r_[True, ks[1:] != ks[:-1]]
    run_id = np.cumsum(first) - 1
    run_start = np.flatnonzero(first)
    j = np.arange(E) - run_start[run_id]

    cnt = np.bincount(key, minlength=NC * T * NK * 128).reshape(NC, T, NK, 128)
    B = cnt.max(axis=(0, 3)).astype(np.int64)        # [T, NK] shared structure
    B = np.maximum(B, 1)         # every (t,k) rect non-empty (simplifies accum)

    Bf = B.reshape(-1)
    off = np.concatenate([[0], np.cumsum(Bf * 128)])  # slot offset per (t,k)
    total_slots = int(off[-1])

    # pad slots point at one of n_sent sentinel rows (spread over HBM banks)
    pad_vals = (CR + (np.arange(total_slots) % cfg.n_sent)).astype(np.int16)
    idx_all = np.broadcast_to(pad_vals, (NC, total_slots)).copy()
    tk = et[o] * NK + ek[o]
    pos_in = off[tk] + j * 128 + ed[o]
    idx_all[ec[o], pos_in] = elocal[o]

    # split into gather instructions and build wrapped idx input
    blk_per_inst = cfg.idx_per_inst // 128
    insts = []           # (t, k, g0blk, nblk, col_off)
    tile_cols = []       # per tile: (col_start, col_end)
    windows = None
    wsegs = []
    col_off = 0
    if cfg.layout == "v3":
        # chunk-major layout: per chunk, all tiles' rects back-to-back;
        # instructions span tile boundaries within a window of win_tiles.
        Bc = np.ascontiguousarray(B.T)               # [NK, T]
        cb = np.concatenate([[0], np.cumsum(Bc.sum(axis=1) * 128)])
        off3 = np.zeros((NK, T), np.int64)
        for k in range(NK):
            off3[k] = cb[k] + np.concatenate(
                [[0], np.cumsum(Bc[k][:-1] * 128)])
        total3 = int(cb[-1])
        pad_vals = (CR + (np.arange(total3) % cfg.n_sent)).astype(np.int16)
        idx_all = np.broadcast_to(pad_vals, (NC, total3)).copy()
        pos_in = off3[ek[o], et[o]] + j * 128 + ed[o]
        idx_all[ec[o], pos_in] = elocal[o]
        windows = []
        for k in range(NK):
            for t0 in range(0, T, cfg.win_tiles):
                t1 = min(t0 + cfg.win_tiles, T)
                WB = int(Bc[k, t0:t1].sum())
                base_slot = int(off3[k, t0])
                inst_list = []
                for g0 in range(0, WB, blk_per_inst):
                    nb = min(blk_per_inst, WB - g0)
                    seg = idx_all[:, base_slot + g0 * 128:
                                  base_slot + (g0 + nb) * 128]
                    wsegs.append(_wrap16(seg))
                    inst_list.append((g0, nb, col_off))
                    col_off += nb * 128 // 16
                reduces = []
                acc = 0
                for t in range(t0, t1):
                    reduces.append((t, acc, int(Bc[k, t])))
                    acc += int(Bc[k, t])
                windows.append(dict(k=k, WB=WB, insts=inst_list,
                                    reduces=reduces))
    else:
        for t in range(T):
            t_start = col_off
            for k in range(NK):
                btk = int(B[t, k])
                base = int(off[t * NK + k])
                for g0 in range(0, btk, blk_per_inst):
                    nb = min(blk_per_inst, btk - g0)
                    n_i = nb * 128
                    seg = idx_all[:, base + g0 * 128: base + g0 * 128 + n_i]
                    wsegs.append(_wrap16(seg))
                    insts.append((t, k, g0, nb, col_off))
                    col_off += n_i // 16
            tile_cols.append((t_start, col_off))
    idxw = np.concatenate(wsegs, axis=2) if wsegs else np.zeros((NC, 128, 0), np.int16)
    W_total = idxw.shape[2]

    # per-core x_own in pos order, padded to T*128 rows
    own_nodes = np.empty((NC, cfg.npc), np.int64)
    own_nodes[core_of[order], pos_of[order]] = order  # own_nodes[c, p] = node id
    x_np = np.asarray(x, dtype=np.float32)
    x_own = np.zeros((NC, T * 128, CH), np.float32)
    x_own[:, : cfg.npc] = x_np[own_nodes]

    x_permT = np.ascontiguousarray(x_perm.T).astype(ml_dtypes.bfloat16)
    x_ownT = np.ascontiguousarray(np.transpose(x_own, (0, 2, 1))).astype(
        ml_dtypes.bfloat16)
    plan = dict(cfg=cfg, B=B, insts=insts, tile_cols=tile_cols, W_total=W_total,
                windows=windows)
    return plan, idxw, x_own, own_nodes, deg, x_perm, x_permT, x_ownT


def build_program(plan, reps=1, phases="abc", exp=None):
    exp = {**dict(gather="on", folds="on", queue="rot", fold_mode="reduce",
                  a_store="act", spf=1),
           **(exp or {})}
    cfg: Cfg = plan["cfg"]
    N, NK, CR, T = cfg.N, cfg.n_chunks, cfg.chunk_real, cfg.tiles
    CS = cfg.chunk_stride
    B, insts, tile_cols, W_total = (
        plan["B"], plan["insts"], plan["tile_cols"], plan["W_total"])

    windows = plan.get("windows")
    nc = bacc.Bacc(None, target_bir_lowering=False, num_swdge_queues=4,
                   dynamic_dma_scratch_size=int(exp.get("scratch", 16384)))
    NA = NK * CR if cfg.balanced else N      # phase-A row count (table layout)
    axt = windows is not None   # v3 always uses the xT phase A
    if axt:
        x_fullT = nc.declare_dram_parameter("x_fullT", [CH, NA], BF16,
                                            isOutput=False)
        x_ownT = nc.declare_dram_parameter("x_ownT", [CH, T * 128], BF16,
                                           isOutput=False)
        wthb = nc.declare_dram_parameter("w_theta_b", [CH, CH], BF16,
                                         isOutput=False)
    else:
        x_full = nc.declare_dram_parameter("x_full", [NA, CH], F32, isOutput=False)
        x_own = nc.declare_dram_parameter("x_own", [T * 128, CH], F32, isOutput=False)
        wth = nc.declare_dram_parameter("w_theta_t", [CH, CH], F32, isOutput=False)
    wph = nc.declare_dram_parameter("w_phi_t", [CH, CH], F32, isOutput=False)
    idxw = nc.declare_dram_parameter("idxw", [128, max(W_total, 16)], I16, isOutput=False)
    out = nc.declare_dram_parameter("out", [T * 128, CH], F32, isOutput=True)

    qc = [0]  # gather queue rotation

    with tile.TileContext(nc) as tc:
        with ExitStack() as ctx:
            consts = ctx.enter_context(tc.tile_pool(name="consts", bufs=1))
            dram = ctx.enter_context(tc.tile_pool(name="dram", bufs=1, space="DRAM"))
            ax = ctx.enter_context(tc.tile_pool(name="ax", bufs=2))
            axT = ctx.enter_context(tc.tile_pool(name="axT", bufs=2))
            ay = ctx.enter_context(tc.tile_pool(name="ay", bufs=2))
            ps_t = ctx.enter_context(tc.tile_pool(name="ps_t", bufs=2, space="PSUM"))
            ps_y = ctx.enter_context(tc.tile_pool(name="ps_y", bufs=2, space="PSUM"))
            ps_c = ctx.enter_context(tc.tile_pool(name="ps_c", bufs=2, space="PSUM"))
            gidx = ctx.enter_context(tc.tile_pool(name="gidx", bufs=4))
            gdst = ctx.enter_context(tc.tile_pool(name="gdst", bufs=2))
            fold = ctx.enter_context(tc.tile_pool(name="fold", bufs=2))
            fin = ctx.enter_context(tc.tile_pool(name="fin", bufs=2))

            if windows is not None:
                y_augs = [dram.tile([CS, CH], BF16, name=f"yaug{k}")
                          for k in range(NK)]
            else:
                y_aug = dram.tile([NK * CS, CH], BF16)

            ident = consts.tile([128, 128], F32)
            make_identity(nc, ident[:])
            if axt:
                wth_sb = consts.tile([CH, CH], BF16)
                nc.sync.dma_start(out=wth_sb[:], in_=wthb[:])
            else:
                wth_sb = consts.tile([CH, CH], F32)
                nc.sync.dma_start(out=wth_sb[:], in_=wth[:])
            wph_sb = consts.tile([CH, CH], F32)
            nc.sync.dma_start(out=wph_sb[:], in_=wph[:])
            y_own_sb = consts.tile([128, T * 128],
                                   BF16 if windows is not None else F32)
            probe = consts.tile([128, CH], BF16)
            nc.gpsimd.memset(probe[:], 0.0)
            cst128 = consts.tile([128, CH], BF16)
            nc.gpsimd.memset(cst128[:], 1.0)
            sent = consts.tile([cfg.n_sent, CH], BF16)
            nc.gpsimd.memset(sent[:], SENT_VAL)
            for k in range(NK):
                nc.sync.dma_start(
                    out=(y_augs[k][CR: CR + cfg.n_sent, :]
                         if windows is not None else
                         y_aug[k * CS + CR: k * CS + CR + cfg.n_sent, :]),
                    in_=sent[:])

            A_MODE = os.environ.get("A_MODE", "full")
            PS_BUFS = int(os.environ.get("PS_BUFS", "2"))
            # ---------------- Phase A: y_aug = (x @ W_theta.T).bf16 ----------
            def emit_group(src, n0, gn, dst):
                """Process rows [n0, n0+gn) of src -> y into dst.
                dst = ("aug",) writes y_aug rows (with chunk-boundary split),
                dst = ("own",) writes y_own_sb cols."""
                nt = (gn + 127) // 128
                xg = ax.tile([128, nt * 128], F32, tag="xg",
                             bufs=int(os.environ.get("XG_BUFS", "2")))
                xg3 = xg[:].rearrange("p (i c) -> p i c", c=CH)
                load_eng = nc.gpsimd if A_MODE == "dma3" else nc.sync
                store_eng = nc.scalar if exp.get("a_store") == "act" else nc.sync
                if gn % 128 == 0:
                    load_eng.dma_start(
                        out=xg3[:, :nt, :],
                        in_=src[n0: n0 + gn, :].rearrange("(i p) c -> p i c", p=128))
                else:
                    for i in range(nt):
                        rn = min(128, gn - i * 128)
                        nc.sync.dma_start(
                            out=xg3[:rn, i, :],
                            in_=src[n0 + i * 128: n0 + i * 128 + rn, :])
                if A_MODE.startswith("dma") and dst == "aug":
                    # dma : load->store dep, both on sync
                    # dma2: stores only dep-free (loads still emitted)
                    # dma3: load on gpsimd, stores dep on load, on sync
                    # dma5: loads only (no stores)
                    if gn % 128 == 0:
                        for i in range(nt):
                            r0 = n0 + i * 128
                            kb = (r0 // CR) * cfg.n_sent
                            if A_MODE == "dma5":
                                continue
                            src_ap = (cst128[:] if A_MODE == "dma2"
                                      else xg3[:, i, :CH // 2].bitcast(BF16))
                            nc.sync.dma_start(
                                out=y_aug[r0 + kb: r0 + kb + 128, :], in_=src_ap)
                    return
                pt = ps_t.tile([128, nt * 128], F32, tag="pt", bufs=PS_BUFS)
                for i in range(nt):
                    rn = min(128, gn - i * 128)
                    nc.tensor.transpose(
                        out=pt[:, i * 128: i * 128 + rn],
                        in_=xg3[:rn, i, :],
                        identity=ident[:rn, :rn])
                xT = axT.tile([128, nt * 128], F32, tag="xT")
                if exp.get("fold_mode") == "reduce":
                    nc.scalar.copy(out=xT[:, : nt * 128], in_=pt[:, : nt * 128])
                else:
                    nc.vector.tensor_copy(out=xT[:, : nt * 128], in_=pt[:, : nt * 128])
                if A_MODE == "nomm" and dst == "aug":
                    for i in range(nt):
                        r0 = n0 + i * 128
                        kb = (r0 // CR) * cfg.n_sent
                        nc.sync.dma_start(
                            out=y_aug[r0 + kb: r0 + kb + 128, :],
                            in_=xT[:, i * 128: i * 128 + 128][:, :CH // 2].bitcast(BF16))
                    return
                py = ps_y.tile([128, nt * 128], F32, tag="py", bufs=PS_BUFS)
                for i in range(nt):
                    rn = min(128, gn - i * 128)
                    nc.tensor.matmul(
                        out=py[:rn, i * 128: (i + 1) * 128],
                        lhsT=xT[:, i * 128: i * 128 + rn],
                        rhs=wth_sb[:],
                        start=True, stop=True)
                if dst == "own":
                    nc.scalar.copy(
                        out=y_own_sb[:, n0: n0 + nt * 128], in_=py[:, : nt * 128])
                    return
                yg = ay.tile([128, nt * 128], BF16, tag="yg")
                copy2 = nc.vector.tensor_copy if A_MODE == "dvecopy" else nc.scalar.copy
                if gn % 128 == 0:
                    copy2(out=yg[:, : gn], in_=py[:, : gn])
                else:
                    for i in range(nt):
                        rn = min(128, gn - i * 128)
                        copy2(
                            out=yg[:rn, i * 128: (i + 1) * 128],
                            in_=py[:rn, i * 128: (i + 1) * 128])
                yg3 = yg[:].rearrange("p (i c) -> p i c", c=CH)
                # write y rows n -> aug rows n + n // CR, splitting at tile level
                for i in range(nt):
                    r0 = n0 + i * 128
                    rn = min(128, gn - i * 128)
                    kc = r0 // CR
                    ec = (r0 + rn - 1) // CR
                    kb = kc * cfg.n_sent
                    ke = ec * cfg.n_sent
                    if kc == ec:
                        store_eng.dma_start(
                            out=y_aug[r0 + kb: r0 + kb + rn, :], in_=yg3[:rn, i, :])
                    else:
                        split = (kc + 1) * CR - r0       # rows before boundary
                        store_eng.dma_start(
                            out=y_aug[r0 + kb: r0 + kb + split, :],
                            in_=yg3[:split, i, :])
                        store_eng.dma_start(
                            out=y_aug[r0 + split + ke: r0 + ke + rn, :],
                            in_=yg3[split:rn, i, :])

            by_tile = {}
            for (t, k, g0, nb, coff) in insts:
                by_tile.setdefault(t, []).append((k, g0, nb, coff))

            def emit_group_xt(srcT, n0, gn, dst):
                """Transpose-free phase A: srcT is [128 in_ch, nodes] bf16;
                y tile = (xT_tile)^T @ W_theta^T via direct PE matmuls."""
                nt = (gn + 127) // 128
                xg = ax.tile([128, nt * 128], BF16, tag="xgt")
                nc.sync.dma_start(out=xg[:, :gn], in_=srcT[:, n0: n0 + gn])
                py = ps_y.tile([128, nt * 128], F32, tag="pyt", bufs=PS_BUFS)
                for i in range(nt):
                    rn = min(128, gn - i * 128)
                    nc.tensor.matmul(
                        out=py[:rn, i * 128: (i + 1) * 128],
                        lhsT=xg[:, i * 128: i * 128 + rn],
                        rhs=wth_sb[:], start=True, stop=True)
                if dst == "own":
                    nc.scalar.copy(out=y_own_sb[:, n0: n0 + nt * 128],
                                   in_=py[:, : nt * 128])
                    return
                yg = ay.tile([128, nt * 128], BF16, tag="ygt")
                nc.scalar.copy(out=yg[:, : nt * 128], in_=py[:, : nt * 128])
                yg3 = yg[:].rearrange("p (i c) -> p i c", c=CH)
                store_eng = nc.scalar if exp.get("a_store") == "act" else nc.sync
                for i in range(nt):
                    r0 = n0 + i * 128
                    rn = min(128, gn - i * 128)
                    kc = r0 // CR
                    ec = (r0 + rn - 1) // CR
                    lo = r0 - kc * CR
                    if kc == ec:
                        store_eng.dma_start(
                            out=y_augs[kc][lo: lo + rn, :],
                            in_=yg3[:rn, i, :])
                    else:
                        split = CR - lo
                        store_eng.dma_start(
                            out=y_augs[kc][lo: CR, :],
                            in_=yg3[:split, i, :])
                        store_eng.dma_start(
                            out=y_augs[ec][0: rn - split, :],
                            in_=yg3[split:rn, i, :])

            for _rep in range(reps):
              if axt:
                for n0 in range(0, NA, 512):
                    emit_group_xt(x_fullT, n0, min(512, NA - n0), "aug")
                for n0 in range(0, T * 128, 512):
                    emit_group_xt(x_ownT, n0, min(512, T * 128 - n0), "own")
              else:
                for n0 in range(0, NA, 512):
                    emit_group(x_full, n0, min(512, NA - n0), "aug")
                for n0 in range(0, T * 128, 512):
                    emit_group(x_own, n0, min(512, T * 128 - n0), "own")

              if windows is not None:
                 # ---------- Phase B + C, chunk-major windows (v3) ----------
                 maccA = consts.tile([128, T * 128], BF16)
                 maccB = consts.tile([128, T * 128], BF16)

                 def emit_phase_c(t, m_ap):
                     aggr = fin.tile([128, CH], F32, tag="aggr")
                     nc.vector.tensor_sub(
                         out=aggr[:],
                         in0=y_own_sb[:, t * 128: (t + 1) * 128], in1=m_ap)
                     ptr = ps_c.tile([128, CH], F32, tag="ctr")
                     nc.tensor.transpose(out=ptr[:], in_=aggr[:],
                                         identity=ident[:])
                     aggrT = fin.tile([128, CH], F32, tag="aggrT")
                     nc.scalar.copy(out=aggrT[:], in_=ptr[:])
                     po = ps_c.tile([128, CH], F32, tag="cmm")
                     nc.tensor.matmul(out=po[:], lhsT=aggrT[:], rhs=wph_sb[:],
                                      start=True, stop=True)
                     osb = fin.tile([128, CH], F32, tag="osb")
                     nc.scalar.copy(out=osb[:], in_=po[:])
                     nc.sync.dma_start(out=out[t * 128: (t + 1) * 128, :],
                                       in_=osb[:])

                 for w in windows:
                     k, WB = w["k"], w["WB"]
                     wc0 = w["insts"][0][2]
                     wc1 = w["insts"][-1][2] + w["insts"][-1][1] * 128 // 16
                     it = gidx.tile([128, max(wc1 - wc0, 16)], I16, tag="it")
                     nc.sync.dma_start(out=it[:, : wc1 - wc0],
                                       in_=idxw[:, wc0:wc1])
                     win = gdst.tile([128, WB * CH], BF16, tag="win",
                                     bufs=int(exp.get("gbufs", 2)))
                     win3 = win[:].rearrange("p (b c) -> p b c", c=CH)
                     for (g0, nb, coff) in w["insts"]:
                         if exp["gather"] == "seq":
                             r0 = min(g0 * 128, CS - nb * 128)
                             nc.sync.dma_start(
                                 out=win3[:, g0: g0 + nb, :],
                                 in_=y_augs[k][r0: r0 + nb * 128, :].rearrange(
                                     "(i p) c -> p i c", p=128))
                             continue
                         nc.gpsimd.dma_gather(
                             out_ap=win3[:, g0: g0 + nb, :],
                             in_ap=y_augs[k][:, :],
                             idxs_ap=it[:, coff - wc0:
                                        coff - wc0 + nb * 128 // 16],
                             num_idxs=nb * 128,
                             num_idxs_reg=nb * 128,
                             elem_size=CH,
                             single_packet=not bool(exp.get("spf")),
                             queue_num=(qc[0] % 4) if exp["queue"] == "rot"
                             else 0,
                         )
                         qc[0] += 1
                     winT = win[:].rearrange("p (b c) -> p c b", c=CH)
                     for (t, boff, btk) in w["reduces"]:
                         if exp.get("reduce") == "off":
                             sl = winT[:, :, 0:1]
                         else:
                             sl = winT[:, :, boff: boff + btk]
                         if k == 0:
                             nc.vector.tensor_reduce(
                                 out=maccA[:, t * 128: (t + 1) * 128],
                                 in_=sl, axis=mybir.AxisListType.X,
                                 op=mybir.AluOpType.min)
                         else:
                             mk = fin.tile([128, CH], BF16, tag="mk", bufs=2)
                             nc.vector.tensor_reduce(
                                 out=mk[:], in_=sl, axis=mybir.AxisListType.X,
                                 op=mybir.AluOpType.min)
                             src = maccA if k % 2 == 1 else maccB
                             dst = maccB if k % 2 == 1 else maccA
                             nc.vector.tensor_tensor(
                                 out=dst[:, t * 128: (t + 1) * 128],
                                 in0=src[:, t * 128: (t + 1) * 128],
                                 in1=mk[:], op=mybir.AluOpType.min)
                             if k == NK - 1:
                                 emit_phase_c(
                                     t, dst[:, t * 128: (t + 1) * 128])
                 continue

              # ---------------- Phase B + C per tile ---------------------------
              for t in range(T):
                 c0, c1 = tile_cols[t]
                 it = gidx.tile([128, max(c1 - c0, 16)], I16, tag="it")
                 if c1 > c0:
                     nc.sync.dma_start(out=it[:, : c1 - c0], in_=idxw[:, c0:c1])
                 if exp.get("fold_mode") == "reduce":
                     kws = [k for k in range(NK) if int(B[t, k]) > 0]
                     koff = {}
                     wt = 0
                     for k in kws:
                         koff[k] = wt
                         wt += int(B[t, k])
                     if wt > 0:
                         dk = gdst.tile([128, wt * CH], BF16, tag="gr",
                                        bufs=int(exp.get("gbufs", 3)))
                         dk3 = dk[:].rearrange("p (b c) -> p b c", c=CH)
                     for (k, g0, nb, coff) in by_tile.get(t, []):
                         n_i = nb * 128
                         col0 = koff[k] + g0
                         if exp["gather"] == "seq":
                             r0 = min(k * CS + g0 * 128, NK * CS - n_i)
                             nc.sync.dma_start(
                                 out=dk3[:, col0: col0 + nb, :],
                                 in_=y_aug[r0: r0 + n_i, :].rearrange(
                                     "(i p) c -> p i c", p=128))
                             continue
                         nc.gpsimd.dma_gather(
                             out_ap=dk3[:, col0: col0 + nb, :],
                             in_ap=y_aug[k * CS: (k + 1) * CS, :],
                             idxs_ap=it[:, coff - c0: coff - c0 + n_i // 16],
                             num_idxs=n_i,
                             num_idxs_reg=n_i,
                             elem_size=CH,
                             queue_num=(qc[0] % 4) if exp["queue"] == "rot" else 0,
                         )
                         qc[0] += 1
                     m = fin.tile([128, CH], F32, tag="m")
                     if wt == 0 or exp.get("reduce") == "off":
                         nc.gpsimd.memset(m[:], SENT_VAL)
                     else:
                         dkT = dk[:].rearrange("p (b c) -> p c b", c=CH)
                         nc.vector.tensor_reduce(
                             out=m[:], in_=dkT, axis=mybir.AxisListType.X,
                             op=mybir.AluOpType.min)
                     aggr = fin.tile([128, CH], F32, tag="aggr")
                     nc.vector.tensor_sub(
                         out=aggr[:], in0=y_own_sb[:, t * 128: (t + 1) * 128],
                         in1=m[:])
                     ptr = ps_c.tile([128, CH], F32, tag="ctr")
                     nc.tensor.transpose(out=ptr[:], in_=aggr[:], identity=ident[:])
                     aggrT = fin.tile([128, CH], F32, tag="aggrT")
                     nc.scalar.copy(out=aggrT[:], in_=ptr[:])
                     po = ps_c.tile([128, CH], F32, tag="cmm")
                     nc.tensor.matmul(out=po[:], lhsT=aggrT[:], rhs=wph_sb[:],
                                      start=True, stop=True)
                     osb = fin.tile([128, CH], F32, tag="osb")
                     nc.scalar.copy(out=osb[:], in_=po[:])
                     nc.sync.dma_start(out=out[t * 128: (t + 1) * 128, :],
                                       in_=osb[:])
                     continue
                 dks = {}
                 for k in range(NK):
                     btk = int(B[t, k])
                     if btk == 0:
                         continue
                     dks[k] = gdst.tile([128, btk * CH], BF16, tag=f"g{k}", name=f"dk{k}")
                 for (k, g0, nb, coff) in by_tile.get(t, []):
                     if exp["gather"] == "off":
                         break
                     dk3 = dks[k][:].rearrange("p (b c) -> p b c", c=CH)
                     n_i = nb * 128
                     ndup = 2 if exp["gather"] == "dup" else 1
                     for di in range(ndup):
                         if di == 0:
                             dst = dk3[:, g0: g0 + nb, :]
                         else:
                             ddup = gdst.tile(
                                 [128, (cfg.idx_per_inst // 128) * CH], BF16,
                                 tag="gdup", bufs=2)
                             dst = ddup[:].rearrange(
                                 "p (b c) -> p b c", c=CH)[:, :nb, :]
                         nc.gpsimd.dma_gather(
                             out_ap=dst,
                             in_ap=y_aug[k * CS: (k + 1) * CS, :],
                             idxs_ap=it[:, coff - c0: coff - c0 + n_i // 16],
                             num_idxs=n_i,
                             num_idxs_reg=n_i,
                             elem_size=CH,
                             queue_num=(qc[0] % 4) if exp["queue"] == "rot" else 0,
                         )
                         qc[0] += 1
                 # fold each chunk's rect down to one [128, CH] min
                 mks = []
                 if exp["folds"] == "off":
                     m = fin.tile([128, CH], F32, tag="m")
                     nc.gpsimd.memset(m[:], SENT_VAL)
                     aggr = fin.tile([128, CH], F32, tag="aggr")
                     nc.vector.tensor_sub(
                         out=aggr[:], in0=y_own_sb[:, t * 128: (t + 1) * 128],
                         in1=m[:])
                     ptr = ps_c.tile([128, CH], F32, tag="ctr")
                     nc.tensor.transpose(out=ptr[:], in_=aggr[:], identity=ident[:])
                     aggrT = fin.tile([128, CH], F32, tag="aggrT")
                     nc.vector.tensor_copy(out=aggrT[:], in_=ptr[:])
                     po = ps_c.tile([128, CH], F32, tag="cmm")
                     nc.tensor.matmul(out=po[:], lhsT=aggrT[:], rhs=wph_sb[:],
                                      start=True, stop=True)
                     osb = fin.tile([128, CH], F32, tag="osb")
                     nc.scalar.copy(out=osb[:], in_=po[:])
                     nc.sync.dma_start(out=out[t * 128: (t + 1) * 128, :], in_=osb[:])
                     continue
                 for k in range(NK):
                     if k not in dks:
                         continue
                     cur = dks[k]
                     nb = int(B[t, k])
                     while nb > 1:
                         half = (nb + 1) // 2
                         nxt = fold.tile([128, half * CH], BF16, tag=f"f{k}", bufs=3)
                         nc.vector.tensor_tensor(
                             out=nxt[:, : half * CH],
                             in0=cur[:, : half * CH],
                             in1=cur[:, (nb - half) * CH: nb * CH],
                             op=mybir.AluOpType.min)
                         cur, nb = nxt, half
                     mks.append(cur)
                 m = fin.tile([128, CH], F32, tag="m")
                 if len(mks) == 0:
                     nc.gpsimd.memset(m[:], SENT_VAL)
                 elif len(mks) == 1:
                     nc.vector.tensor_copy(out=m[:], in_=mks[0][:, :CH])
                 else:
                     # sequential accumulate with alternating tags (max 2 live)
                     acc = mks[0]
                     for i in range(1, len(mks) - 1):
                         mm = fold.tile([128, CH], BF16, tag=f"mrg{i % 2}")
                         nc.vector.tensor_tensor(
                             out=mm[:], in0=acc[:, :CH], in1=mks[i][:, :CH],
                             op=mybir.AluOpType.min)
                         acc = mm
                     nc.vector.tensor_tensor(
                         out=m[:], in0=acc[:, :CH], in1=mks[-1][:, :CH],
                         op=mybir.AluOpType.min)
                 # aggr = y_own - m ; out_tile = aggr @ W_phi.T
                 aggr = fin.tile([128, CH], F32, tag="aggr")
                 nc.vector.tensor_sub(
                     out=aggr[:], in0=y_own_sb[:, t * 128: (t + 1) * 128], in1=m[:])
                 ptr = ps_c.tile([128, CH], F32, tag="ctr")
                 nc.tensor.transpose(out=ptr[:], in_=aggr[:], identity=ident[:])
                 aggrT = fin.tile([128, CH], F32, tag="aggrT")
                 nc.vector.tensor_copy(out=aggrT[:], in_=ptr[:])
                 po = ps_c.tile([128, CH], F32, tag="cmm")
                 nc.tensor.matmul(out=po[:], lhsT=aggrT[:], rhs=wph_sb[:],
                                  start=True, stop=True)
                 osb = fin.tile([128, CH], F32, tag="osb")
                 nc.scalar.copy(out=osb[:], in_=po[:])
                 nc.sync.dma_start(out=out[t * 128: (t + 1) * 128, :], in_=osb[:])

            if phases != "abc":
                fillz = consts.tile([128, CH], F32)
                nc.vector.tensor_copy(out=fillz[:], in_=probe[:])
                for t in range(T):
                    nc.sync.dma_start(out=out[t * 128: (t + 1) * 128, :], in_=fillz[:])
    nc.compile()
    return nc


_CACHE = {}


def _get_runner_and_plan(x, edge_index, cfg: Cfg, reps=1, phases="abc", exp=None):
    prepped = prep(x, edge_index, cfg)
    plan = prepped[0]
    skey = (cfg, reps, phases, tuple(sorted((exp or {}).items())),
            tuple(plan["B"].reshape(-1).tolist()))
    if skey not in _CACHE:
        nc = build_program(plan, reps=reps, phases=phases, exp=exp)
        _CACHE[skey] = BassRunner(nc, cfg.n_cores)
    return (_CACHE[skey],) + prepped


def run_cfg(x, edge_index, W_theta, W_phi, cfg: Cfg, time_iters=0, reps=1, phases="abc", exp=None):
    (runner, plan, idxw, x_own, own_nodes, deg, x_perm, x_permT,
     x_ownT) = _get_runner_and_plan(
        x, edge_index, cfg, reps=reps, phases=phases, exp=exp)
    if exp and exp.get("sent_idx"):
        idxw = np.full_like(idxw, cfg.chunk_real)
    wtt = np.ascontiguousarray(np.asarray(W_theta, np.float32).T)
    wpt = np.ascontiguousarray(np.asarray(W_phi, np.float32).T)
    wtb = wtt.astype(ml_dtypes.bfloat16)
    in_maps = [
        dict(x_full=x_perm, x_own=x_own[c], w_theta_t=wtt, w_phi_t=wpt,
             x_fullT=x_permT, x_ownT=x_ownT[c], w_theta_b=wtb,
             idxw=np.ascontiguousarray(idxw[c]) if plan["W_total"] > 0
             else np.zeros((128, 16), np.int16))
        for c in range(cfg.n_cores)
    ]
    runner.prepare(in_maps)
    outs = runner.run()
    t_ns = runner.time_ns(iters=time_iters) if time_iters else None
    res = runner.results(outs)
    out_full = np.empty((cfg.N, CH), np.float32)
    for c in range(cfg.n_cores):
        out_full[own_nodes[c]] = res[c]["out"][: cfg.npc]
    out_full[deg == 0] = 0.0
    return out_full, t_ns


def kernel(x, edge_index, W_theta, W_phi):
    out, _ = run_cfg(x, edge_index, W_theta, W_phi, Cfg())
    return out

